# revision 11
# baseline (speedup 1.0000x reference)
"""AttentiveFP forward on 8 Trainium2 NeuronCores (Bass/Tile).

Sharding: 2048 graphs (nodes contiguous, batch sorted) split into 8 blocks of
256 graphs; each core owns the edges whose dst node falls in its block. Per
round each core computes its nodes' features, all-gathers a compact per-node
table [xt | alpha_src] (fp16, 65 wide), expands it locally to 256B-aligned
rows, then fetches per-edge src rows with nc.gpsimd.dma_gather (the token
gather ucode: thousands of int16 indices per call, round-robined over 4 SWDGE
queues). Indices are int16, so slots are grouped per (node-tile-pair,
32768-row source window); within a call, tile-a slots carry rel in [0,128)
and tile-b slots rel in [128,256), so one 256-wide is_equal one-hot serves
both tiles' PSUM segment-matmuls and the alpha_dst select (one-hot x
replicated alpha row, reduced on DVE). Per-edge alpha_dst needs no gather
(dst is always local). Node phases run feature-major, half-packed, with
block-diagonal [128,128] fp16 weights; GRU hidden state stays fp32. The gate
round's edge-attr term (W1b @ e_attr) is precomputed on the host per slot.
Readout uses a 256-wide graph one-hot per tile-pair plus a replicated
per-graph alpha row (no gathers).

Softmax max-subtraction is skipped (logits O(1), shift-invariant).
"""
import sys
sys.path.insert(0, '/opt/trn_rl_repo')
sys.path.insert(0, '/root/.axon_site')

import numpy as np

F16 = np.float16
NC = 8
D = 64
G_TOT = 2048
G_LOC = G_TOT // NC
F_IN = 25
E_DIM = 4
NEG = 0.01
P = 128
W = 65            # compact table row: [xt(64) | alpha_src]
WG = 128          # gathered row width (256B-aligned)
WIN = 32768       # int16 index window (rows)


def _prep(x, edge_index, edge_attr, batch):
    src = edge_index[0].astype(np.int64)
    dst = edge_index[1].astype(np.int64)
    batch = batch.astype(np.int64)

    gstart = np.searchsorted(batch, np.arange(0, G_TOT + 1, G_LOC))
    n0 = gstart[:-1]
    nloc = np.diff(gstart)
    n_pad = int(np.ceil((nloc.max() + 1) / 256) * 256)
    t_tiles = n_pad // P
    HC = n_pad // 2
    HT = t_tiles // 2
    NW = (NC * n_pad + WIN - 1) // WIN

    def pi_row(n):
        h = n // HC
        r = n % HC
        return (2 * (r // P) + h) * P + (r % P)

    src_dev = np.searchsorted(gstart[1:], src, side='right')
    dst_dev = np.searchsorted(gstart[1:], dst, side='right')
    gidx_all = src_dev * n_pad + pi_row(src - n0[src_dev])

    # ---- pass 1: bucket edges per core into (pair, window, half) ----
    buckets = [[[[None, None] for _ in range(NW)] for _ in range(HT)]
               for _ in range(NC)]
    for c in range(NC):
        sel = np.where(dst_dev == c)[0]
        dl = dst[sel] - n0[c]
        j_dst = 2 * ((dl % HC) // P) + dl // HC
        p_dst = dl % P
        gi = gidx_all[sel]
        w_of = gi // WIN
        for tp in range(HT):
            for h in range(2):
                m = j_dst == 2 * tp + h
                gi_m, p_m, w_m, sel_m = gi[m], p_dst[m], w_of[m], sel[m]
                for w in range(NW):
                    mm = w_m == w
                    buckets[c][tp][w][h] = (gi_m[mm] - w * WIN, p_m[mm], sel_m[mm])
    # ---- pass 2: SPMD-uniform call metadata (max counts over cores) ----
    meta = []       # per pair: [w, ncols, acols, bcol0, amax, bmax]
    for tp in range(HT):
        calls = []
        for w in range(NW):
            amax = max(len(buckets[c][tp][w][0][0]) for c in range(NC))
            bmax = max(len(buckets[c][tp][w][1][0]) for c in range(NC))
            if amax + bmax == 0:
                continue
            ncols = (amax + bmax + P - 1) // P
            calls.append([w, ncols, (amax + P - 1) // P, amax // P, amax, bmax])
        if not any(cl[4] for cl in calls):
            calls.insert(0, [0, 1, 1, 0, P, 0])
        if not any(cl[5] for cl in calls):
            calls.append([0, 1, 0, 0, 0, P])
        meta.append(calls)
    TOTC = sum(cl[1] for calls in meta for cl in calls)

    per = []
    for c in range(NC):
        idx16 = np.zeros((16, TOTC * 8), np.int16)
        rel = np.full((P, TOTC), 300.0, np.float32)
        attr_s = np.zeros((TOTC * P, E_DIM), np.float32)
        col0 = 0
        for tp in range(HT):
            for w_, ncols, acols, bcol0, amax, bmax in meta[tp]:
                flat_idx = np.zeros(ncols * P, np.int16)
                flat_rel = np.full(ncols * P, 300.0, np.float32)
                flat_attr = np.zeros((ncols * P, E_DIM), np.float32)
                pos = 0
                for h, hmax in ((0, amax), (1, bmax)):
                    gi_l, p_l, sel_l = buckets[c][tp][w_][h]
                    k = len(gi_l)
                    flat_idx[pos:pos + k] = gi_l.astype(np.int16)
                    flat_rel[pos:pos + k] = p_l + h * P
                    flat_attr[pos:pos + k] = edge_attr[sel_l]
                    pos += hmax
                idx16[:, col0 * 8:(col0 + ncols) * 8] = \
                    flat_idx.reshape(ncols * 8, 16).T
                rel[:, col0:col0 + ncols] = flat_rel.reshape(ncols, P).T
                attr_s[col0 * P:(col0 + ncols) * P] = flat_attr
                col0 += ncols
        per.append(dict(idx16=np.tile(idx16, (8, 1)),
                        rel=rel.astype(F16), attr_s=attr_s))
        nl = int(nloc[c])
        gl = batch[n0[c]:n0[c] + nl] - G_LOC * c
        grel = np.full((P, t_tiles), 300.0, np.float32)
        n_ids = np.arange(n_pad)
        h_a = n_ids // HC
        j_a = 2 * ((n_ids % HC) // P) + h_a
        p_a = n_ids % P
        valid = n_ids < nl
        grel[p_a[valid], j_a[valid]] = gl[n_ids[valid]]
        per[c]['grel'] = np.ascontiguousarray(grel.astype(F16))
        xp = np.zeros((n_pad, F_IN), np.float32)
        xp[:nl] = x[n0[c]:n0[c] + nl]
        xfm = np.zeros((P, HC), np.float32)
        xfm[:F_IN] = xp[:HC].T
        xfm[D:D + F_IN] = xp[HC:].T
        per[c]['xfm'] = xfm.astype(F16)
    return per, n_pad, t_tiles, meta, TOTC


def _mk_weights(kw):
    w = {}
    def bd(a):
        t = a.T
        z = np.zeros((P, P), np.float32)
        z[0:D, 0:D] = t
        z[D:2 * D, D:2 * D] = t
        return z
    def col(a):
        return np.concatenate([a, a])[:, None]
    def rep2(a):
        return np.tile(np.concatenate([a, a])[None, :], (P, 1))
    def rep1(a):
        return np.tile(a[None, :], (P, 1))
    def gb(a):
        t = a.reshape(3, D).T
        return np.concatenate([t, t], 0)
    def gru_bd(wg):
        out = np.zeros((P, 3 * P), np.float32)
        for g in range(3):
            out[:, g * P:(g + 1) * P] = bd(wg[g * D:(g + 1) * D])
        return out

    B, F = 'b', 'f'
    lin1 = np.zeros((P, P), np.float32)
    lin1[0:F_IN, 0:D] = kw["lin1_w"].T
    lin1[D:D + F_IN, D:2 * D] = kw["lin1_w"].T
    w["lin1_bd"] = (lin1, B)
    w["lin1_b"] = (col(kw["lin1_b"]), F)
    w["gate_w1a_bd"] = (bd(kw["gate_lin1_w"][:, :D]), B)
    w["attlRep"] = (rep1(kw["gate_att_l"]), B)
    w["gateattrRep2"] = (rep2(kw["gate_att_r"]), B)
    w["gate_w2_bd"] = (bd(kw["gate_lin2_w"]), B)
    w["gate_bias"] = (col(kw["gate_bias"]), F)
    w["gru0_wih"] = (gru_bd(kw["gru0_wih"]), B)
    w["gru0_whh"] = (gru_bd(kw["gru0_whh"]), B)
    w["gru0_bih"] = (gb(kw["gru0_bih"]), F)
    w["gru0_bhh"] = (gb(kw["gru0_bhh"]), F)
    w["gru0_bsum"] = (gb(kw["gru0_bih"] + kw["gru0_bhh"]), F)
    for l in range(4):
        pre = f"at{l}_"
        w[pre + "wT"] = (bd(kw["atom_lin_w"][l]), B)
        w[pre + "srcRep2"] = (rep2(kw["atom_att_src"][l]), B)
        w[pre + "dstRep2"] = (rep2(kw["atom_att_dst"][l]), B)
        w[pre + "bias"] = (col(kw["atom_bias"][l]), F)
        w[pre + "gru_wih"] = (gru_bd(kw["atom_gru_wih"][l]), B)
        w[pre + "gru_whh"] = (gru_bd(kw["atom_gru_whh"][l]), B)
        w[pre + "gru_bih"] = (gb(kw["atom_gru_bih"][l]), F)
        w[pre + "gru_bhh"] = (gb(kw["atom_gru_bhh"][l]), F)
        w[pre + "gru_bsum"] = (gb(kw["atom_gru_bih"][l] + kw["atom_gru_bhh"][l]), F)
    w["mol_bd"] = (bd(kw["mol_lin_w"]), B)
    w["mol_wT32"] = (kw["mol_lin_w"].T.copy(), F)
    w["molsrcRep2"] = (rep2(kw["mol_att_src"]), B)
    w["moldstCol"] = (kw["mol_att_dst"][:, None].copy(), F)
    w["mol_biasRep"] = (rep1(kw["mol_bias"]), F)
    w["mol_gru_wih"] = (kw["mol_gru_wih"].T.copy(), F)
    w["mol_gru_whh"] = (kw["mol_gru_whh"].T.copy(), F)
    w["mol_gru_bih"] = (gb(kw["mol_gru_bih"])[:D], F)
    w["mol_gru_bhh"] = (gb(kw["mol_gru_bhh"])[:D], F)
    w["mol_gru_bsum"] = (gb(kw["mol_gru_bih"] + kw["mol_gru_bhh"])[:D], F)
    w["lin2_wT"] = (kw["lin2_w"].T.copy(), F)
    w["lin2_b"] = (kw["lin2_b"][:, None].copy(), F)
    out = {}
    for k, (v, tag) in w.items():
        v = np.ascontiguousarray(v, np.float32)
        out[k] = v.astype(F16) if tag == B else v
    return out


def _build(n_pad, t_tiles, meta, TOTC, wmeta):
    import concourse.bacc as bacc
    import concourse.mybir as mybir
    import concourse.tile as tile
    from concourse.masks import make_identity

    dt = mybir.dt
    AF = mybir.ActivationFunctionType
    OP = mybir.AluOpType
    AX = mybir.AxisListType
    BF = dt.float16
    F32 = dt.float32

    HC = n_pad // 2
    HT = t_tiles // 2
    NCH = (HC + 511) // 512
    MXC = max(sum(cl[1] for cl in calls) for calls in meta)
    NW = (NC * n_pad + WIN - 1) // WIN

    nc = bacc.Bacc("TRN2", target_bir_lowering=False, debug=False, num_devices=NC,
                   num_swdge_queues=4)

    xfm_h = nc.dram_tensor("xfm", [P, HC], BF, kind="ExternalInput")
    idx_h = nc.dram_tensor("idx16", [P, TOTC * 8], dt.int16, kind="ExternalInput")
    rel_h = nc.dram_tensor("rel", [P, TOTC], BF, kind="ExternalInput")
    be_h = nc.dram_tensor("b_e", [P, TOTC * D], BF, kind="ExternalInput")
    grel_h = nc.dram_tensor("grel", [P, t_tiles], BF, kind="ExternalInput")
    iota_h = nc.dram_tensor("iota256", [P, 2 * P], BF, kind="ExternalInput")
    cst_h = {k: nc.dram_tensor("w_" + k, list(s_), BF if isbf else F32,
                               kind="ExternalInput")
             for k, (s_, isbf) in wmeta.items()}
    y_out = nc.dram_tensor("y", [1, G_LOC], F32, kind="ExternalOutput")

    with tile.TileContext(nc) as tc:
      with (
        tc.tile_pool(name="cst", bufs=1) as cst,
        tc.tile_pool(name="st", bufs=1) as st,
        tc.tile_pool(name="ep", bufs=2) as ep,
        tc.tile_pool(name="sp", bufs=2) as sp,
        tc.tile_pool(name="ps", bufs=2, space="PSUM") as ps,
        tc.tile_pool(name="ps_seg", bufs=2, space="PSUM") as ps_seg,
        tc.tile_pool(name="ps_big", bufs=2, space="PSUM") as ps_big,
        tc.tile_pool(name="dram", bufs=1, space="DRAM") as dp,
      ):
        def load(name):
            h = cst_h[name]
            t = cst.tile(list(h.shape), h.dtype, name="c_" + name)
            nc.sync.dma_start(out=t[:], in_=h[:])
            return t
        Wt = {k: load(k) for k in cst_h}
        idx_sb = cst.tile([P, TOTC * 8], dt.int16, name="idx_sb")
        nc.sync.dma_start(out=idx_sb[:], in_=idx_h[:])
        rel_sb = cst.tile([P, TOTC], BF, name="rel_sb")
        nc.sync.dma_start(out=rel_sb[:], in_=rel_h[:])
        grel_sb = cst.tile([P, t_tiles], BF, name="grel_sb")
        nc.sync.dma_start(out=grel_sb[:], in_=grel_h[:])
        iota_sb = cst.tile([P, 2 * P], BF, name="iota_sb")
        nc.sync.dma_start(out=iota_sb[:], in_=iota_h[:])
        identb = cst.tile([P, P], BF, name="identb")
        make_identity(nc, identb[:])
        ident = cst.tile([P, P], F32, name="ident")
        make_identity(nc, ident[:])
        ones1 = cst.tile([1, P], BF, name="ones1")
        nc.vector.memset(ones1[:], 1.0)
        onesf = cst.tile([P, P], BF, name="onesf")
        nc.vector.memset(onesf[:], 1.0)

        XC = st.tile([P, HC], F32, name="XC")
        XCb = st.tile([P, HC], BF, name="XCb")
        HXb = st.tile([P, HC], BF, name="HXb")
        ad_nm = st.tile([P, t_tiles], F32, name="ad_nm")
        adTs = st.tile([P, P], BF, name="adTs")
        row_all = st.tile([P, HT * 2 * W], BF, name="row_all")
        tbl_locs = [dp.tile([n_pad, W], BF, name=f"tbl_loc{i}") for i in range(5)]
        tbl_alls = [dp.tile([NC * n_pad, W], BF, addr_space="Shared",
                            name=f"tbl_all{i}") for i in range(5)]
        tbl_wides = [dp.tile([NC * n_pad, WG], BF, name=f"tbl_wide{i}")
                     for i in range(5)]

        def mm_node(dst, wkey, srcb, act=AF.Copy, bias=None, alpha=0.0):
            for ci in range(NCH):
                c0, c1 = ci * 512, min((ci + 1) * 512, HC)
                pt = ps_big.tile([P, 512], F32, name="mmp", tag="mmp")
                nc.tensor.matmul(pt[:, 0:c1 - c0], Wt[wkey][:], srcb[:, c0:c1],
                                 start=True, stop=True)
                b = Wt[bias][:, 0:1] if bias else 0.0
                nc.scalar.activation(dst[:, c0:c1], pt[:, 0:c1 - c0],
                                     act, bias=b, alpha=alpha)

        def elu_inplace(t_fm, bias):
            for ci in range(NCH):
                c0, c1 = ci * 512, min((ci + 1) * 512, HC)
                w_ = c1 - c0
                v = t_fm[:, c0:c1]
                tin = sp.tile([P, 512], F32, name="eluin", tag="eluin")
                nc.vector.tensor_scalar(out=tin[:, 0:w_], in0=v,
                                        scalar1=Wt[bias][:, 0:1],
                                        scalar2=None, op0=OP.add)
                r = sp.tile([P, 512], F32, name="elur", tag="elur")
                nc.scalar.activation(r[:, 0:w_], tin[:, 0:w_], AF.Relu)
                nc.vector.tensor_sub(tin[:, 0:w_], tin[:, 0:w_], r[:, 0:w_])
                nc.scalar.activation(tin[:, 0:w_], tin[:, 0:w_], AF.Exp)
                nc.vector.scalar_tensor_tensor(
                    out=v, in0=tin[:, 0:w_], scalar=-1.0,
                    in1=r[:, 0:w_], op0=OP.add, op1=OP.add)

        def gru_relu(pre):
            wih = Wt[pre + "wih"]
            whh = Wt[pre + "whh"]
            bs = Wt[pre + "bsum"]
            for ci in range(NCH):
                c0, c1 = ci * 512, min((ci + 1) * 512, HC)
                w_ = c1 - c0
                def gate2(g):
                    pt = ps_big.tile([P, 512], F32, name="grup", tag="mmp")
                    nc.tensor.matmul(pt[:, 0:w_], wih[:, g * P:(g + 1) * P],
                                     HXb[:, c0:c1], start=True, stop=False)
                    nc.tensor.matmul(pt[:, 0:w_], whh[:, g * P:(g + 1) * P],
                                     XCb[:, c0:c1], start=False, stop=True)
                    return pt
                pr = gate2(0)
                r = sp.tile([P, 512], F32, name="grur", tag="grur")
                nc.scalar.activation(r[:, 0:w_], pr[:, 0:w_], AF.Sigmoid,
                                     bias=bs[:, 0:1])
                pz = gate2(1)
                z = sp.tile([P, 512], F32, name="gruz", tag="gruz")
                nc.scalar.activation(z[:, 0:w_], pz[:, 0:w_], AF.Sigmoid,
                                     bias=bs[:, 1:2])
                pin = ps_big.tile([P, 512], F32, name="grupi", tag="mmp")
                nc.tensor.matmul(pin[:, 0:w_], wih[:, 2 * P:3 * P],
                                 HXb[:, c0:c1], start=True, stop=True)
                phn = ps_big.tile([P, 512], F32, name="gruph", tag="mmp")
                nc.tensor.matmul(phn[:, 0:w_], whh[:, 2 * P:3 * P],
                                 XCb[:, c0:c1], start=True, stop=True)
                hn = sp.tile([P, 512], F32, name="gruhn", tag="gruhn", bufs=1)
                nc.vector.tensor_scalar(out=hn[:, 0:w_], in0=phn[:, 0:w_],
                                        scalar1=Wt[pre + "bhh"][:, 2:3],
                                        scalar2=None, op0=OP.add)
                nc.vector.tensor_mul(hn[:, 0:w_], hn[:, 0:w_], r[:, 0:w_])
                nc.vector.tensor_tensor(out=hn[:, 0:w_], in0=hn[:, 0:w_],
                                        in1=pin[:, 0:w_], op=OP.add)
                n_t = sp.tile([P, 512], F32, name="grun", tag="grun", bufs=1)
                nc.scalar.activation(n_t[:, 0:w_], hn[:, 0:w_], AF.Tanh,
                                     bias=Wt[pre + "bih"][:, 2:3])
                d_t = sp.tile([P, 512], F32, name="grud", tag="grud", bufs=1)
                nc.vector.tensor_sub(d_t[:, 0:w_], XC[:, c0:c1], n_t[:, 0:w_])
                nc.vector.tensor_mul(d_t[:, 0:w_], d_t[:, 0:w_], z[:, 0:w_])
                nc.vector.tensor_tensor(out=d_t[:, 0:w_], in0=d_t[:, 0:w_],
                                        in1=n_t[:, 0:w_], op=OP.add)
                nc.scalar.activation(XC[:, c0:c1], d_t[:, 0:w_], AF.Relu)

        def build_table(srcb, srcRep2, dstRep2, ad_from_xc, li):
            for tp in range(HT):
                cc = tp * P
                pT = ps.tile([P, P], BF, name="tabT", tag="miscP")
                nc.tensor.transpose(out=pT[:], in_=srcb[:, cc:cc + P],
                                    identity=identb[:])
                row3 = row_all[:, tp * 2 * W:(tp + 1) * 2 * W].rearrange(
                    "p (h w) -> p h w", w=W)
                nc.vector.tensor_copy(
                    out=row3[:, :, 0:D],
                    in_=pT[:].rearrange("q (h f) -> q h f", f=D))
                if srcRep2 is None:
                    nc.vector.memset(row3[:, :, D:W], 0.0)
                else:
                    tmp = sp.tile([P, P], F32, name="tabm", tag="tabm")
                    nc.vector.tensor_tensor(out=tmp[:], in0=pT[:],
                                            in1=Wt[srcRep2][:], op=OP.mult)
                    asr = sp.tile([P, 2], F32, name="asr", tag="asr")
                    nc.vector.tensor_reduce(asr[:].unsqueeze(2),
                                            tmp[:].rearrange("q (h f) -> q h f", f=D),
                                            axis=AX.X, op=OP.add)
                    nc.vector.tensor_copy(out=row3[:, :, D:W],
                                          in_=asr[:].unsqueeze(2))
                if ad_from_xc:
                    pTx = ps.tile([P, P], BF, name="tabTx", tag="miscP")
                    nc.tensor.transpose(out=pTx[:], in_=XCb[:, cc:cc + P],
                                        identity=identb[:])
                    dsrc = pTx
                else:
                    dsrc = pT
                tmp2 = sp.tile([P, P], F32, name="tabm2", tag="tabm2")
                nc.vector.tensor_tensor(out=tmp2[:], in0=dsrc[:],
                                        in1=Wt[dstRep2][:], op=OP.mult)
                nc.vector.tensor_reduce(ad_nm[:, 2 * tp:2 * tp + 2].unsqueeze(2),
                                        tmp2[:].rearrange("q (h f) -> q h f", f=D),
                                        axis=AX.X, op=OP.add)
            nc.sync.dma_start(
                out=tbl_locs[li][:].rearrange("(tp h p) w -> p tp h w", h=2, p=P),
                in_=row_all[:])
            # alpha_dst transposed: adTs[j, q] = ad of node (tile j, row q)
            pAd = ps.tile([P, P], F32, name="adT", tag="miscP")
            nc.tensor.transpose(out=pAd[0:t_tiles, :], in_=ad_nm[:],
                                identity=ident[:])
            nc.vector.tensor_copy(out=adTs[0:t_tiles, :], in_=pAd[0:t_tiles, :])
            nc.gpsimd.collective_compute(
                "AllGather", mybir.AluOpType.bypass,
                replica_groups=[list(range(NC))],
                ins=[tbl_locs[li].opt()], outs=[tbl_alls[li].opt()])
            for w_ in range(NW):
                r0 = w_ * WIN
                r1 = min((w_ + 1) * WIN, NC * n_pad)
                nc.sync.dma_start(out=tbl_wides[li][r0:r1, 0:W],
                                  in_=tbl_alls[li][r0:r1, :])

        def edge_phase(is_gate, li):
            """Message round; writes agg (normalized, fp16) into HXb (fm)."""
            col0 = 0
            qrr = 0
            for tp in range(HT):
                calls = meta[tp]
                TC = sum(cl[1] for cl in calls)
                ar2 = sp.tile([1, 2 * P], BF, name="ar2", tag="ar2")
                nc.sync.dma_start(out=ar2[0:1, :], in_=adTs[2 * tp:2 * tp + 2, :])
                prp = ps.tile([P, 2 * P], F32, name="repP", tag="rowP")
                nc.tensor.matmul(prp[:], ones1[:], ar2[0:1, :],
                                 start=True, stop=True)
                rep_pair = sp.tile([P, 2 * P], BF, name="rep_pair", tag="rep_pair")
                nc.vector.tensor_copy(out=rep_pair[:], in_=prp[:])
                gt = ep.tile([P, MXC * WG], BF, name="gt", tag="gt", bufs=2)
                cc = 0
                for w_, ncols, acols, bcol0, amax, bmax in calls:
                    r0 = w_ * WIN
                    r1 = min((w_ + 1) * WIN, NC * n_pad)
                    nc.gpsimd.dma_gather(
                        out_ap=gt[:, cc * WG:(cc + ncols) * WG].rearrange(
                            "p (c w) -> p c w", w=WG),
                        in_ap=tbl_wides[li][r0:r1, :],
                        idxs_ap=idx_sb[:, (col0 + cc) * 8:(col0 + cc + ncols) * 8],
                        num_idxs=ncols * P, num_idxs_reg=ncols * P,
                        elem_size=WG, queue_num=qrr % 4)
                    qrr += 1
                    cc += ncols
                gt3 = gt[:].rearrange("p (c w) -> p c w", w=WG)
                s256 = ep.tile([P, MXC * 2 * P], BF, name="s256", tag="s256")
                nc.vector.tensor_tensor(
                    out=s256[:, :TC * 2 * P],
                    in0=rel_sb[:, col0:col0 + TC].unsqueeze(2).to_broadcast(
                        [P, TC, 2 * P]),
                    in1=iota_sb[:].unsqueeze(1).to_broadcast([P, TC, 2 * P]),
                    op=OP.is_equal)
                sat = ep.tile([P, MXC * 2 * P], BF, name="sat", tag="sat", bufs=1)
                nc.vector.tensor_tensor(
                    out=sat[:, :TC * 2 * P],
                    in0=s256[:, :TC * 2 * P],
                    in1=rep_pair[:].unsqueeze(1).to_broadcast([P, TC, 2 * P]),
                    op=OP.mult)
                aslot = sp.tile([P, MXC], F32, name="aslot", tag="aslot")
                nc.vector.tensor_reduce(
                    aslot[:, 0:TC].unsqueeze(2),
                    sat[:, :TC * 2 * P].rearrange("p (c q) -> p c q", q=2 * P),
                    axis=AX.X, op=OP.add)
                q = sp.tile([P, MXC], F32, name="q", tag="q")
                if is_gate:
                    be_sb = ep.tile([P, MXC * D], BF, name="be_sb", tag="be_sb", bufs=1)
                    nc.sync.dma_start(out=be_sb[:, 0:TC * D],
                                      in_=be_h[:, col0 * D:(col0 + TC) * D])
                    m_all = ep.tile([P, MXC * D], BF, name="m_all", tag="m_all")
                    m3 = m_all[:].rearrange("p (c w) -> p c w", w=D)
                    nc.vector.tensor_tensor(
                        out=m3[:, 0:TC, :], in0=gt3[:, 0:TC, 0:D],
                        in1=be_sb[:, 0:TC * D].rearrange("p (c w) -> p c w", w=D),
                        op=OP.add)
                    nc.scalar.activation(m_all[:, :TC * D], m_all[:, :TC * D],
                                         AF.Prelu, alpha=NEG)
                    lt = ep.tile([P, MXC * D], BF, name="lt", tag="lt", bufs=1)
                    nc.vector.tensor_tensor(
                        out=lt[:, :TC * D], in0=m3[:, 0:TC, :],
                        in1=Wt["attlRep"][:].unsqueeze(1).to_broadcast([P, TC, D]),
                        op=OP.mult)
                    nc.vector.tensor_reduce(q[:, 0:TC].unsqueeze(2),
                                            lt[:, :TC * D].rearrange(
                                                "p (c w) -> p c w", w=D),
                                            axis=AX.X, op=OP.add)
                    nc.vector.tensor_tensor(out=q[:, 0:TC], in0=q[:, 0:TC],
                                            in1=aslot[:, 0:TC], op=OP.add)
                    msg3 = m3
                else:
                    nc.vector.tensor_tensor(out=q[:, 0:TC],
                                            in0=gt3[:, 0:TC, D:D + 1].squeeze(2),
                                            in1=aslot[:, 0:TC], op=OP.add)
                    msg3 = gt3
                e_t = sp.tile([P, MXC], F32, name="e_t", tag="e_t")
                nc.scalar.activation(e_t[:, 0:TC], q[:, 0:TC], AF.Prelu, alpha=NEG)
                nc.scalar.activation(e_t[:, 0:TC], e_t[:, 0:TC], AF.Exp)
                rhs = ep.tile([P, MXC * W], BF, name="rhs", tag="rhs")
                r3 = rhs[:].rearrange("p (c w) -> p c w", w=W)
                nc.vector.tensor_tensor(
                    out=r3[:, 0:TC, 0:D], in0=msg3[:, 0:TC, 0:D],
                    in1=e_t[:, 0:TC].unsqueeze(2).to_broadcast([P, TC, D]),
                    op=OP.mult)
                nc.vector.tensor_copy(out=r3[:, 0:TC, D:W],
                                      in_=e_t[:, 0:TC].unsqueeze(2))
                amms = []
                bmms = []
                cc = 0
                for w_, ncols, acols, bcol0, amax, bmax in calls:
                    if amax:
                        amms += [cc + c for c in range(acols)]
                    if bmax:
                        bmms += [cc + c for c in range(bcol0, ncols)]
                    cc += ncols
                aggb = sp.tile([P, P], BF, name="aggb", tag="aggb")
                for h, mms, qofs in ((0, amms, 0), (1, bmms, P)):
                    pseg = ps_seg.tile([P, W], F32, name="pseg", tag="pseg")
                    for i, c in enumerate(mms):
                        nc.tensor.matmul(
                            pseg[:],
                            s256[:, c * 2 * P + qofs:c * 2 * P + qofs + P],
                            rhs[:, c * W:(c + 1) * W],
                            start=(i == 0), stop=(i == len(mms) - 1))
                    sn = sp.tile([P, 1], F32, name="sn", tag="sn")
                    nc.vector.tensor_single_scalar(out=sn[:], in_=pseg[:, D:W],
                                                   scalar=1e-16, op=OP.add)
                    rcp = sp.tile([P, 1], F32, name="rcp", tag="rcp")
                    nc.vector.reciprocal(rcp[:], sn[:])
                    nc.vector.tensor_tensor(out=aggb[:, h * D:(h + 1) * D],
                                            in0=pseg[:, 0:D],
                                            in1=rcp[:].to_broadcast([P, D]),
                                            op=OP.mult)
                pT2 = ps.tile([P, P], F32, name="aggT", tag="miscP")
                nc.tensor.matmul(pT2[:], aggb[:], identb[:], start=True, stop=True)
                nc.vector.tensor_copy(out=HXb[:, tp * P:(tp + 1) * P], in_=pT2[:])
                col0 += TC

        # ================= forward =================
        for ci in range(NCH):
            c0, c1 = ci * 512, min((ci + 1) * 512, HC)
            xin = sp.tile([P, 512], BF, name="xin", tag="xin")
            nc.sync.dma_start(out=xin[:, :c1 - c0], in_=xfm_h[:, c0:c1])
            pt = ps_big.tile([P, 512], F32, name="mmp0", tag="mmp")
            nc.tensor.matmul(pt[:, 0:c1 - c0], Wt["lin1_bd"][:],
                             xin[:, 0:c1 - c0], start=True, stop=True)
            nc.scalar.activation(XC[:, c0:c1], pt[:, 0:c1 - c0],
                                 AF.Prelu, bias=Wt["lin1_b"][:, 0:1], alpha=NEG)
        nc.vector.tensor_copy(out=XCb[:], in_=XC[:])
        # --- GATEConv ---
        mm_node(HXb, "gate_w1a_bd", XCb)
        build_table(HXb, None, "gateattrRep2", True, 0)
        edge_phase(True, 0)
        mm_node(HXb, "gate_w2_bd", HXb)
        elu_inplace(HXb, "gate_bias")
        gru_relu("gru0_")
        nc.vector.tensor_copy(out=XCb[:], in_=XC[:])
        # --- atom layers ---
        for l in range(4):
            pre = f"at{l}_"
            mm_node(HXb, pre + "wT", XCb)
            build_table(HXb, pre + "srcRep2", pre + "dstRep2", False, 1 + l)
            edge_phase(False, 1 + l)
            elu_inplace(HXb, pre + "bias")
            gru_relu(pre + "gru_")
            nc.vector.tensor_copy(out=XCb[:], in_=XC[:])

        # ================= readout =================
        mm_node(HXb, "mol_bd", XCb)          # xs into HXb
        asrc_nm = st.tile([P, t_tiles], F32, name="asrc_nm")
        for tp in range(HT):
            pT = ps.tile([P, P], BF, name="xsT", tag="miscP")
            nc.tensor.transpose(out=pT[:], in_=HXb[:, tp * P:(tp + 1) * P],
                                identity=identb[:])
            nc.vector.tensor_copy(out=row_all[:, tp * 2 * D:(tp + 1) * 2 * D],
                                  in_=pT[:])
            tmp = sp.tile([P, P], F32, name="xsm", tag="tabm")
            nc.vector.tensor_tensor(out=tmp[:], in0=pT[:],
                                    in1=Wt["molsrcRep2"][:], op=OP.mult)
            nc.vector.tensor_reduce(asrc_nm[:, 2 * tp:2 * tp + 2].unsqueeze(2),
                                    tmp[:].rearrange("q (h f) -> q h f", f=D),
                                    axis=AX.X, op=OP.add)
        for tp in range(HT):
            pT = ps.tile([P, P], BF, name="xcT", tag="miscP")
            nc.tensor.transpose(out=pT[:], in_=XCb[:, tp * P:(tp + 1) * P],
                                identity=identb[:])
            nc.vector.tensor_copy(out=XCb[:, tp * P:(tp + 1) * P], in_=pT[:])
        xc_nm = XCb
        xs_nm = row_all
        OUTT = st.tile([D, 2 * P], F32, name="OUTT")
        for k in range(2):
            pg = ps_seg.tile([P, D], F32, name="pg", tag="pseg")
            for tp in range(HT):
                sgp = sp.tile([P, 2 * 2 * P], BF, name="sgp", tag="sgp")
                nc.vector.tensor_tensor(
                    out=sgp[:],
                    in0=grel_sb[:, 2 * tp:2 * tp + 2].unsqueeze(2).to_broadcast(
                        [P, 2, 2 * P]),
                    in1=iota_sb[:].unsqueeze(1).to_broadcast([P, 2, 2 * P]),
                    op=OP.is_equal)
                for h in range(2):
                    j = 2 * tp + h
                    nc.tensor.matmul(
                        pg[:],
                        sgp[:, h * 2 * P + k * P:h * 2 * P + (k + 1) * P],
                        xc_nm[:, j * D:(j + 1) * D],
                        start=(j == 0), stop=(j == t_tiles - 1))
            og = sp.tile([P, D], F32, name="og", tag="og")
            nc.scalar.activation(og[:], pg[:], AF.Relu)
            pTo = ps.tile([D, P], F32, name="ogT", tag="miscP")
            nc.tensor.transpose(out=pTo[:], in_=og[:], identity=ident[:])
            nc.vector.tensor_copy(out=OUTT[:, k * P:(k + 1) * P], in_=pTo[:])
        HG = st.tile([D, 2 * P], F32, name="HG")
        for ts in range(3):
            pxd = ps_big.tile([D, 512], F32, name="xdp", tag="mmp")
            nc.tensor.matmul(pxd[:, 0:2 * P], Wt["mol_wT32"][:], OUTT[:],
                             start=True, stop=True)
            xds = sp.tile([D, 2 * P], F32, name="xds", tag="xds", bufs=1)
            nc.vector.tensor_copy(out=xds[:], in_=pxd[:, 0:2 * P])
            pag = ps.tile([1, 2 * P], F32, name="agp", tag="rowP")
            nc.tensor.matmul(pag[:], Wt["moldstCol"][:], xds[:],
                             start=True, stop=True)
            agr = sp.tile([1, 2 * P], BF, name="agr", tag="agr")
            nc.vector.tensor_copy(out=agr[:], in_=pag[:])
            prg = ps.tile([P, 2 * P], F32, name="repG", tag="rowP")
            nc.tensor.matmul(prg[:], ones1[:], agr[:], start=True, stop=True)
            rep_agr = sp.tile([P, 2 * P], F32, name="rep_agr", tag="rep_agr", bufs=1)
            nc.vector.tensor_copy(out=rep_agr[:], in_=prg[:])
            ag2 = sp.tile([P, 2], F32, name="ag2", tag="ag2")
            psg = [ps_seg.tile([P, W], F32, name=f"psg{k}", tag="pseg")
                   for k in range(2)]
            rh = ep.tile([P, t_tiles * W], BF, name="rh", tag="sat", bufs=1)
            for tp in range(HT):
                sgp = sp.tile([P, 2 * 2 * P], BF, name="sgp2", tag="sgp")
                nc.vector.tensor_tensor(
                    out=sgp[:],
                    in0=grel_sb[:, 2 * tp:2 * tp + 2].unsqueeze(2).to_broadcast(
                        [P, 2, 2 * P]),
                    in1=iota_sb[:].unsqueeze(1).to_broadcast([P, 2, 2 * P]),
                    op=OP.is_equal)
                sag = sp.tile([P, 2 * 2 * P], BF, name="sag", tag="sag")
                nc.vector.tensor_tensor(
                    out=sag[:].rearrange("p (h q) -> p h q", q=2 * P),
                    in0=sgp[:].rearrange("p (h q) -> p h q", q=2 * P),
                    in1=rep_agr[:].unsqueeze(1).to_broadcast([P, 2, 2 * P]),
                    op=OP.mult)
                nc.vector.tensor_reduce(
                    ag2[:].unsqueeze(2),
                    sag[:].rearrange("p (h q) -> p h q", q=2 * P),
                    axis=AX.X, op=OP.add)
                q2 = sp.tile([P, 2], F32, name="q2", tag="q2")
                nc.vector.tensor_tensor(out=q2[:], in0=asrc_nm[:, 2 * tp:2 * tp + 2],
                                        in1=ag2[:], op=OP.add)
                nc.scalar.activation(q2[:], q2[:], AF.Prelu, alpha=NEG)
                nc.scalar.activation(q2[:], q2[:], AF.Exp)
                rh3 = rh[:, tp * 2 * W:(tp + 1) * 2 * W].rearrange(
                    "p (h w) -> p h w", w=W)
                nc.vector.tensor_tensor(
                    out=rh3[:, :, 0:D],
                    in0=xs_nm[:, tp * 2 * D:(tp + 1) * 2 * D].rearrange(
                        "p (h f) -> p h f", f=D),
                    in1=q2[:].unsqueeze(2).to_broadcast([P, 2, D]),
                    op=OP.mult)
                nc.vector.tensor_copy(out=rh3[:, :, D:W], in_=q2[:].unsqueeze(2))
                for h in range(2):
                    j = 2 * tp + h
                    for k in range(2):
                        nc.tensor.matmul(
                            psg[k][:],
                            sgp[:, h * 2 * P + k * P:h * 2 * P + (k + 1) * P],
                            rh[:, j * W:(j + 1) * W],
                            start=(j == 0), stop=(j == t_tiles - 1))
            for k in range(2):
                sn = sp.tile([P, 1], F32, name="sng", tag="sn")
                nc.vector.tensor_single_scalar(out=sn[:], in_=psg[k][:, D:W],
                                               scalar=1e-16, op=OP.add)
                rcp = sp.tile([P, 1], F32, name="rcpg", tag="rcp")
                nc.vector.reciprocal(rcp[:], sn[:])
                aggg = sp.tile([P, D], F32, name="aggg", tag="aggg")
                nc.vector.tensor_tensor(out=aggg[:], in0=psg[k][:, 0:D],
                                        in1=rcp[:].to_broadcast([P, D]), op=OP.mult)
                nc.vector.tensor_tensor(out=aggg[:], in0=aggg[:],
                                        in1=Wt["mol_biasRep"][:], op=OP.add)
                r = sp.tile([P, D], F32, name="rg", tag="rg")
                nc.scalar.activation(r[:], aggg[:], AF.Relu)
                xm = sp.tile([P, D], F32, name="xmg", tag="xmg")
                nc.vector.tensor_sub(xm[:], aggg[:], r[:])
                nc.scalar.activation(xm[:], xm[:], AF.Exp)
                nc.vector.scalar_tensor_tensor(out=aggg[:], in0=xm[:], scalar=-1.0,
                                               in1=r[:], op0=OP.add, op1=OP.add)
                pTh = ps.tile([D, P], F32, name="hgT", tag="miscP")
                nc.tensor.transpose(out=pTh[:], in_=aggg[:], identity=ident[:])
                nc.vector.tensor_copy(out=HG[:, k * P:(k + 1) * P], in_=pTh[:])
            wih = Wt["mol_gru_wih"]
            whh = Wt["mol_gru_whh"]
            bs = Wt["mol_gru_bsum"]
            def gate2g(g):
                pt = ps_big.tile([D, 512], F32, name="ggp", tag="mmp")
                nc.tensor.matmul(pt[:, 0:2 * P], wih[:, g * D:(g + 1) * D], HG[:],
                                 start=True, stop=False)
                nc.tensor.matmul(pt[:, 0:2 * P], whh[:, g * D:(g + 1) * D], OUTT[:],
                                 start=False, stop=True)
                return pt
            prg2 = gate2g(0)
            rg2 = sp.tile([D, 2 * P], F32, name="ggr", tag="ggr", bufs=1)
            nc.scalar.activation(rg2[:], prg2[:, 0:2 * P], AF.Sigmoid, bias=bs[:, 0:1])
            pzg = gate2g(1)
            zg = sp.tile([D, 2 * P], F32, name="ggz", tag="ggz", bufs=1)
            nc.scalar.activation(zg[:], pzg[:, 0:2 * P], AF.Sigmoid, bias=bs[:, 1:2])
            pig = ps_big.tile([D, 512], F32, name="ggpi", tag="mmp")
            nc.tensor.matmul(pig[:, 0:2 * P], wih[:, 2 * D:3 * D], HG[:],
                             start=True, stop=True)
            phg = ps_big.tile([D, 512], F32, name="ggph", tag="mmp")
            nc.tensor.matmul(phg[:, 0:2 * P], whh[:, 2 * D:3 * D], OUTT[:],
                             start=True, stop=True)
            hng = sp.tile([D, 2 * P], F32, name="gghn", tag="gghn", bufs=1)
            nc.vector.tensor_scalar(out=hng[:], in0=phg[:, 0:2 * P],
                                    scalar1=Wt["mol_gru_bhh"][:, 2:3],
                                    scalar2=None, op0=OP.add)
            nc.vector.tensor_mul(hng[:], hng[:], rg2[:])
            nc.vector.tensor_tensor(out=hng[:], in0=hng[:], in1=pig[:, 0:2 * P],
                                    op=OP.add)
            ng = sp.tile([D, 2 * P], F32, name="ggn", tag="ggn", bufs=1)
            nc.scalar.activation(ng[:], hng[:], AF.Tanh,
                                 bias=Wt["mol_gru_bih"][:, 2:3])
            dg = sp.tile([D, 2 * P], F32, name="ggd", tag="ggd", bufs=1)
            nc.vector.tensor_sub(dg[:], OUTT[:], ng[:])
            nc.vector.tensor_mul(dg[:], dg[:], zg[:])
            nc.vector.tensor_tensor(out=dg[:], in0=dg[:], in1=ng[:], op=OP.add)
            nc.scalar.activation(OUTT[:], dg[:], AF.Relu)
        py = ps.tile([1, 2 * P], F32, name="py", tag="rowP")
        nc.tensor.matmul(py[:], Wt["lin2_wT"][:], OUTT[:], start=True, stop=True)
        ysb = sp.tile([1, 2 * P], F32, name="ysb", tag="ysb")
        nc.vector.tensor_scalar(out=ysb[:], in0=py[:], scalar1=Wt["lin2_b"][0:1, 0:1],
                                scalar2=None, op0=OP.add)
        nc.sync.dma_start(out=y_out[:], in_=ysb[0:1, 0:G_LOC])
    nc.compile()
    return nc


_CACHE = {}


def kernel(**inputs):
    from concourse.bass_utils import run_bass_kernel_spmd
    x = np.asarray(inputs["x"], np.float32)
    ei = np.asarray(inputs["edge_index"])
    ea = np.asarray(inputs["edge_attr"], np.float32)
    bt = np.asarray(inputs["batch"])
    per, n_pad, t_tiles, meta, TOTC = _prep(x, ei, ea, bt)
    kwf = {k: np.asarray(v, np.float32) for k, v in inputs.items()
           if k not in ("x", "edge_index", "edge_attr", "batch")}
    weights = _mk_weights(kwf)
    key = (n_pad, TOTC, tuple(tuple(tuple(cl) for cl in calls) for calls in meta))
    if key not in _CACHE:
        _CACHE[key] = _build(n_pad, t_tiles, meta, TOTC,
                             {k: (v.shape, v.dtype == F16)
                              for k, v in weights.items()})
    nc = _CACHE[key]
    iota = np.tile(np.arange(2 * P).astype(np.float32)[None, :], (P, 1)).astype(F16)
    w1b = kwf["gate_lin1_w"][:, D:]
    in_maps = []
    for c in range(NC):
        b_e = (per[c]["attr_s"] @ w1b.T).astype(F16)
        b_e = np.ascontiguousarray(
            b_e.reshape(TOTC, P, D).transpose(1, 0, 2).reshape(P, TOTC * D))
        m = dict(xfm=per[c]["xfm"], idx16=per[c]["idx16"], rel=per[c]["rel"],
                 b_e=b_e, grel=per[c]["grel"], iota256=iota)
        for k, v in weights.items():
            m["w_" + k] = v
        in_maps.append(m)
    res = run_bass_kernel_spmd(nc, in_maps, core_ids=list(range(NC)))
    return np.concatenate([res.results[c]["y"][0] for c in range(NC)]).astype(np.float32)


# revision 14
# speedup vs baseline: 1.2833x; 1.2833x over previous
"""AttentiveFP forward on 8 Trainium2 NeuronCores (Bass/Tile).

Sharding: 2048 graphs (nodes contiguous, batch sorted) split into 8 blocks of
256 graphs; each core owns the edges whose dst node falls in its block. Per
round each core computes its nodes' features, all-gathers a compact per-node
table [xt | alpha_src] (fp16, 65 wide), expands it locally to 256B-aligned
rows, then fetches per-edge src rows with nc.gpsimd.dma_gather (the token
gather ucode: thousands of int16 indices per call, round-robined over 4 SWDGE
queues). Indices are int16, so slots are grouped per (node-tile-pair,
32768-row source window); within a call, tile-a slots carry rel in [0,128)
and tile-b slots rel in [128,256), so one 256-wide is_equal one-hot serves
both tiles' PSUM segment-matmuls and the alpha_dst select (one-hot x
replicated alpha row, reduced on DVE). Per-edge alpha_dst needs no gather
(dst is always local). Node phases run feature-major, half-packed, with
block-diagonal [128,128] fp16 weights; GRU hidden state stays fp32. The gate
round's edge-attr term (W1b @ e_attr) is precomputed on the host per slot.
Readout uses a 256-wide graph one-hot per tile-pair plus a replicated
per-graph alpha row (no gathers).

Softmax max-subtraction is skipped (logits O(1), shift-invariant).
"""
import sys
sys.path.insert(0, '/opt/trn_rl_repo')
sys.path.insert(0, '/root/.axon_site')

import numpy as np

F16 = np.float16
NC = 8
D = 64
G_TOT = 2048
G_LOC = G_TOT // NC
F_IN = 25
E_DIM = 4
NEG = 0.01
P = 128
W = 65            # compact table row: [xt(64) | alpha_src]
WG = 128          # gathered row width (256B-aligned)
WIN = 32768       # int16 index window (rows)


def _prep(x, edge_index, edge_attr, batch):
    src = edge_index[0].astype(np.int64)
    dst = edge_index[1].astype(np.int64)
    batch = batch.astype(np.int64)

    gstart = np.searchsorted(batch, np.arange(0, G_TOT + 1, G_LOC))
    n0 = gstart[:-1]
    nloc = np.diff(gstart)
    n_pad = int(np.ceil((nloc.max() + 1) / 256) * 256)
    t_tiles = n_pad // P
    HC = n_pad // 2
    HT = t_tiles // 2
    NW = (NC * n_pad + WIN - 1) // WIN

    def pi_row(n):
        h = n // HC
        r = n % HC
        return (2 * (r // P) + h) * P + (r % P)

    src_dev = np.searchsorted(gstart[1:], src, side='right')
    dst_dev = np.searchsorted(gstart[1:], dst, side='right')
    gidx_all = src_dev * n_pad + pi_row(src - n0[src_dev])

    # ---- pass 1: bucket edges per core into (pair, window, half) ----
    buckets = [[[[None, None] for _ in range(NW)] for _ in range(HT)]
               for _ in range(NC)]
    for c in range(NC):
        sel = np.where(dst_dev == c)[0]
        dl = dst[sel] - n0[c]
        j_dst = 2 * ((dl % HC) // P) + dl // HC
        p_dst = dl % P
        gi = gidx_all[sel]
        w_of = gi // WIN
        for tp in range(HT):
            for h in range(2):
                m = j_dst == 2 * tp + h
                gi_m, p_m, w_m, sel_m = gi[m], p_dst[m], w_of[m], sel[m]
                for w in range(NW):
                    mm = w_m == w
                    buckets[c][tp][w][h] = (gi_m[mm] - w * WIN, p_m[mm], sel_m[mm])
    # ---- pass 2: SPMD-uniform call metadata (max counts over cores) ----
    meta = []       # per pair: [w, ncols, acols, bcol0, amax, bmax]
    for tp in range(HT):
        calls = []
        for w in range(NW):
            amax = max(len(buckets[c][tp][w][0][0]) for c in range(NC))
            bmax = max(len(buckets[c][tp][w][1][0]) for c in range(NC))
            if amax + bmax == 0:
                continue
            ncols = (amax + bmax + P - 1) // P
            calls.append([w, ncols, (amax + P - 1) // P, amax // P, amax, bmax])
        if not any(cl[4] for cl in calls):
            calls.insert(0, [0, 1, 1, 0, P, 0])
        if not any(cl[5] for cl in calls):
            calls.append([0, 1, 0, 0, 0, P])
        meta.append(calls)
    TOTC = sum(cl[1] for calls in meta for cl in calls)

    per = []
    for c in range(NC):
        idx16 = np.zeros((16, TOTC * 8), np.int16)
        rel = np.full((P, TOTC), 300.0, np.float32)
        attr_s = np.zeros((TOTC * P, E_DIM), np.float32)
        col0 = 0
        for tp in range(HT):
            for w_, ncols, acols, bcol0, amax, bmax in meta[tp]:
                flat_idx = np.zeros(ncols * P, np.int16)
                flat_rel = np.full(ncols * P, 300.0, np.float32)
                flat_attr = np.zeros((ncols * P, E_DIM), np.float32)
                pos = 0
                for h, hmax in ((0, amax), (1, bmax)):
                    gi_l, p_l, sel_l = buckets[c][tp][w_][h]
                    k = len(gi_l)
                    flat_idx[pos:pos + k] = gi_l.astype(np.int16)
                    flat_rel[pos:pos + k] = p_l + h * P
                    flat_attr[pos:pos + k] = edge_attr[sel_l]
                    pos += hmax
                idx16[:, col0 * 8:(col0 + ncols) * 8] = \
                    flat_idx.reshape(ncols * 8, 16).T
                rel[:, col0:col0 + ncols] = flat_rel.reshape(ncols, P).T
                attr_s[col0 * P:(col0 + ncols) * P] = flat_attr
                col0 += ncols
        s256 = (rel.astype(np.int32)[:, :, None] ==
                np.arange(2 * P, dtype=np.int32)[None, None, :]).astype(F16)
        per.append(dict(idx16=np.tile(idx16, (8, 1)),
                        s256=np.ascontiguousarray(s256.reshape(P, TOTC * 2 * P)),
                        attr_s=attr_s))
        nl = int(nloc[c])
        gl = batch[n0[c]:n0[c] + nl] - G_LOC * c
        grel = np.full((P, t_tiles), 300.0, np.float32)
        n_ids = np.arange(n_pad)
        h_a = n_ids // HC
        j_a = 2 * ((n_ids % HC) // P) + h_a
        p_a = n_ids % P
        valid = n_ids < nl
        grel[p_a[valid], j_a[valid]] = gl[n_ids[valid]]
        per[c]['grel'] = np.ascontiguousarray(grel.astype(F16))
        xp = np.zeros((n_pad, F_IN), np.float32)
        xp[:nl] = x[n0[c]:n0[c] + nl]
        xfm = np.zeros((P, HC), np.float32)
        xfm[:F_IN] = xp[:HC].T
        xfm[D:D + F_IN] = xp[HC:].T
        per[c]['xfm'] = xfm.astype(F16)
    return per, n_pad, t_tiles, meta, TOTC


def _mk_weights(kw):
    w = {}
    def bd(a):
        t = a.T
        z = np.zeros((P, P), np.float32)
        z[0:D, 0:D] = t
        z[D:2 * D, D:2 * D] = t
        return z
    def col(a):
        return np.concatenate([a, a])[:, None]
    def rep2(a):
        return np.tile(np.concatenate([a, a])[None, :], (P, 1))
    def rep1(a):
        return np.tile(a[None, :], (P, 1))
    def gb(a):
        t = a.reshape(3, D).T
        return np.concatenate([t, t], 0)
    def gru_bd(wg):
        out = np.zeros((P, 3 * P), np.float32)
        for g in range(3):
            out[:, g * P:(g + 1) * P] = bd(wg[g * D:(g + 1) * D])
        return out

    B, F = 'b', 'f'
    lin1 = np.zeros((P, P), np.float32)
    lin1[0:F_IN, 0:D] = kw["lin1_w"].T
    lin1[D:D + F_IN, D:2 * D] = kw["lin1_w"].T
    w["lin1_bd"] = (lin1, B)
    w["lin1_b"] = (col(kw["lin1_b"]), F)
    w["gate_w1a_bd"] = (bd(kw["gate_lin1_w"][:, :D]), B)
    w["attlRep"] = (rep1(kw["gate_att_l"]), B)
    w["gateattrRep2"] = (rep2(kw["gate_att_r"]), B)
    w["gate_w2_bd"] = (bd(kw["gate_lin2_w"]), B)
    w["gate_bias"] = (col(kw["gate_bias"]), F)
    w["gru0_wih"] = (gru_bd(kw["gru0_wih"]), B)
    w["gru0_whh"] = (gru_bd(kw["gru0_whh"]), B)
    w["gru0_bih"] = (gb(kw["gru0_bih"]), F)
    w["gru0_bhh"] = (gb(kw["gru0_bhh"]), F)
    w["gru0_bsum"] = (gb(kw["gru0_bih"] + kw["gru0_bhh"]), F)
    for l in range(4):
        pre = f"at{l}_"
        w[pre + "wT"] = (bd(kw["atom_lin_w"][l]), B)
        w[pre + "srcRep2"] = (rep2(kw["atom_att_src"][l]), B)
        w[pre + "dstRep2"] = (rep2(kw["atom_att_dst"][l]), B)
        w[pre + "bias"] = (col(kw["atom_bias"][l]), F)
        w[pre + "gru_wih"] = (gru_bd(kw["atom_gru_wih"][l]), B)
        w[pre + "gru_whh"] = (gru_bd(kw["atom_gru_whh"][l]), B)
        w[pre + "gru_bih"] = (gb(kw["atom_gru_bih"][l]), F)
        w[pre + "gru_bhh"] = (gb(kw["atom_gru_bhh"][l]), F)
        w[pre + "gru_bsum"] = (gb(kw["atom_gru_bih"][l] + kw["atom_gru_bhh"][l]), F)
    w["mol_bd"] = (bd(kw["mol_lin_w"]), B)
    w["mol_wT32"] = (kw["mol_lin_w"].T.copy(), F)
    w["molsrcRep2"] = (rep2(kw["mol_att_src"]), B)
    w["moldstCol"] = (kw["mol_att_dst"][:, None].copy(), F)
    w["mol_biasRep"] = (rep1(kw["mol_bias"]), F)
    w["mol_gru_wih"] = (kw["mol_gru_wih"].T.copy(), F)
    w["mol_gru_whh"] = (kw["mol_gru_whh"].T.copy(), F)
    w["mol_gru_bih"] = (gb(kw["mol_gru_bih"])[:D], F)
    w["mol_gru_bhh"] = (gb(kw["mol_gru_bhh"])[:D], F)
    w["mol_gru_bsum"] = (gb(kw["mol_gru_bih"] + kw["mol_gru_bhh"])[:D], F)
    w["lin2_wT"] = (kw["lin2_w"].T.copy(), F)
    w["lin2_b"] = (kw["lin2_b"][:, None].copy(), F)
    out = {}
    for k, (v, tag) in w.items():
        v = np.ascontiguousarray(v, np.float32)
        out[k] = v.astype(F16) if tag == B else v
    return out


def _build(n_pad, t_tiles, meta, TOTC, wmeta):
    import concourse.bacc as bacc
    import concourse.mybir as mybir
    import concourse.tile as tile
    from concourse.masks import make_identity

    dt = mybir.dt
    AF = mybir.ActivationFunctionType
    OP = mybir.AluOpType
    AX = mybir.AxisListType
    BF = dt.float16
    F32 = dt.float32

    HC = n_pad // 2
    HT = t_tiles // 2
    NCH = (HC + 511) // 512
    MXC = max(sum(cl[1] for cl in calls) for calls in meta)
    NW = (NC * n_pad + WIN - 1) // WIN

    nc = bacc.Bacc("TRN2", target_bir_lowering=False, debug=False, num_devices=NC,
                   num_swdge_queues=4)

    xfm_h = nc.dram_tensor("xfm", [P, HC], BF, kind="ExternalInput")
    idx_h = nc.dram_tensor("idx16", [P, TOTC * 8], dt.int16, kind="ExternalInput")
    s256_h = nc.dram_tensor("s256", [P, TOTC * 2 * P], BF, kind="ExternalInput")
    be_h = nc.dram_tensor("b_e", [P, TOTC * D], BF, kind="ExternalInput")
    grel_h = nc.dram_tensor("grel", [P, t_tiles], BF, kind="ExternalInput")
    iota_h = nc.dram_tensor("iota256", [P, 2 * P], BF, kind="ExternalInput")
    cst_h = {k: nc.dram_tensor("w_" + k, list(s_), BF if isbf else F32,
                               kind="ExternalInput")
             for k, (s_, isbf) in wmeta.items()}
    y_out = nc.dram_tensor("y", [1, G_LOC], F32, kind="ExternalOutput")

    with tile.TileContext(nc) as tc:
      with (
        tc.tile_pool(name="cst", bufs=1) as cst,
        tc.tile_pool(name="st", bufs=1) as st,
        tc.tile_pool(name="ep", bufs=2) as ep,
        tc.tile_pool(name="sp", bufs=2) as sp,
        tc.tile_pool(name="ps", bufs=2, space="PSUM") as ps,
        tc.tile_pool(name="ps_seg", bufs=2, space="PSUM") as ps_seg,
        tc.tile_pool(name="ps_big", bufs=2, space="PSUM") as ps_big,
        tc.tile_pool(name="dram", bufs=1, space="DRAM") as dp,
      ):
        def load(name):
            h = cst_h[name]
            t = cst.tile(list(h.shape), h.dtype, name="c_" + name)
            nc.sync.dma_start(out=t[:], in_=h[:])
            return t
        Wt = {k: load(k) for k in cst_h}
        idx_sb = cst.tile([P, TOTC * 8], dt.int16, name="idx_sb")
        nc.sync.dma_start(out=idx_sb[:], in_=idx_h[:])
        grel_sb = cst.tile([P, t_tiles], BF, name="grel_sb")
        nc.sync.dma_start(out=grel_sb[:], in_=grel_h[:])
        grel32 = cst.tile([P, t_tiles], F32, name="grel32")
        nc.vector.tensor_copy(out=grel32[:], in_=grel_sb[:])
        iota_sb = cst.tile([P, 2 * P], BF, name="iota_sb")
        nc.sync.dma_start(out=iota_sb[:], in_=iota_h[:])
        identb = cst.tile([P, P], BF, name="identb")
        make_identity(nc, identb[:])
        ident = cst.tile([P, P], F32, name="ident")
        make_identity(nc, ident[:])
        ones1 = cst.tile([1, P], BF, name="ones1")
        nc.vector.memset(ones1[:], 1.0)
        onesf = cst.tile([P, P], BF, name="onesf")
        nc.vector.memset(onesf[:], 1.0)

        XC = st.tile([P, HC], F32, name="XC")
        XCb = st.tile([P, HC], BF, name="XCb")
        HXb = st.tile([P, HC], BF, name="HXb")
        ad_nm = st.tile([P, t_tiles], F32, name="ad_nm")
        adTs = st.tile([P, P], BF, name="adTs")
        row_all = st.tile([P, HT * 2 * WG], BF, name="row_all")
        tbl_locs = [dp.tile([n_pad, WG], BF, name=f"tbl_loc{i}") for i in range(5)]
        tbl_alls = [dp.tile([NC * n_pad, WG], BF, addr_space="Shared",
                            name=f"tbl_all{i}") for i in range(5)]

        def mm_node(dst, wkey, srcb, act=AF.Copy, bias=None, alpha=0.0):
            for ci in range(NCH):
                c0, c1 = ci * 512, min((ci + 1) * 512, HC)
                pt = ps_big.tile([P, 512], F32, name="mmp", tag="mmp")
                nc.tensor.matmul(pt[:, 0:c1 - c0], Wt[wkey][:], srcb[:, c0:c1],
                                 start=True, stop=True)
                b = Wt[bias][:, 0:1] if bias else 0.0
                nc.scalar.activation(dst[:, c0:c1], pt[:, 0:c1 - c0],
                                     act, bias=b, alpha=alpha)

        def elu_inplace(t_fm, bias):
            for ci in range(NCH):
                c0, c1 = ci * 512, min((ci + 1) * 512, HC)
                w_ = c1 - c0
                v = t_fm[:, c0:c1]
                tin = sp.tile([P, 512], F32, name="eluin", tag="eluin")
                nc.vector.tensor_scalar(out=tin[:, 0:w_], in0=v,
                                        scalar1=Wt[bias][:, 0:1],
                                        scalar2=None, op0=OP.add)
                r = sp.tile([P, 512], F32, name="elur", tag="elur")
                nc.scalar.activation(r[:, 0:w_], tin[:, 0:w_], AF.Relu)
                nc.vector.tensor_sub(tin[:, 0:w_], tin[:, 0:w_], r[:, 0:w_])
                nc.scalar.activation(tin[:, 0:w_], tin[:, 0:w_], AF.Exp)
                nc.vector.scalar_tensor_tensor(
                    out=v, in0=tin[:, 0:w_], scalar=-1.0,
                    in1=r[:, 0:w_], op0=OP.add, op1=OP.add)

        def gru_relu(pre):
            wih = Wt[pre + "wih"]
            whh = Wt[pre + "whh"]
            bs = Wt[pre + "bsum"]
            for ci in range(NCH):
                c0, c1 = ci * 512, min((ci + 1) * 512, HC)
                w_ = c1 - c0
                def gate2(g):
                    pt = ps_big.tile([P, 512], F32, name="grup", tag="mmp")
                    nc.tensor.matmul(pt[:, 0:w_], wih[:, g * P:(g + 1) * P],
                                     HXb[:, c0:c1], start=True, stop=False)
                    nc.tensor.matmul(pt[:, 0:w_], whh[:, g * P:(g + 1) * P],
                                     XCb[:, c0:c1], start=False, stop=True)
                    return pt
                pr = gate2(0)
                r = sp.tile([P, 512], F32, name="grur", tag="grur")
                nc.scalar.activation(r[:, 0:w_], pr[:, 0:w_], AF.Sigmoid,
                                     bias=bs[:, 0:1])
                pz = gate2(1)
                z = sp.tile([P, 512], F32, name="gruz", tag="gruz")
                nc.scalar.activation(z[:, 0:w_], pz[:, 0:w_], AF.Sigmoid,
                                     bias=bs[:, 1:2])
                pin = ps_big.tile([P, 512], F32, name="grupi", tag="mmp")
                nc.tensor.matmul(pin[:, 0:w_], wih[:, 2 * P:3 * P],
                                 HXb[:, c0:c1], start=True, stop=True)
                phn = ps_big.tile([P, 512], F32, name="gruph", tag="mmp")
                nc.tensor.matmul(phn[:, 0:w_], whh[:, 2 * P:3 * P],
                                 XCb[:, c0:c1], start=True, stop=True)
                hn = sp.tile([P, 512], F32, name="gruhn", tag="gruhn", bufs=1)
                nc.vector.tensor_scalar(out=hn[:, 0:w_], in0=phn[:, 0:w_],
                                        scalar1=Wt[pre + "bhh"][:, 2:3],
                                        scalar2=None, op0=OP.add)
                nc.vector.tensor_mul(hn[:, 0:w_], hn[:, 0:w_], r[:, 0:w_])
                nc.vector.tensor_tensor(out=hn[:, 0:w_], in0=hn[:, 0:w_],
                                        in1=pin[:, 0:w_], op=OP.add)
                n_t = sp.tile([P, 512], F32, name="grun", tag="grun", bufs=1)
                nc.scalar.activation(n_t[:, 0:w_], hn[:, 0:w_], AF.Tanh,
                                     bias=Wt[pre + "bih"][:, 2:3])
                d_t = sp.tile([P, 512], F32, name="grud", tag="grud", bufs=1)
                nc.vector.tensor_sub(d_t[:, 0:w_], XC[:, c0:c1], n_t[:, 0:w_])
                nc.vector.tensor_mul(d_t[:, 0:w_], d_t[:, 0:w_], z[:, 0:w_])
                nc.vector.tensor_tensor(out=d_t[:, 0:w_], in0=d_t[:, 0:w_],
                                        in1=n_t[:, 0:w_], op=OP.add)
                nc.scalar.activation(XC[:, c0:c1], d_t[:, 0:w_], AF.Relu)

        def build_table(srcb, srcRep2, dstRep2, ad_from_xc, li):
            for tp in range(HT):
                cc = tp * P
                pT = ps.tile([P, P], BF, name="tabT", tag="miscP")
                nc.tensor.transpose(out=pT[:], in_=srcb[:, cc:cc + P],
                                    identity=identb[:])
                row3 = row_all[:, tp * 2 * WG:(tp + 1) * 2 * WG].rearrange(
                    "p (h w) -> p h w", w=WG)
                nc.vector.tensor_copy(
                    out=row3[:, :, 0:D],
                    in_=pT[:].rearrange("q (h f) -> q h f", f=D))
                if srcRep2 is None:
                    nc.vector.memset(row3[:, :, D:W], 0.0)
                else:
                    tmp = sp.tile([P, P], F32, name="tabm", tag="tabm")
                    nc.vector.tensor_tensor(out=tmp[:], in0=pT[:],
                                            in1=Wt[srcRep2][:], op=OP.mult)
                    asr = sp.tile([P, 2], F32, name="asr", tag="asr")
                    nc.vector.tensor_reduce(asr[:].unsqueeze(2),
                                            tmp[:].rearrange("q (h f) -> q h f", f=D),
                                            axis=AX.X, op=OP.add)
                    nc.vector.tensor_copy(out=row3[:, :, D:W],
                                          in_=asr[:].unsqueeze(2))
                if ad_from_xc:
                    pTx = ps.tile([P, P], BF, name="tabTx", tag="miscP")
                    nc.tensor.transpose(out=pTx[:], in_=XCb[:, cc:cc + P],
                                        identity=identb[:])
                    dsrc = pTx
                else:
                    dsrc = pT
                tmp2 = sp.tile([P, P], F32, name="tabm2", tag="tabm2")
                nc.vector.tensor_tensor(out=tmp2[:], in0=dsrc[:],
                                        in1=Wt[dstRep2][:], op=OP.mult)
                nc.vector.tensor_reduce(ad_nm[:, 2 * tp:2 * tp + 2].unsqueeze(2),
                                        tmp2[:].rearrange("q (h f) -> q h f", f=D),
                                        axis=AX.X, op=OP.add)
            nc.sync.dma_start(
                out=tbl_locs[li][:].rearrange("(tp h p) w -> p tp h w", h=2, p=P),
                in_=row_all[:])
            # alpha_dst transposed: adTs[j, q] = ad of node (tile j, row q)
            pAd = ps.tile([P, P], F32, name="adT", tag="miscP")
            nc.tensor.transpose(out=pAd[0:t_tiles, :], in_=ad_nm[:],
                                identity=ident[:])
            nc.vector.tensor_copy(out=adTs[0:t_tiles, :], in_=pAd[0:t_tiles, :])
            nc.gpsimd.collective_compute(
                "AllGather", mybir.AluOpType.bypass,
                replica_groups=[list(range(NC))],
                ins=[tbl_locs[li].opt()], outs=[tbl_alls[li].opt()])

        def edge_phase(is_gate, li):
            """Message round; writes agg (normalized, fp16) into HXb (fm)."""
            col0 = 0
            qrr = 0
            for tp in range(HT):
                calls = meta[tp]
                TC = sum(cl[1] for cl in calls)
                ar2 = sp.tile([1, 2 * P], BF, name="ar2", tag="ar2")
                nc.sync.dma_start(out=ar2[0:1, :], in_=adTs[2 * tp:2 * tp + 2, :])
                prp = ps.tile([P, 2 * P], F32, name="repP", tag="rowP")
                nc.tensor.matmul(prp[:], ones1[:], ar2[0:1, :],
                                 start=True, stop=True)
                rep_pair = sp.tile([P, 2 * P], BF, name="rep_pair", tag="rep_pair")
                nc.vector.tensor_copy(out=rep_pair[:], in_=prp[:])
                gt = ep.tile([P, MXC * WG], BF, name="gt", tag="gt", bufs=3)
                cc = 0
                for w_, ncols, acols, bcol0, amax, bmax in calls:
                    r0 = w_ * WIN
                    r1 = min((w_ + 1) * WIN, NC * n_pad)
                    nc.gpsimd.dma_gather(
                        out_ap=gt[:, cc * WG:(cc + ncols) * WG].rearrange(
                            "p (c w) -> p c w", w=WG),
                        in_ap=tbl_alls[li][r0:r1, :],
                        idxs_ap=idx_sb[:, (col0 + cc) * 8:(col0 + cc + ncols) * 8],
                        num_idxs=ncols * P, num_idxs_reg=ncols * P,
                        elem_size=WG, queue_num=qrr % 4)
                    qrr += 1
                    cc += ncols
                gt3 = gt[:].rearrange("p (c w) -> p c w", w=WG)
                s256 = ep.tile([P, MXC * 2 * P], BF, name="s256", tag="s256")
                nc.sync.dma_start(out=s256[:, 0:TC * 2 * P],
                                  in_=s256_h[:, col0 * 2 * P:(col0 + TC) * 2 * P])
                aslot = sp.tile([P, MXC], F32, name="aslot", tag="aslot")
                for c_ in range(TC):
                    scr = sp.tile([P, 2 * P], BF, name="scr", tag="scr")
                    nc.vector.tensor_tensor(
                        out=scr[:], in0=s256[:, c_ * 2 * P:(c_ + 1) * 2 * P],
                        in1=rep_pair[:], op=OP.mult)
                    nc.vector.tensor_reduce(aslot[:, c_:c_ + 1], scr[:],
                                            axis=AX.X, op=OP.add)
                q = sp.tile([P, MXC], F32, name="q", tag="q")
                if is_gate:
                    be_sb = ep.tile([P, MXC * D], BF, name="be_sb", tag="be_sb", bufs=1)
                    nc.sync.dma_start(out=be_sb[:, 0:TC * D],
                                      in_=be_h[:, col0 * D:(col0 + TC) * D])
                    m_all = ep.tile([P, MXC * D], BF, name="m_all", tag="m_all")
                    m3 = m_all[:].rearrange("p (c w) -> p c w", w=D)
                    nc.vector.tensor_tensor(
                        out=m3[:, 0:TC, :], in0=gt3[:, 0:TC, 0:D],
                        in1=be_sb[:, 0:TC * D].rearrange("p (c w) -> p c w", w=D),
                        op=OP.add)
                    nc.scalar.activation(m_all[:, :TC * D], m_all[:, :TC * D],
                                         AF.Prelu, alpha=NEG)
                    lt = ep.tile([P, MXC * D], BF, name="lt", tag="lt", bufs=1)
                    nc.vector.tensor_tensor(
                        out=lt[:, :TC * D], in0=m3[:, 0:TC, :],
                        in1=Wt["attlRep"][:].unsqueeze(1).to_broadcast([P, TC, D]),
                        op=OP.mult)
                    nc.vector.tensor_reduce(q[:, 0:TC].unsqueeze(2),
                                            lt[:, :TC * D].rearrange(
                                                "p (c w) -> p c w", w=D),
                                            axis=AX.X, op=OP.add)
                    nc.vector.tensor_tensor(out=q[:, 0:TC], in0=q[:, 0:TC],
                                            in1=aslot[:, 0:TC], op=OP.add)
                    msg3 = m3
                else:
                    nc.vector.tensor_tensor(out=q[:, 0:TC],
                                            in0=gt3[:, 0:TC, D:D + 1].squeeze(2),
                                            in1=aslot[:, 0:TC], op=OP.add)
                    msg3 = gt3
                e_t = sp.tile([P, MXC], F32, name="e_t", tag="e_t")
                nc.scalar.activation(e_t[:, 0:TC], q[:, 0:TC], AF.Prelu, alpha=NEG)
                nc.scalar.activation(e_t[:, 0:TC], e_t[:, 0:TC], AF.Exp)
                rhs = ep.tile([P, MXC * W], BF, name="rhs", tag="rhs")
                r3 = rhs[:].rearrange("p (c w) -> p c w", w=W)
                nc.vector.tensor_tensor(
                    out=r3[:, 0:TC, 0:D], in0=msg3[:, 0:TC, 0:D],
                    in1=e_t[:, 0:TC].unsqueeze(2).to_broadcast([P, TC, D]),
                    op=OP.mult)
                nc.vector.tensor_copy(out=r3[:, 0:TC, D:W],
                                      in_=e_t[:, 0:TC].unsqueeze(2))
                amms = []
                bmms = []
                cc = 0
                for w_, ncols, acols, bcol0, amax, bmax in calls:
                    if amax:
                        amms += [cc + c for c in range(acols)]
                    if bmax:
                        bmms += [cc + c for c in range(bcol0, ncols)]
                    cc += ncols
                aggb = sp.tile([P, P], BF, name="aggb", tag="aggb")
                for h, mms, qofs in ((0, amms, 0), (1, bmms, P)):
                    pseg = ps_seg.tile([P, W], F32, name="pseg", tag="pseg")
                    for i, c in enumerate(mms):
                        nc.tensor.matmul(
                            pseg[:],
                            s256[:, c * 2 * P + qofs:c * 2 * P + qofs + P],
                            rhs[:, c * W:(c + 1) * W],
                            start=(i == 0), stop=(i == len(mms) - 1))
                    sn = sp.tile([P, 1], F32, name="sn", tag="sn")
                    nc.vector.tensor_single_scalar(out=sn[:], in_=pseg[:, D:W],
                                                   scalar=1e-16, op=OP.add)
                    rcp = sp.tile([P, 1], F32, name="rcp", tag="rcp")
                    nc.vector.reciprocal(rcp[:], sn[:])
                    nc.vector.tensor_tensor(out=aggb[:, h * D:(h + 1) * D],
                                            in0=pseg[:, 0:D],
                                            in1=rcp[:].to_broadcast([P, D]),
                                            op=OP.mult)
                pT2 = ps.tile([P, P], F32, name="aggT", tag="miscP")
                nc.tensor.matmul(pT2[:], aggb[:], identb[:], start=True, stop=True)
                nc.vector.tensor_copy(out=HXb[:, tp * P:(tp + 1) * P], in_=pT2[:])
                col0 += TC

        # ================= forward =================
        for ci in range(NCH):
            c0, c1 = ci * 512, min((ci + 1) * 512, HC)
            xin = sp.tile([P, 512], BF, name="xin", tag="xin")
            nc.sync.dma_start(out=xin[:, :c1 - c0], in_=xfm_h[:, c0:c1])
            pt = ps_big.tile([P, 512], F32, name="mmp0", tag="mmp")
            nc.tensor.matmul(pt[:, 0:c1 - c0], Wt["lin1_bd"][:],
                             xin[:, 0:c1 - c0], start=True, stop=True)
            nc.scalar.activation(XC[:, c0:c1], pt[:, 0:c1 - c0],
                                 AF.Prelu, bias=Wt["lin1_b"][:, 0:1], alpha=NEG)
        nc.vector.tensor_copy(out=XCb[:], in_=XC[:])
        # --- GATEConv ---
        mm_node(HXb, "gate_w1a_bd", XCb)
        build_table(HXb, None, "gateattrRep2", True, 0)
        edge_phase(True, 0)
        mm_node(HXb, "gate_w2_bd", HXb)
        elu_inplace(HXb, "gate_bias")
        gru_relu("gru0_")
        nc.vector.tensor_copy(out=XCb[:], in_=XC[:])
        # --- atom layers ---
        for l in range(4):
            pre = f"at{l}_"
            mm_node(HXb, pre + "wT", XCb)
            build_table(HXb, pre + "srcRep2", pre + "dstRep2", False, 1 + l)
            edge_phase(False, 1 + l)
            elu_inplace(HXb, pre + "bias")
            gru_relu(pre + "gru_")
            nc.vector.tensor_copy(out=XCb[:], in_=XC[:])

        # ================= readout =================
        mm_node(HXb, "mol_bd", XCb)          # xs into HXb
        asrc_nm = st.tile([P, t_tiles], F32, name="asrc_nm")
        for tp in range(HT):
            pT = ps.tile([P, P], BF, name="xsT", tag="miscP")
            nc.tensor.transpose(out=pT[:], in_=HXb[:, tp * P:(tp + 1) * P],
                                identity=identb[:])
            nc.vector.tensor_copy(out=row_all[:, tp * 2 * D:(tp + 1) * 2 * D],
                                  in_=pT[:])
            tmp = sp.tile([P, P], F32, name="xsm", tag="tabm")
            nc.vector.tensor_tensor(out=tmp[:], in0=pT[:],
                                    in1=Wt["molsrcRep2"][:], op=OP.mult)
            nc.vector.tensor_reduce(asrc_nm[:, 2 * tp:2 * tp + 2].unsqueeze(2),
                                    tmp[:].rearrange("q (h f) -> q h f", f=D),
                                    axis=AX.X, op=OP.add)
        for tp in range(HT):
            pT = ps.tile([P, P], BF, name="xcT", tag="miscP")
            nc.tensor.transpose(out=pT[:], in_=XCb[:, tp * P:(tp + 1) * P],
                                identity=identb[:])
            nc.vector.tensor_copy(out=XCb[:, tp * P:(tp + 1) * P], in_=pT[:])
        xc_nm = XCb
        xs_nm = row_all
        OUTT = st.tile([D, 2 * P], F32, name="OUTT")
        for k in range(2):
            pg = ps_seg.tile([P, D], F32, name="pg", tag="pseg")
            for tp in range(HT):
                sgp = sp.tile([P, 2 * 2 * P], BF, name="sgp", tag="sgp")
                for h in range(2):
                    nc.vector.tensor_scalar(
                        out=sgp[:, h * 2 * P:(h + 1) * 2 * P], in0=iota_sb[:],
                        scalar1=grel32[:, 2 * tp + h:2 * tp + h + 1],
                        scalar2=None, op0=OP.is_equal)
                for h in range(2):
                    j = 2 * tp + h
                    nc.tensor.matmul(
                        pg[:],
                        sgp[:, h * 2 * P + k * P:h * 2 * P + (k + 1) * P],
                        xc_nm[:, j * D:(j + 1) * D],
                        start=(j == 0), stop=(j == t_tiles - 1))
            og = sp.tile([P, D], F32, name="og", tag="og")
            nc.scalar.activation(og[:], pg[:], AF.Relu)
            pTo = ps.tile([D, P], F32, name="ogT", tag="miscP")
            nc.tensor.transpose(out=pTo[:], in_=og[:], identity=ident[:])
            nc.vector.tensor_copy(out=OUTT[:, k * P:(k + 1) * P], in_=pTo[:])
        HG = st.tile([D, 2 * P], F32, name="HG")
        for ts in range(3):
            pxd = ps_big.tile([D, 512], F32, name="xdp", tag="mmp")
            nc.tensor.matmul(pxd[:, 0:2 * P], Wt["mol_wT32"][:], OUTT[:],
                             start=True, stop=True)
            xds = sp.tile([D, 2 * P], F32, name="xds", tag="xds", bufs=1)
            nc.vector.tensor_copy(out=xds[:], in_=pxd[:, 0:2 * P])
            pag = ps.tile([1, 2 * P], F32, name="agp", tag="rowP")
            nc.tensor.matmul(pag[:], Wt["moldstCol"][:], xds[:],
                             start=True, stop=True)
            agr = sp.tile([1, 2 * P], BF, name="agr", tag="agr")
            nc.vector.tensor_copy(out=agr[:], in_=pag[:])
            prg = ps.tile([P, 2 * P], F32, name="repG", tag="rowP")
            nc.tensor.matmul(prg[:], ones1[:], agr[:], start=True, stop=True)
            rep_agr = sp.tile([P, 2 * P], BF, name="rep_agr", tag="rep_agr", bufs=1)
            nc.vector.tensor_copy(out=rep_agr[:], in_=prg[:])
            ag2 = sp.tile([P, 2], F32, name="ag2", tag="ag2")
            psg = [ps_seg.tile([P, W], F32, name=f"psg{k}", tag="pseg")
                   for k in range(2)]
            for tp in range(HT):
                sgp = sp.tile([P, 2 * 2 * P], BF, name="sgp2", tag="sgp")
                for h in range(2):
                    nc.vector.tensor_scalar(
                        out=sgp[:, h * 2 * P:(h + 1) * 2 * P], in0=iota_sb[:],
                        scalar1=grel32[:, 2 * tp + h:2 * tp + h + 1],
                        scalar2=None, op0=OP.is_equal)
                    scr2 = sp.tile([P, 2 * P], BF, name="scr2", tag="scr")
                    nc.vector.tensor_tensor(
                        out=scr2[:], in0=sgp[:, h * 2 * P:(h + 1) * 2 * P],
                        in1=rep_agr[:], op=OP.mult)
                    nc.vector.tensor_reduce(ag2[:, h:h + 1], scr2[:],
                                            axis=AX.X, op=OP.add)
                q2 = sp.tile([P, 2], F32, name="q2", tag="q2")
                nc.vector.tensor_tensor(out=q2[:], in0=asrc_nm[:, 2 * tp:2 * tp + 2],
                                        in1=ag2[:], op=OP.add)
                nc.scalar.activation(q2[:], q2[:], AF.Prelu, alpha=NEG)
                nc.scalar.activation(q2[:], q2[:], AF.Exp)
                rh = ep.tile([P, 2 * W], BF, name="rh", tag="rh", bufs=2)
                rh3 = rh[:].rearrange("p (h w) -> p h w", w=W)
                nc.vector.tensor_tensor(
                    out=rh3[:, :, 0:D],
                    in0=xs_nm[:, tp * 2 * D:(tp + 1) * 2 * D].rearrange(
                        "p (h f) -> p h f", f=D),
                    in1=q2[:].unsqueeze(2).to_broadcast([P, 2, D]),
                    op=OP.mult)
                nc.vector.tensor_copy(out=rh3[:, :, D:W], in_=q2[:].unsqueeze(2))
                for h in range(2):
                    j = 2 * tp + h
                    for k in range(2):
                        nc.tensor.matmul(
                            psg[k][:],
                            sgp[:, h * 2 * P + k * P:h * 2 * P + (k + 1) * P],
                            rh[:, h * W:(h + 1) * W],
                            start=(j == 0), stop=(j == t_tiles - 1))
            for k in range(2):
                sn = sp.tile([P, 1], F32, name="sng", tag="sn")
                nc.vector.tensor_single_scalar(out=sn[:], in_=psg[k][:, D:W],
                                               scalar=1e-16, op=OP.add)
                rcp = sp.tile([P, 1], F32, name="rcpg", tag="rcp")
                nc.vector.reciprocal(rcp[:], sn[:])
                aggg = sp.tile([P, D], F32, name="aggg", tag="aggg")
                nc.vector.tensor_tensor(out=aggg[:], in0=psg[k][:, 0:D],
                                        in1=rcp[:].to_broadcast([P, D]), op=OP.mult)
                nc.vector.tensor_tensor(out=aggg[:], in0=aggg[:],
                                        in1=Wt["mol_biasRep"][:], op=OP.add)
                r = sp.tile([P, D], F32, name="rg", tag="rg")
                nc.scalar.activation(r[:], aggg[:], AF.Relu)
                xm = sp.tile([P, D], F32, name="xmg", tag="xmg")
                nc.vector.tensor_sub(xm[:], aggg[:], r[:])
                nc.scalar.activation(xm[:], xm[:], AF.Exp)
                nc.vector.scalar_tensor_tensor(out=aggg[:], in0=xm[:], scalar=-1.0,
                                               in1=r[:], op0=OP.add, op1=OP.add)
                pTh = ps.tile([D, P], F32, name="hgT", tag="miscP")
                nc.tensor.transpose(out=pTh[:], in_=aggg[:], identity=ident[:])
                nc.vector.tensor_copy(out=HG[:, k * P:(k + 1) * P], in_=pTh[:])
            wih = Wt["mol_gru_wih"]
            whh = Wt["mol_gru_whh"]
            bs = Wt["mol_gru_bsum"]
            def gate2g(g):
                pt = ps_big.tile([D, 512], F32, name="ggp", tag="mmp")
                nc.tensor.matmul(pt[:, 0:2 * P], wih[:, g * D:(g + 1) * D], HG[:],
                                 start=True, stop=False)
                nc.tensor.matmul(pt[:, 0:2 * P], whh[:, g * D:(g + 1) * D], OUTT[:],
                                 start=False, stop=True)
                return pt
            prg2 = gate2g(0)
            rg2 = sp.tile([D, 2 * P], F32, name="ggr", tag="ggr", bufs=1)
            nc.scalar.activation(rg2[:], prg2[:, 0:2 * P], AF.Sigmoid, bias=bs[:, 0:1])
            pzg = gate2g(1)
            zg = sp.tile([D, 2 * P], F32, name="ggz", tag="ggz", bufs=1)
            nc.scalar.activation(zg[:], pzg[:, 0:2 * P], AF.Sigmoid, bias=bs[:, 1:2])
            pig = ps_big.tile([D, 512], F32, name="ggpi", tag="mmp")
            nc.tensor.matmul(pig[:, 0:2 * P], wih[:, 2 * D:3 * D], HG[:],
                             start=True, stop=True)
            phg = ps_big.tile([D, 512], F32, name="ggph", tag="mmp")
            nc.tensor.matmul(phg[:, 0:2 * P], whh[:, 2 * D:3 * D], OUTT[:],
                             start=True, stop=True)
            hng = sp.tile([D, 2 * P], F32, name="gghn", tag="gghn", bufs=1)
            nc.vector.tensor_scalar(out=hng[:], in0=phg[:, 0:2 * P],
                                    scalar1=Wt["mol_gru_bhh"][:, 2:3],
                                    scalar2=None, op0=OP.add)
            nc.vector.tensor_mul(hng[:], hng[:], rg2[:])
            nc.vector.tensor_tensor(out=hng[:], in0=hng[:], in1=pig[:, 0:2 * P],
                                    op=OP.add)
            ng = sp.tile([D, 2 * P], F32, name="ggn", tag="ggn", bufs=1)
            nc.scalar.activation(ng[:], hng[:], AF.Tanh,
                                 bias=Wt["mol_gru_bih"][:, 2:3])
            dg = sp.tile([D, 2 * P], F32, name="ggd", tag="ggd", bufs=1)
            nc.vector.tensor_sub(dg[:], OUTT[:], ng[:])
            nc.vector.tensor_mul(dg[:], dg[:], zg[:])
            nc.vector.tensor_tensor(out=dg[:], in0=dg[:], in1=ng[:], op=OP.add)
            nc.scalar.activation(OUTT[:], dg[:], AF.Relu)
        py = ps.tile([1, 2 * P], F32, name="py", tag="rowP")
        nc.tensor.matmul(py[:], Wt["lin2_wT"][:], OUTT[:], start=True, stop=True)
        ysb = sp.tile([1, 2 * P], F32, name="ysb", tag="ysb")
        nc.vector.tensor_scalar(out=ysb[:], in0=py[:], scalar1=Wt["lin2_b"][0:1, 0:1],
                                scalar2=None, op0=OP.add)
        nc.sync.dma_start(out=y_out[:], in_=ysb[0:1, 0:G_LOC])
    nc.compile()
    return nc


_CACHE = {}


def kernel(**inputs):
    from concourse.bass_utils import run_bass_kernel_spmd
    x = np.asarray(inputs["x"], np.float32)
    ei = np.asarray(inputs["edge_index"])
    ea = np.asarray(inputs["edge_attr"], np.float32)
    bt = np.asarray(inputs["batch"])
    per, n_pad, t_tiles, meta, TOTC = _prep(x, ei, ea, bt)
    kwf = {k: np.asarray(v, np.float32) for k, v in inputs.items()
           if k not in ("x", "edge_index", "edge_attr", "batch")}
    weights = _mk_weights(kwf)
    key = (n_pad, TOTC, tuple(tuple(tuple(cl) for cl in calls) for calls in meta))
    if key not in _CACHE:
        _CACHE[key] = _build(n_pad, t_tiles, meta, TOTC,
                             {k: (v.shape, v.dtype == F16)
                              for k, v in weights.items()})
    nc = _CACHE[key]
    iota = np.tile(np.arange(2 * P).astype(np.float32)[None, :], (P, 1)).astype(F16)
    w1b = kwf["gate_lin1_w"][:, D:]
    in_maps = []
    for c in range(NC):
        b_e = (per[c]["attr_s"] @ w1b.T).astype(F16)
        b_e = np.ascontiguousarray(
            b_e.reshape(TOTC, P, D).transpose(1, 0, 2).reshape(P, TOTC * D))
        m = dict(xfm=per[c]["xfm"], idx16=per[c]["idx16"], s256=per[c]["s256"],
                 b_e=b_e, grel=per[c]["grel"], iota256=iota)
        for k, v in weights.items():
            m["w_" + k] = v
        in_maps.append(m)
    res = run_bass_kernel_spmd(nc, in_maps, core_ids=list(range(NC)))
    return np.concatenate([res.results[c]["y"][0] for c in range(NC)]).astype(np.float32)


# revision 16
# speedup vs baseline: 1.3774x; 1.0734x over previous
"""AttentiveFP forward on 8 Trainium2 NeuronCores (Bass/Tile).

Sharding: 2048 graphs (nodes contiguous, batch sorted) split into 8 blocks of
256 graphs; each core owns the edges whose dst node falls in its block. Per
round each core computes its nodes' features, all-gathers a compact per-node
table [xt | alpha_src] (fp16, 65 wide), expands it locally to 256B-aligned
rows, then fetches per-edge src rows with nc.gpsimd.dma_gather (the token
gather ucode: thousands of int16 indices per call, round-robined over 4 SWDGE
queues). Indices are int16, so slots are grouped per (node-tile-pair,
32768-row source window); within a call, tile-a slots carry rel in [0,128)
and tile-b slots rel in [128,256), so one 256-wide is_equal one-hot serves
both tiles' PSUM segment-matmuls and the alpha_dst select (one-hot x
replicated alpha row, reduced on DVE). Per-edge alpha_dst needs no gather
(dst is always local). Node phases run feature-major, half-packed, with
block-diagonal [128,128] fp16 weights; GRU hidden state stays fp32. The gate
round's edge-attr term (W1b @ e_attr) is precomputed on the host per slot.
Readout uses a 256-wide graph one-hot per tile-pair plus a replicated
per-graph alpha row (no gathers).

Softmax max-subtraction is skipped (logits O(1), shift-invariant).
"""
import sys
sys.path.insert(0, '/opt/trn_rl_repo')
sys.path.insert(0, '/root/.axon_site')

import numpy as np

F16 = np.float16
NC = 8
D = 64
G_TOT = 2048
G_LOC = G_TOT // NC
F_IN = 25
E_DIM = 4
NEG = 0.01
P = 128
W = 65            # compact table row: [xt(64) | alpha_src]
WG = 128          # gathered row width (256B-aligned)
WIN = 32768       # int16 index window (rows)


def _prep(x, edge_index, edge_attr, batch):
    src = edge_index[0].astype(np.int64)
    dst = edge_index[1].astype(np.int64)
    batch = batch.astype(np.int64)

    gstart = np.searchsorted(batch, np.arange(0, G_TOT + 1, G_LOC))
    n0 = gstart[:-1]
    nloc = np.diff(gstart)
    n_pad = int(np.ceil((nloc.max() + 1) / 256) * 256)
    t_tiles = n_pad // P
    HC = n_pad // 2
    HT = t_tiles // 2
    NW = (NC * n_pad + WIN - 1) // WIN

    def pi_row(n):
        h = n // HC
        r = n % HC
        return (2 * (r // P) + h) * P + (r % P)

    src_dev = np.searchsorted(gstart[1:], src, side='right')
    dst_dev = np.searchsorted(gstart[1:], dst, side='right')
    gidx_all = src_dev * n_pad + pi_row(src - n0[src_dev])

    # ---- pass 1: bucket edges per core into (pair, window, half) ----
    buckets = [[[[None, None] for _ in range(NW)] for _ in range(HT)]
               for _ in range(NC)]
    for c in range(NC):
        sel = np.where(dst_dev == c)[0]
        dl = dst[sel] - n0[c]
        j_dst = 2 * ((dl % HC) // P) + dl // HC
        p_dst = dl % P
        gi = gidx_all[sel]
        w_of = gi // WIN
        for tp in range(HT):
            for h in range(2):
                m = j_dst == 2 * tp + h
                gi_m, p_m, w_m, sel_m = gi[m], p_dst[m], w_of[m], sel[m]
                for w in range(NW):
                    mm = w_m == w
                    buckets[c][tp][w][h] = (gi_m[mm] - w * WIN, p_m[mm], sel_m[mm])
    # ---- pass 2: SPMD-uniform call metadata (max counts over cores) ----
    meta = []       # per pair: [w, ncols, acols, bcol0, amax, bmax]
    for tp in range(HT):
        calls = []
        for w in range(NW):
            amax = max(len(buckets[c][tp][w][0][0]) for c in range(NC))
            bmax = max(len(buckets[c][tp][w][1][0]) for c in range(NC))
            if amax + bmax == 0:
                continue
            ncols = (amax + bmax + P - 1) // P
            calls.append([w, ncols, (amax + P - 1) // P, amax // P, amax, bmax])
        if not any(cl[4] for cl in calls):
            calls.insert(0, [0, 1, 1, 0, P, 0])
        if not any(cl[5] for cl in calls):
            calls.append([0, 1, 0, 0, 0, P])
        meta.append(calls)
    TOTC = sum(cl[1] for calls in meta for cl in calls)

    per = []
    for c in range(NC):
        idx16 = np.zeros((16, TOTC * 8), np.int16)
        rel = np.full((P, TOTC), 300.0, np.float32)
        attr_s = np.zeros((TOTC * P, E_DIM), np.float32)
        col0 = 0
        for tp in range(HT):
            for w_, ncols, acols, bcol0, amax, bmax in meta[tp]:
                flat_idx = np.zeros(ncols * P, np.int16)
                flat_rel = np.full(ncols * P, 300.0, np.float32)
                flat_attr = np.zeros((ncols * P, E_DIM), np.float32)
                pos = 0
                for h, hmax in ((0, amax), (1, bmax)):
                    gi_l, p_l, sel_l = buckets[c][tp][w_][h]
                    k = len(gi_l)
                    flat_idx[pos:pos + k] = gi_l.astype(np.int16)
                    flat_rel[pos:pos + k] = p_l + h * P
                    flat_attr[pos:pos + k] = edge_attr[sel_l]
                    pos += hmax
                idx16[:, col0 * 8:(col0 + ncols) * 8] = \
                    flat_idx.reshape(ncols * 8, 16).T
                rel[:, col0:col0 + ncols] = flat_rel.reshape(ncols, P).T
                attr_s[col0 * P:(col0 + ncols) * P] = flat_attr
                col0 += ncols
        s256 = (rel.astype(np.int32)[:, :, None] ==
                np.arange(2 * P, dtype=np.int32)[None, None, :]).astype(F16)
        per.append(dict(idx16=np.tile(idx16, (8, 1)),
                        s256=np.ascontiguousarray(s256.reshape(P, TOTC * 2 * P)),
                        attr_s=attr_s))
        nl = int(nloc[c])
        gl = batch[n0[c]:n0[c] + nl] - G_LOC * c
        grel = np.full((P, t_tiles), 300.0, np.float32)
        n_ids = np.arange(n_pad)
        h_a = n_ids // HC
        j_a = 2 * ((n_ids % HC) // P) + h_a
        p_a = n_ids % P
        valid = n_ids < nl
        grel[p_a[valid], j_a[valid]] = gl[n_ids[valid]]
        per[c]['grel'] = np.ascontiguousarray(grel.astype(F16))
        xp = np.zeros((n_pad, F_IN), np.float32)
        xp[:nl] = x[n0[c]:n0[c] + nl]
        xfm = np.zeros((P, HC), np.float32)
        xfm[:F_IN] = xp[:HC].T
        xfm[D:D + F_IN] = xp[HC:].T
        per[c]['xfm'] = xfm.astype(F16)
    return per, n_pad, t_tiles, meta, TOTC


def _mk_weights(kw):
    w = {}
    def bd(a):
        t = a.T
        z = np.zeros((P, P), np.float32)
        z[0:D, 0:D] = t
        z[D:2 * D, D:2 * D] = t
        return z
    def col(a):
        return np.concatenate([a, a])[:, None]
    def rep2(a):
        return np.tile(np.concatenate([a, a])[None, :], (P, 1))
    def rep1(a):
        return np.tile(a[None, :], (P, 1))
    def gb(a):
        t = a.reshape(3, D).T
        return np.concatenate([t, t], 0)
    def gru_bd(wg):
        out = np.zeros((P, 3 * P), np.float32)
        for g in range(3):
            out[:, g * P:(g + 1) * P] = bd(wg[g * D:(g + 1) * D])
        return out

    B, F = 'b', 'f'
    lin1 = np.zeros((P, P), np.float32)
    lin1[0:F_IN, 0:D] = kw["lin1_w"].T
    lin1[D:D + F_IN, D:2 * D] = kw["lin1_w"].T
    w["lin1_bd"] = (lin1, B)
    w["lin1_b"] = (col(kw["lin1_b"]), F)
    w["gate_w1a_bd"] = (bd(kw["gate_lin1_w"][:, :D]), B)
    w["attlRep"] = (rep1(kw["gate_att_l"]), B)
    w["gateattrRep2"] = (rep2(kw["gate_att_r"]), B)
    w["gate_w2_bd"] = (bd(kw["gate_lin2_w"]), B)
    w["gate_bias"] = (col(kw["gate_bias"]), F)
    w["gru0_wih"] = (gru_bd(kw["gru0_wih"]), B)
    w["gru0_whh"] = (gru_bd(kw["gru0_whh"]), B)
    w["gru0_bih"] = (gb(kw["gru0_bih"]), F)
    w["gru0_bhh"] = (gb(kw["gru0_bhh"]), F)
    w["gru0_bsum"] = (gb(kw["gru0_bih"] + kw["gru0_bhh"]), F)
    for l in range(4):
        pre = f"at{l}_"
        w[pre + "wT"] = (bd(kw["atom_lin_w"][l]), B)
        w[pre + "srcRep2"] = (rep2(kw["atom_att_src"][l]), B)
        w[pre + "dstRep2"] = (rep2(kw["atom_att_dst"][l]), B)
        w[pre + "bias"] = (col(kw["atom_bias"][l]), F)
        w[pre + "gru_wih"] = (gru_bd(kw["atom_gru_wih"][l]), B)
        w[pre + "gru_whh"] = (gru_bd(kw["atom_gru_whh"][l]), B)
        w[pre + "gru_bih"] = (gb(kw["atom_gru_bih"][l]), F)
        w[pre + "gru_bhh"] = (gb(kw["atom_gru_bhh"][l]), F)
        w[pre + "gru_bsum"] = (gb(kw["atom_gru_bih"][l] + kw["atom_gru_bhh"][l]), F)
    w["mol_bd"] = (bd(kw["mol_lin_w"]), B)
    w["mol_wT32"] = (kw["mol_lin_w"].T.copy(), F)
    w["molsrcRep2"] = (rep2(kw["mol_att_src"]), B)
    w["moldstCol"] = (kw["mol_att_dst"][:, None].copy(), F)
    w["mol_biasRep"] = (rep1(kw["mol_bias"]), F)
    w["mol_gru_wih"] = (kw["mol_gru_wih"].T.copy(), F)
    w["mol_gru_whh"] = (kw["mol_gru_whh"].T.copy(), F)
    w["mol_gru_bih"] = (gb(kw["mol_gru_bih"])[:D], F)
    w["mol_gru_bhh"] = (gb(kw["mol_gru_bhh"])[:D], F)
    w["mol_gru_bsum"] = (gb(kw["mol_gru_bih"] + kw["mol_gru_bhh"])[:D], F)
    w["lin2_wT"] = (kw["lin2_w"].T.copy(), F)
    w["lin2_b"] = (kw["lin2_b"][:, None].copy(), F)
    out = {}
    for k, (v, tag) in w.items():
        v = np.ascontiguousarray(v, np.float32)
        out[k] = v.astype(F16) if tag == B else v
    return out


def _build(n_pad, t_tiles, meta, TOTC, wmeta):
    import concourse.bacc as bacc
    import concourse.mybir as mybir
    import concourse.tile as tile
    from concourse.masks import make_identity

    dt = mybir.dt
    AF = mybir.ActivationFunctionType
    OP = mybir.AluOpType
    AX = mybir.AxisListType
    BF = dt.float16
    F32 = dt.float32

    HC = n_pad // 2
    HT = t_tiles // 2
    NCH = (HC + 511) // 512
    MXC = max(sum(cl[1] for cl in calls) for calls in meta)
    NW = (NC * n_pad + WIN - 1) // WIN

    nc = bacc.Bacc("TRN2", target_bir_lowering=False, debug=False, num_devices=NC,
                   num_swdge_queues=4)

    xfm_h = nc.dram_tensor("xfm", [P, HC], BF, kind="ExternalInput")
    idx_h = nc.dram_tensor("idx16", [P, TOTC * 8], dt.int16, kind="ExternalInput")
    s256_h = nc.dram_tensor("s256", [P, TOTC * 2 * P], BF, kind="ExternalInput")
    be_h = nc.dram_tensor("b_e", [P, TOTC * D], BF, kind="ExternalInput")
    grel_h = nc.dram_tensor("grel", [P, t_tiles], BF, kind="ExternalInput")
    iota_h = nc.dram_tensor("iota256", [P, 2 * P], BF, kind="ExternalInput")
    cst_h = {k: nc.dram_tensor("w_" + k, list(s_), BF if isbf else F32,
                               kind="ExternalInput")
             for k, (s_, isbf) in wmeta.items()}
    y_out = nc.dram_tensor("y", [1, G_LOC], F32, kind="ExternalOutput")

    with tile.TileContext(nc) as tc:
      with (
        tc.tile_pool(name="cst", bufs=1) as cst,
        tc.tile_pool(name="st", bufs=1) as st,
        tc.tile_pool(name="ep", bufs=2) as ep,
        tc.tile_pool(name="sp", bufs=2) as sp,
        tc.tile_pool(name="ps", bufs=2, space="PSUM") as ps,
        tc.tile_pool(name="ps_seg", bufs=2, space="PSUM") as ps_seg,
        tc.tile_pool(name="ps_big", bufs=2, space="PSUM") as ps_big,
        tc.tile_pool(name="dram", bufs=1, space="DRAM") as dp,
      ):
        def load(name):
            h = cst_h[name]
            t = cst.tile(list(h.shape), h.dtype, name="c_" + name)
            nc.sync.dma_start(out=t[:], in_=h[:])
            return t
        Wt = {k: load(k) for k in cst_h}
        idx_sb = cst.tile([P, TOTC * 8], dt.int16, name="idx_sb")
        nc.sync.dma_start(out=idx_sb[:], in_=idx_h[:])
        grel_sb = cst.tile([P, t_tiles], BF, name="grel_sb")
        nc.sync.dma_start(out=grel_sb[:], in_=grel_h[:])
        grel32 = cst.tile([P, t_tiles], F32, name="grel32")
        nc.vector.tensor_copy(out=grel32[:], in_=grel_sb[:])
        iota_sb = cst.tile([P, 2 * P], BF, name="iota_sb")
        nc.sync.dma_start(out=iota_sb[:], in_=iota_h[:])
        identb = cst.tile([P, P], BF, name="identb")
        make_identity(nc, identb[:])
        ident = cst.tile([P, P], F32, name="ident")
        make_identity(nc, ident[:])
        ones1 = cst.tile([1, P], BF, name="ones1")
        nc.vector.memset(ones1[:], 1.0)
        onesf = cst.tile([P, P], BF, name="onesf")
        nc.vector.memset(onesf[:], 1.0)

        XC = st.tile([P, HC], F32, name="XC")
        XCb = st.tile([P, HC], BF, name="XCb")
        HXb = st.tile([P, HC], BF, name="HXb")
        ad_nm = st.tile([P, t_tiles], F32, name="ad_nm")
        adTs = st.tile([P, P], BF, name="adTs")
        row_all = st.tile([P, HT * 2 * WG], BF, name="row_all")
        tbl_locs = [dp.tile([n_pad, WG], BF, name=f"tbl_loc{i}") for i in range(5)]
        tbl_alls = [dp.tile([NC * n_pad, WG], BF, addr_space="Shared",
                            name=f"tbl_all{i}") for i in range(5)]

        def mm_node(dst, wkey, srcb, act=AF.Copy, bias=None, alpha=0.0):
            for ci in range(NCH):
                c0, c1 = ci * 512, min((ci + 1) * 512, HC)
                pt = ps_big.tile([P, 512], F32, name="mmp", tag="mmp")
                nc.tensor.matmul(pt[:, 0:c1 - c0], Wt[wkey][:], srcb[:, c0:c1],
                                 start=True, stop=True)
                b = Wt[bias][:, 0:1] if bias else 0.0
                nc.scalar.activation(dst[:, c0:c1], pt[:, 0:c1 - c0],
                                     act, bias=b, alpha=alpha)

        def elu_inplace(t_fm, bias):
            for ci in range(NCH):
                c0, c1 = ci * 512, min((ci + 1) * 512, HC)
                w_ = c1 - c0
                v = t_fm[:, c0:c1]
                tin = sp.tile([P, 512], F32, name="eluin", tag="eluin", bufs=1)
                nc.vector.tensor_scalar(out=tin[:, 0:w_], in0=v,
                                        scalar1=Wt[bias][:, 0:1],
                                        scalar2=None, op0=OP.add)
                r = sp.tile([P, 512], F32, name="elur", tag="elur", bufs=1)
                nc.scalar.activation(r[:, 0:w_], tin[:, 0:w_], AF.Relu)
                nc.vector.tensor_sub(tin[:, 0:w_], tin[:, 0:w_], r[:, 0:w_])
                nc.scalar.activation(tin[:, 0:w_], tin[:, 0:w_], AF.Exp)
                nc.vector.scalar_tensor_tensor(
                    out=v, in0=tin[:, 0:w_], scalar=-1.0,
                    in1=r[:, 0:w_], op0=OP.add, op1=OP.add)

        def gru_relu(pre):
            wih = Wt[pre + "wih"]
            whh = Wt[pre + "whh"]
            bs = Wt[pre + "bsum"]
            for ci in range(NCH):
                c0, c1 = ci * 512, min((ci + 1) * 512, HC)
                w_ = c1 - c0
                def gate2(g):
                    pt = ps_big.tile([P, 512], F32, name="grup", tag="mmp")
                    nc.tensor.matmul(pt[:, 0:w_], wih[:, g * P:(g + 1) * P],
                                     HXb[:, c0:c1], start=True, stop=False)
                    nc.tensor.matmul(pt[:, 0:w_], whh[:, g * P:(g + 1) * P],
                                     XCb[:, c0:c1], start=False, stop=True)
                    return pt
                pr = gate2(0)
                r = sp.tile([P, 512], F32, name="grur", tag="grur")
                nc.scalar.activation(r[:, 0:w_], pr[:, 0:w_], AF.Sigmoid,
                                     bias=bs[:, 0:1])
                pz = gate2(1)
                z = sp.tile([P, 512], F32, name="gruz", tag="gruz", bufs=1)
                nc.scalar.activation(z[:, 0:w_], pz[:, 0:w_], AF.Sigmoid,
                                     bias=bs[:, 1:2])
                pin = ps_big.tile([P, 512], F32, name="grupi", tag="mmp")
                nc.tensor.matmul(pin[:, 0:w_], wih[:, 2 * P:3 * P],
                                 HXb[:, c0:c1], start=True, stop=True)
                phn = ps_big.tile([P, 512], F32, name="gruph", tag="mmp")
                nc.tensor.matmul(phn[:, 0:w_], whh[:, 2 * P:3 * P],
                                 XCb[:, c0:c1], start=True, stop=True)
                hn = sp.tile([P, 512], F32, name="gruhn", tag="gruhn", bufs=1)
                nc.vector.tensor_scalar(out=hn[:, 0:w_], in0=phn[:, 0:w_],
                                        scalar1=Wt[pre + "bhh"][:, 2:3],
                                        scalar2=None, op0=OP.add)
                nc.vector.tensor_mul(hn[:, 0:w_], hn[:, 0:w_], r[:, 0:w_])
                nc.vector.tensor_tensor(out=hn[:, 0:w_], in0=hn[:, 0:w_],
                                        in1=pin[:, 0:w_], op=OP.add)
                n_t = sp.tile([P, 512], F32, name="grun", tag="grun", bufs=1)
                nc.scalar.activation(n_t[:, 0:w_], hn[:, 0:w_], AF.Tanh,
                                     bias=Wt[pre + "bih"][:, 2:3])
                d_t = sp.tile([P, 512], F32, name="grud", tag="grud", bufs=1)
                nc.vector.tensor_sub(d_t[:, 0:w_], XC[:, c0:c1], n_t[:, 0:w_])
                nc.vector.tensor_mul(d_t[:, 0:w_], d_t[:, 0:w_], z[:, 0:w_])
                nc.vector.tensor_tensor(out=d_t[:, 0:w_], in0=d_t[:, 0:w_],
                                        in1=n_t[:, 0:w_], op=OP.add)
                nc.scalar.activation(XC[:, c0:c1], d_t[:, 0:w_], AF.Relu)

        def build_table(srcb, srcRep2, dstRep2, ad_from_xc, li):
            for tp in range(HT):
                cc = tp * P
                pT = ps.tile([P, P], BF, name="tabT", tag="miscP")
                nc.tensor.transpose(out=pT[:], in_=srcb[:, cc:cc + P],
                                    identity=identb[:])
                row3 = row_all[:, tp * 2 * WG:(tp + 1) * 2 * WG].rearrange(
                    "p (h w) -> p h w", w=WG)
                nc.vector.tensor_copy(
                    out=row3[:, :, 0:D],
                    in_=pT[:].rearrange("q (h f) -> q h f", f=D))
                if srcRep2 is None:
                    nc.vector.memset(row3[:, :, D:W], 0.0)
                else:
                    tmp = sp.tile([P, P], F32, name="tabm", tag="tabm")
                    nc.vector.tensor_tensor(out=tmp[:], in0=pT[:],
                                            in1=Wt[srcRep2][:], op=OP.mult)
                    asr = sp.tile([P, 2], F32, name="asr", tag="asr")
                    nc.vector.tensor_reduce(asr[:].unsqueeze(2),
                                            tmp[:].rearrange("q (h f) -> q h f", f=D),
                                            axis=AX.X, op=OP.add)
                    nc.vector.tensor_copy(out=row3[:, :, D:W],
                                          in_=asr[:].unsqueeze(2))
                if ad_from_xc:
                    pTx = ps.tile([P, P], BF, name="tabTx", tag="miscP")
                    nc.tensor.transpose(out=pTx[:], in_=XCb[:, cc:cc + P],
                                        identity=identb[:])
                    dsrc = pTx
                else:
                    dsrc = pT
                tmp2 = sp.tile([P, P], F32, name="tabm2", tag="tabm2")
                nc.vector.tensor_tensor(out=tmp2[:], in0=dsrc[:],
                                        in1=Wt[dstRep2][:], op=OP.mult)
                nc.vector.tensor_reduce(ad_nm[:, 2 * tp:2 * tp + 2].unsqueeze(2),
                                        tmp2[:].rearrange("q (h f) -> q h f", f=D),
                                        axis=AX.X, op=OP.add)
            nc.sync.dma_start(
                out=tbl_locs[li][:].rearrange("(tp h p) w -> p tp h w", h=2, p=P),
                in_=row_all[:])
            # alpha_dst transposed: adTs[j, q] = ad of node (tile j, row q)
            pAd = ps.tile([P, P], F32, name="adT", tag="miscP")
            nc.tensor.transpose(out=pAd[0:t_tiles, :], in_=ad_nm[:],
                                identity=ident[:])
            nc.vector.tensor_copy(out=adTs[0:t_tiles, :], in_=pAd[0:t_tiles, :])
            nc.gpsimd.collective_compute(
                "AllGather", mybir.AluOpType.bypass,
                replica_groups=[list(range(NC))],
                ins=[tbl_locs[li].opt()], outs=[tbl_alls[li].opt()])

        def edge_phase(is_gate, li):
            """Message round; writes agg (normalized, fp16) into HXb (fm)."""
            col0 = 0
            qrr = 0
            for tp in range(HT):
                calls = meta[tp]
                TC = sum(cl[1] for cl in calls)
                ar2 = sp.tile([1, 2 * P], BF, name="ar2", tag="ar2")
                nc.sync.dma_start(out=ar2[0:1, :], in_=adTs[2 * tp:2 * tp + 2, :])
                prp = ps.tile([P, 2 * P], F32, name="repP", tag="rowP")
                nc.tensor.matmul(prp[:], ones1[:], ar2[0:1, :],
                                 start=True, stop=True)
                rep_pair = sp.tile([P, 2 * P], BF, name="rep_pair", tag="rep_pair")
                nc.vector.tensor_copy(out=rep_pair[:], in_=prp[:])
                gt = ep.tile([P, MXC * WG], BF, name="gt", tag="gt", bufs=3)
                cc = 0
                for w_, ncols, acols, bcol0, amax, bmax in calls:
                    r0 = w_ * WIN
                    r1 = min((w_ + 1) * WIN, NC * n_pad)
                    nc.gpsimd.dma_gather(
                        out_ap=gt[:, cc * WG:(cc + ncols) * WG].rearrange(
                            "p (c w) -> p c w", w=WG),
                        in_ap=tbl_alls[li][r0:r1, :],
                        idxs_ap=idx_sb[:, (col0 + cc) * 8:(col0 + cc + ncols) * 8],
                        num_idxs=ncols * P, num_idxs_reg=ncols * P,
                        elem_size=WG, queue_num=qrr % 4)
                    qrr += 1
                    cc += ncols
                gt3 = gt[:].rearrange("p (c w) -> p c w", w=WG)
                s256 = ep.tile([P, MXC * 2 * P], BF, name="s256", tag="s256")
                nc.sync.dma_start(out=s256[:, 0:TC * 2 * P],
                                  in_=s256_h[:, col0 * 2 * P:(col0 + TC) * 2 * P])
                sat = ep.tile([P, MXC * 2 * P], BF, name="sat", tag="sat", bufs=1)
                nc.vector.tensor_tensor(
                    out=sat[:, :TC * 2 * P].rearrange("p (c q) -> p c q", q=2 * P),
                    in0=s256[:, :TC * 2 * P].rearrange("p (c q) -> p c q", q=2 * P),
                    in1=rep_pair[:].unsqueeze(1).to_broadcast([P, TC, 2 * P]),
                    op=OP.mult)
                s3 = sat[:].rearrange("p (c q) -> p c q", q=2 * P)
                wdt = P
                while wdt >= 1:
                    nc.vector.tensor_tensor(
                        out=s3[:, 0:TC, 0:wdt], in0=s3[:, 0:TC, 0:wdt],
                        in1=s3[:, 0:TC, wdt:2 * wdt], op=OP.add)
                    wdt //= 2
                aslot = sp.tile([P, MXC], F32, name="aslot", tag="aslot")
                nc.vector.tensor_copy(out=aslot[:, 0:TC],
                                      in_=s3[:, 0:TC, 0:1].squeeze(2))
                q = sp.tile([P, MXC], F32, name="q", tag="q")
                if is_gate:
                    be_sb = ep.tile([P, MXC * D], BF, name="be_sb", tag="be_sb", bufs=1)
                    nc.sync.dma_start(out=be_sb[:, 0:TC * D],
                                      in_=be_h[:, col0 * D:(col0 + TC) * D])
                    m_all = ep.tile([P, MXC * D], BF, name="m_all", tag="m_all")
                    m3 = m_all[:].rearrange("p (c w) -> p c w", w=D)
                    nc.vector.tensor_tensor(
                        out=m3[:, 0:TC, :], in0=gt3[:, 0:TC, 0:D],
                        in1=be_sb[:, 0:TC * D].rearrange("p (c w) -> p c w", w=D),
                        op=OP.add)
                    nc.scalar.activation(m_all[:, :TC * D], m_all[:, :TC * D],
                                         AF.Prelu, alpha=NEG)
                    lt = ep.tile([P, MXC * D], BF, name="lt", tag="lt", bufs=1)
                    nc.vector.tensor_tensor(
                        out=lt[:, :TC * D], in0=m3[:, 0:TC, :],
                        in1=Wt["attlRep"][:].unsqueeze(1).to_broadcast([P, TC, D]),
                        op=OP.mult)
                    nc.vector.tensor_reduce(q[:, 0:TC].unsqueeze(2),
                                            lt[:, :TC * D].rearrange(
                                                "p (c w) -> p c w", w=D),
                                            axis=AX.X, op=OP.add)
                    nc.vector.tensor_tensor(out=q[:, 0:TC], in0=q[:, 0:TC],
                                            in1=aslot[:, 0:TC], op=OP.add)
                    msg3 = m3
                else:
                    nc.vector.tensor_tensor(out=q[:, 0:TC],
                                            in0=gt3[:, 0:TC, D:D + 1].squeeze(2),
                                            in1=aslot[:, 0:TC], op=OP.add)
                    msg3 = gt3
                e_t = sp.tile([P, MXC], F32, name="e_t", tag="e_t")
                nc.scalar.activation(e_t[:, 0:TC], q[:, 0:TC], AF.Prelu, alpha=NEG)
                nc.scalar.activation(e_t[:, 0:TC], e_t[:, 0:TC], AF.Exp)
                rhs = ep.tile([P, MXC * W], BF, name="rhs", tag="rhs")
                r3 = rhs[:].rearrange("p (c w) -> p c w", w=W)
                nc.vector.tensor_tensor(
                    out=r3[:, 0:TC, 0:D], in0=msg3[:, 0:TC, 0:D],
                    in1=e_t[:, 0:TC].unsqueeze(2).to_broadcast([P, TC, D]),
                    op=OP.mult)
                nc.vector.tensor_copy(out=r3[:, 0:TC, D:W],
                                      in_=e_t[:, 0:TC].unsqueeze(2))
                amms = []
                bmms = []
                cc = 0
                for w_, ncols, acols, bcol0, amax, bmax in calls:
                    if amax:
                        amms += [cc + c for c in range(acols)]
                    if bmax:
                        bmms += [cc + c for c in range(bcol0, ncols)]
                    cc += ncols
                aggb = sp.tile([P, P], BF, name="aggb", tag="aggb")
                for h, mms, qofs in ((0, amms, 0), (1, bmms, P)):
                    pseg = ps_seg.tile([P, W], F32, name="pseg", tag="pseg")
                    for i, c in enumerate(mms):
                        nc.tensor.matmul(
                            pseg[:],
                            s256[:, c * 2 * P + qofs:c * 2 * P + qofs + P],
                            rhs[:, c * W:(c + 1) * W],
                            start=(i == 0), stop=(i == len(mms) - 1))
                    sn = sp.tile([P, 1], F32, name="sn", tag="sn")
                    nc.vector.tensor_single_scalar(out=sn[:], in_=pseg[:, D:W],
                                                   scalar=1e-16, op=OP.add)
                    rcp = sp.tile([P, 1], F32, name="rcp", tag="rcp")
                    nc.vector.reciprocal(rcp[:], sn[:])
                    nc.vector.tensor_tensor(out=aggb[:, h * D:(h + 1) * D],
                                            in0=pseg[:, 0:D],
                                            in1=rcp[:].to_broadcast([P, D]),
                                            op=OP.mult)
                pT2 = ps.tile([P, P], F32, name="aggT", tag="miscP")
                nc.tensor.matmul(pT2[:], aggb[:], identb[:], start=True, stop=True)
                nc.vector.tensor_copy(out=HXb[:, tp * P:(tp + 1) * P], in_=pT2[:])
                col0 += TC

        # ================= forward =================
        for ci in range(NCH):
            c0, c1 = ci * 512, min((ci + 1) * 512, HC)
            xin = sp.tile([P, 512], BF, name="xin", tag="xin")
            nc.sync.dma_start(out=xin[:, :c1 - c0], in_=xfm_h[:, c0:c1])
            pt = ps_big.tile([P, 512], F32, name="mmp0", tag="mmp")
            nc.tensor.matmul(pt[:, 0:c1 - c0], Wt["lin1_bd"][:],
                             xin[:, 0:c1 - c0], start=True, stop=True)
            nc.scalar.activation(XC[:, c0:c1], pt[:, 0:c1 - c0],
                                 AF.Prelu, bias=Wt["lin1_b"][:, 0:1], alpha=NEG)
        nc.vector.tensor_copy(out=XCb[:], in_=XC[:])
        # --- GATEConv ---
        mm_node(HXb, "gate_w1a_bd", XCb)
        build_table(HXb, None, "gateattrRep2", True, 0)
        edge_phase(True, 0)
        mm_node(HXb, "gate_w2_bd", HXb)
        elu_inplace(HXb, "gate_bias")
        gru_relu("gru0_")
        nc.vector.tensor_copy(out=XCb[:], in_=XC[:])
        # --- atom layers ---
        for l in range(4):
            pre = f"at{l}_"
            mm_node(HXb, pre + "wT", XCb)
            build_table(HXb, pre + "srcRep2", pre + "dstRep2", False, 1 + l)
            edge_phase(False, 1 + l)
            elu_inplace(HXb, pre + "bias")
            gru_relu(pre + "gru_")
            nc.vector.tensor_copy(out=XCb[:], in_=XC[:])

        # ================= readout =================
        mm_node(HXb, "mol_bd", XCb)          # xs into HXb
        asrc_nm = st.tile([P, t_tiles], F32, name="asrc_nm")
        for tp in range(HT):
            pT = ps.tile([P, P], BF, name="xsT", tag="miscP")
            nc.tensor.transpose(out=pT[:], in_=HXb[:, tp * P:(tp + 1) * P],
                                identity=identb[:])
            nc.vector.tensor_copy(out=row_all[:, tp * 2 * D:(tp + 1) * 2 * D],
                                  in_=pT[:])
            tmp = sp.tile([P, P], F32, name="xsm", tag="tabm")
            nc.vector.tensor_tensor(out=tmp[:], in0=pT[:],
                                    in1=Wt["molsrcRep2"][:], op=OP.mult)
            nc.vector.tensor_reduce(asrc_nm[:, 2 * tp:2 * tp + 2].unsqueeze(2),
                                    tmp[:].rearrange("q (h f) -> q h f", f=D),
                                    axis=AX.X, op=OP.add)
        for tp in range(HT):
            pT = ps.tile([P, P], BF, name="xcT", tag="miscP")
            nc.tensor.transpose(out=pT[:], in_=XCb[:, tp * P:(tp + 1) * P],
                                identity=identb[:])
            nc.vector.tensor_copy(out=XCb[:, tp * P:(tp + 1) * P], in_=pT[:])
        xc_nm = XCb
        xs_nm = row_all
        OUTT = st.tile([D, 2 * P], F32, name="OUTT")
        for k in range(2):
            pg = ps_seg.tile([P, D], F32, name="pg", tag="pseg")
            for tp in range(HT):
                sgp = sp.tile([P, 2 * 2 * P], BF, name="sgp", tag="sgp")
                for h in range(2):
                    nc.vector.tensor_scalar(
                        out=sgp[:, h * 2 * P:(h + 1) * 2 * P], in0=iota_sb[:],
                        scalar1=grel32[:, 2 * tp + h:2 * tp + h + 1],
                        scalar2=None, op0=OP.is_equal)
                for h in range(2):
                    j = 2 * tp + h
                    nc.tensor.matmul(
                        pg[:],
                        sgp[:, h * 2 * P + k * P:h * 2 * P + (k + 1) * P],
                        xc_nm[:, j * D:(j + 1) * D],
                        start=(j == 0), stop=(j == t_tiles - 1))
            og = sp.tile([P, D], F32, name="og", tag="og")
            nc.scalar.activation(og[:], pg[:], AF.Relu)
            pTo = ps.tile([D, P], F32, name="ogT", tag="miscP")
            nc.tensor.transpose(out=pTo[:], in_=og[:], identity=ident[:])
            nc.vector.tensor_copy(out=OUTT[:, k * P:(k + 1) * P], in_=pTo[:])
        HG = st.tile([D, 2 * P], F32, name="HG")
        for ts in range(3):
            pxd = ps_big.tile([D, 512], F32, name="xdp", tag="mmp")
            nc.tensor.matmul(pxd[:, 0:2 * P], Wt["mol_wT32"][:], OUTT[:],
                             start=True, stop=True)
            xds = sp.tile([D, 2 * P], F32, name="xds", tag="xds", bufs=1)
            nc.vector.tensor_copy(out=xds[:], in_=pxd[:, 0:2 * P])
            pag = ps.tile([1, 2 * P], F32, name="agp", tag="rowP")
            nc.tensor.matmul(pag[:], Wt["moldstCol"][:], xds[:],
                             start=True, stop=True)
            agr = sp.tile([1, 2 * P], BF, name="agr", tag="agr")
            nc.vector.tensor_copy(out=agr[:], in_=pag[:])
            prg = ps.tile([P, 2 * P], F32, name="repG", tag="rowP")
            nc.tensor.matmul(prg[:], ones1[:], agr[:], start=True, stop=True)
            rep_agr = sp.tile([P, 2 * P], BF, name="rep_agr", tag="rep_agr", bufs=1)
            nc.vector.tensor_copy(out=rep_agr[:], in_=prg[:])
            ag2 = sp.tile([P, 2], F32, name="ag2", tag="ag2")
            psg = [ps_seg.tile([P, W], F32, name=f"psg{k}", tag="pseg")
                   for k in range(2)]
            for tp in range(HT):
                sgp = sp.tile([P, 2 * 2 * P], BF, name="sgp2", tag="sgp")
                for h in range(2):
                    nc.vector.tensor_scalar(
                        out=sgp[:, h * 2 * P:(h + 1) * 2 * P], in0=iota_sb[:],
                        scalar1=grel32[:, 2 * tp + h:2 * tp + h + 1],
                        scalar2=None, op0=OP.is_equal)
                scr2 = sp.tile([P, 2 * 2 * P], BF, name="scr2", tag="scr")
                g3 = scr2[:].rearrange("p (h q) -> p h q", q=2 * P)
                nc.vector.tensor_tensor(
                    out=g3[:],
                    in0=sgp[:].rearrange("p (h q) -> p h q", q=2 * P),
                    in1=rep_agr[:].unsqueeze(1).to_broadcast([P, 2, 2 * P]),
                    op=OP.mult)
                wdt = P
                while wdt >= 1:
                    nc.vector.tensor_tensor(
                        out=g3[:, :, 0:wdt], in0=g3[:, :, 0:wdt],
                        in1=g3[:, :, wdt:2 * wdt], op=OP.add)
                    wdt //= 2
                nc.vector.tensor_copy(out=ag2[:], in_=g3[:, :, 0:1].squeeze(2))
                q2 = sp.tile([P, 2], F32, name="q2", tag="q2")
                nc.vector.tensor_tensor(out=q2[:], in0=asrc_nm[:, 2 * tp:2 * tp + 2],
                                        in1=ag2[:], op=OP.add)
                nc.scalar.activation(q2[:], q2[:], AF.Prelu, alpha=NEG)
                nc.scalar.activation(q2[:], q2[:], AF.Exp)
                rh = ep.tile([P, 2 * W], BF, name="rh", tag="rh", bufs=2)
                rh3 = rh[:].rearrange("p (h w) -> p h w", w=W)
                nc.vector.tensor_tensor(
                    out=rh3[:, :, 0:D],
                    in0=xs_nm[:, tp * 2 * D:(tp + 1) * 2 * D].rearrange(
                        "p (h f) -> p h f", f=D),
                    in1=q2[:].unsqueeze(2).to_broadcast([P, 2, D]),
                    op=OP.mult)
                nc.vector.tensor_copy(out=rh3[:, :, D:W], in_=q2[:].unsqueeze(2))
                for h in range(2):
                    j = 2 * tp + h
                    for k in range(2):
                        nc.tensor.matmul(
                            psg[k][:],
                            sgp[:, h * 2 * P + k * P:h * 2 * P + (k + 1) * P],
                            rh[:, h * W:(h + 1) * W],
                            start=(j == 0), stop=(j == t_tiles - 1))
            for k in range(2):
                sn = sp.tile([P, 1], F32, name="sng", tag="sn")
                nc.vector.tensor_single_scalar(out=sn[:], in_=psg[k][:, D:W],
                                               scalar=1e-16, op=OP.add)
                rcp = sp.tile([P, 1], F32, name="rcpg", tag="rcp")
                nc.vector.reciprocal(rcp[:], sn[:])
                aggg = sp.tile([P, D], F32, name="aggg", tag="aggg")
                nc.vector.tensor_tensor(out=aggg[:], in0=psg[k][:, 0:D],
                                        in1=rcp[:].to_broadcast([P, D]), op=OP.mult)
                nc.vector.tensor_tensor(out=aggg[:], in0=aggg[:],
                                        in1=Wt["mol_biasRep"][:], op=OP.add)
                r = sp.tile([P, D], F32, name="rg", tag="rg")
                nc.scalar.activation(r[:], aggg[:], AF.Relu)
                xm = sp.tile([P, D], F32, name="xmg", tag="xmg")
                nc.vector.tensor_sub(xm[:], aggg[:], r[:])
                nc.scalar.activation(xm[:], xm[:], AF.Exp)
                nc.vector.scalar_tensor_tensor(out=aggg[:], in0=xm[:], scalar=-1.0,
                                               in1=r[:], op0=OP.add, op1=OP.add)
                pTh = ps.tile([D, P], F32, name="hgT", tag="miscP")
                nc.tensor.transpose(out=pTh[:], in_=aggg[:], identity=ident[:])
                nc.vector.tensor_copy(out=HG[:, k * P:(k + 1) * P], in_=pTh[:])
            wih = Wt["mol_gru_wih"]
            whh = Wt["mol_gru_whh"]
            bs = Wt["mol_gru_bsum"]
            def gate2g(g):
                pt = ps_big.tile([D, 512], F32, name="ggp", tag="mmp")
                nc.tensor.matmul(pt[:, 0:2 * P], wih[:, g * D:(g + 1) * D], HG[:],
                                 start=True, stop=False)
                nc.tensor.matmul(pt[:, 0:2 * P], whh[:, g * D:(g + 1) * D], OUTT[:],
                                 start=False, stop=True)
                return pt
            prg2 = gate2g(0)
            rg2 = sp.tile([D, 2 * P], F32, name="ggr", tag="ggr", bufs=1)
            nc.scalar.activation(rg2[:], prg2[:, 0:2 * P], AF.Sigmoid, bias=bs[:, 0:1])
            pzg = gate2g(1)
            zg = sp.tile([D, 2 * P], F32, name="ggz", tag="ggz", bufs=1)
            nc.scalar.activation(zg[:], pzg[:, 0:2 * P], AF.Sigmoid, bias=bs[:, 1:2])
            pig = ps_big.tile([D, 512], F32, name="ggpi", tag="mmp")
            nc.tensor.matmul(pig[:, 0:2 * P], wih[:, 2 * D:3 * D], HG[:],
                             start=True, stop=True)
            phg = ps_big.tile([D, 512], F32, name="ggph", tag="mmp")
            nc.tensor.matmul(phg[:, 0:2 * P], whh[:, 2 * D:3 * D], OUTT[:],
                             start=True, stop=True)
            hng = sp.tile([D, 2 * P], F32, name="gghn", tag="gghn", bufs=1)
            nc.vector.tensor_scalar(out=hng[:], in0=phg[:, 0:2 * P],
                                    scalar1=Wt["mol_gru_bhh"][:, 2:3],
                                    scalar2=None, op0=OP.add)
            nc.vector.tensor_mul(hng[:], hng[:], rg2[:])
            nc.vector.tensor_tensor(out=hng[:], in0=hng[:], in1=pig[:, 0:2 * P],
                                    op=OP.add)
            ng = sp.tile([D, 2 * P], F32, name="ggn", tag="ggn", bufs=1)
            nc.scalar.activation(ng[:], hng[:], AF.Tanh,
                                 bias=Wt["mol_gru_bih"][:, 2:3])
            dg = sp.tile([D, 2 * P], F32, name="ggd", tag="ggd", bufs=1)
            nc.vector.tensor_sub(dg[:], OUTT[:], ng[:])
            nc.vector.tensor_mul(dg[:], dg[:], zg[:])
            nc.vector.tensor_tensor(out=dg[:], in0=dg[:], in1=ng[:], op=OP.add)
            nc.scalar.activation(OUTT[:], dg[:], AF.Relu)
        py = ps.tile([1, 2 * P], F32, name="py", tag="rowP")
        nc.tensor.matmul(py[:], Wt["lin2_wT"][:], OUTT[:], start=True, stop=True)
        ysb = sp.tile([1, 2 * P], F32, name="ysb", tag="ysb")
        nc.vector.tensor_scalar(out=ysb[:], in0=py[:], scalar1=Wt["lin2_b"][0:1, 0:1],
                                scalar2=None, op0=OP.add)
        nc.sync.dma_start(out=y_out[:], in_=ysb[0:1, 0:G_LOC])
    nc.compile()
    return nc


_CACHE = {}


def kernel(**inputs):
    from concourse.bass_utils import run_bass_kernel_spmd
    x = np.asarray(inputs["x"], np.float32)
    ei = np.asarray(inputs["edge_index"])
    ea = np.asarray(inputs["edge_attr"], np.float32)
    bt = np.asarray(inputs["batch"])
    per, n_pad, t_tiles, meta, TOTC = _prep(x, ei, ea, bt)
    kwf = {k: np.asarray(v, np.float32) for k, v in inputs.items()
           if k not in ("x", "edge_index", "edge_attr", "batch")}
    weights = _mk_weights(kwf)
    key = (n_pad, TOTC, tuple(tuple(tuple(cl) for cl in calls) for calls in meta))
    if key not in _CACHE:
        _CACHE[key] = _build(n_pad, t_tiles, meta, TOTC,
                             {k: (v.shape, v.dtype == F16)
                              for k, v in weights.items()})
    nc = _CACHE[key]
    iota = np.tile(np.arange(2 * P).astype(np.float32)[None, :], (P, 1)).astype(F16)
    w1b = kwf["gate_lin1_w"][:, D:]
    in_maps = []
    for c in range(NC):
        b_e = (per[c]["attr_s"] @ w1b.T).astype(F16)
        b_e = np.ascontiguousarray(
            b_e.reshape(TOTC, P, D).transpose(1, 0, 2).reshape(P, TOTC * D))
        m = dict(xfm=per[c]["xfm"], idx16=per[c]["idx16"], s256=per[c]["s256"],
                 b_e=b_e, grel=per[c]["grel"], iota256=iota)
        for k, v in weights.items():
            m["w_" + k] = v
        in_maps.append(m)
    res = run_bass_kernel_spmd(nc, in_maps, core_ids=list(range(NC)))
    return np.concatenate([res.results[c]["y"][0] for c in range(NC)]).astype(np.float32)


# revision 20
# speedup vs baseline: 1.8703x; 1.3578x over previous
"""AttentiveFP forward on 8 Trainium2 NeuronCores (Bass/Tile).

Sharding: 2048 graphs (nodes contiguous, batch sorted) split into 8 blocks of
256 graphs; each core owns the edges whose dst node falls in its block. Per
round each core computes its nodes' features, all-gathers a compact per-node
table [xt | alpha_src] (fp16, 65 wide), expands it locally to 256B-aligned
rows, then fetches per-edge src rows with nc.gpsimd.dma_gather (the token
gather ucode: thousands of int16 indices per call, round-robined over 4 SWDGE
queues). Indices are int16, so slots are grouped per (node-tile-pair,
32768-row source window); within a call, tile-a slots carry rel in [0,128)
and tile-b slots rel in [128,256), so one 256-wide is_equal one-hot serves
both tiles' PSUM segment-matmuls and the alpha_dst select (one-hot x
replicated alpha row, reduced on DVE). Per-edge alpha_dst needs no gather
(dst is always local). Node phases run feature-major, half-packed, with
block-diagonal [128,128] fp16 weights; GRU hidden state stays fp32. The gate
round's edge-attr term (W1b @ e_attr) is precomputed on the host per slot.
Readout uses a 256-wide graph one-hot per tile-pair plus a replicated
per-graph alpha row (no gathers).

Softmax max-subtraction is skipped (logits O(1), shift-invariant).
"""
import sys
sys.path.insert(0, '/opt/trn_rl_repo')
sys.path.insert(0, '/root/.axon_site')

import numpy as np

F16 = np.float16
NC = 8
D = 64
G_TOT = 2048
G_LOC = G_TOT // NC
F_IN = 25
E_DIM = 4
NEG = 0.01
P = 128
W = 65            # compact table row: [xt(64) | alpha_src]
WG = 128          # gathered row width (256B-aligned)
WIN = 32768       # int16 index window (rows)


def _prep(x, edge_index, edge_attr, batch):
    src = edge_index[0].astype(np.int64)
    dst = edge_index[1].astype(np.int64)
    batch = batch.astype(np.int64)

    gstart = np.searchsorted(batch, np.arange(0, G_TOT + 1, G_LOC))
    n0 = gstart[:-1]
    nloc = np.diff(gstart)
    n_pad = int(np.ceil((nloc.max() + 1) / 256) * 256)
    t_tiles = n_pad // P
    HC = n_pad // 2
    HT = t_tiles // 2
    NW = 4
    bp = HT // NW
    pr_cnt = [bp + (1 if i < HT % NW else 0) for i in range(NW)]
    sbp = np.cumsum([0] + pr_cnt)                 # pair boundaries per slice
    sbr = sbp * 2 * P                             # row boundaries per slice
    srows = np.diff(sbr)

    def pi_row(n):
        h = n // HC
        r = n % HC
        return (2 * (r // P) + h) * P + (r % P)

    src_dev = np.searchsorted(gstart[1:], src, side='right')
    dst_dev = np.searchsorted(gstart[1:], dst, side='right')
    pr_all = pi_row(src - n0[src_dev])
    w_all = np.searchsorted(sbr[1:], pr_all, side='right')
    blk0 = np.concatenate([[0], np.cumsum(srows * NC)])
    gidx_all = (blk0[w_all] + src_dev * srows[w_all]
                + (pr_all - sbr[w_all]))

    # ---- pass 1: bucket edges per core into (pair, window, half) ----
    buckets = [[[[None, None] for _ in range(NW)] for _ in range(HT)]
               for _ in range(NC)]
    for c in range(NC):
        sel = np.where(dst_dev == c)[0]
        dl = dst[sel] - n0[c]
        j_dst = 2 * ((dl % HC) // P) + dl // HC
        p_dst = dl % P
        gi = gidx_all[sel]
        w_of = np.searchsorted(blk0[1:], gi, side='right')
        for tp in range(HT):
            for h in range(2):
                m = j_dst == 2 * tp + h
                gi_m, p_m, w_m, sel_m = gi[m], p_dst[m], w_of[m], sel[m]
                for w in range(NW):
                    mm = w_m == w
                    buckets[c][tp][w][h] = (gi_m[mm] - blk0[w], p_m[mm],
                                            sel_m[mm])
    # ---- pass 2: SPMD-uniform call metadata (max counts over cores) ----
    meta = []       # per pair: [w, ncols, acols, bcol0, amax, bmax]
    for tp in range(HT):
        calls = []
        for w in range(NW):
            amax = max(len(buckets[c][tp][w][0][0]) for c in range(NC))
            bmax = max(len(buckets[c][tp][w][1][0]) for c in range(NC))
            if amax + bmax == 0:
                continue
            ncols = (amax + bmax + P - 1) // P
            calls.append([w, ncols, (amax + P - 1) // P, amax // P, amax, bmax])
        if not any(cl[4] for cl in calls):
            calls.insert(0, [0, 1, 1, 0, P, 0])
        if not any(cl[5] for cl in calls):
            calls.append([0, 1, 0, 0, 0, P])
        meta.append(calls)
    TOTC = sum(cl[1] for calls in meta for cl in calls)

    per = []
    for c in range(NC):
        idx16 = np.zeros((16, TOTC * 8), np.int16)
        rel = np.full((P, TOTC), 300.0, np.float32)
        attr_s = np.zeros((TOTC * P, E_DIM), np.float32)
        col0 = 0
        for tp in range(HT):
            for w_, ncols, acols, bcol0, amax, bmax in meta[tp]:
                flat_idx = np.zeros(ncols * P, np.int16)
                flat_rel = np.full(ncols * P, 300.0, np.float32)
                flat_attr = np.zeros((ncols * P, E_DIM), np.float32)
                pos = 0
                for h, hmax in ((0, amax), (1, bmax)):
                    gi_l, p_l, sel_l = buckets[c][tp][w_][h]
                    k = len(gi_l)
                    flat_idx[pos:pos + k] = gi_l.astype(np.int16)
                    flat_rel[pos:pos + k] = p_l + h * P
                    flat_attr[pos:pos + k] = edge_attr[sel_l]
                    pos += hmax
                idx16[:, col0 * 8:(col0 + ncols) * 8] = \
                    flat_idx.reshape(ncols * 8, 16).T
                rel[:, col0:col0 + ncols] = flat_rel.reshape(ncols, P).T
                attr_s[col0 * P:(col0 + ncols) * P] = flat_attr
                col0 += ncols
        s256 = (rel.astype(np.int32)[:, :, None] ==
                np.arange(2 * P, dtype=np.int32)[None, None, :]).astype(F16)
        per.append(dict(idx16=np.tile(idx16, (8, 1)),
                        s256=np.ascontiguousarray(s256.reshape(P, TOTC * 2 * P)),
                        attr_s=attr_s))
        nl = int(nloc[c])
        gl = batch[n0[c]:n0[c] + nl] - G_LOC * c
        grel = np.full((P, t_tiles), 300.0, np.float32)
        n_ids = np.arange(n_pad)
        h_a = n_ids // HC
        j_a = 2 * ((n_ids % HC) // P) + h_a
        p_a = n_ids % P
        valid = n_ids < nl
        grel[p_a[valid], j_a[valid]] = gl[n_ids[valid]]
        per[c]['grel'] = np.ascontiguousarray(grel.astype(F16))
        xp = np.zeros((n_pad, F_IN), np.float32)
        xp[:nl] = x[n0[c]:n0[c] + nl]
        xfm = np.zeros((P, HC), np.float32)
        xfm[:F_IN] = xp[:HC].T
        xfm[D:D + F_IN] = xp[HC:].T
        per[c]['xfm'] = xfm.astype(F16)
    return per, n_pad, t_tiles, (meta, [int(v) for v in sbp]), TOTC


def _mk_weights(kw):
    w = {}
    def bd(a):
        t = a.T
        z = np.zeros((P, P), np.float32)
        z[0:D, 0:D] = t
        z[D:2 * D, D:2 * D] = t
        return z
    def col(a):
        return np.concatenate([a, a])[:, None]
    def rep2(a):
        return np.tile(np.concatenate([a, a])[None, :], (P, 1))
    def rep1(a):
        return np.tile(a[None, :], (P, 1))
    def gb(a):
        t = a.reshape(3, D).T
        return np.concatenate([t, t], 0)
    def gru_bd(wg):
        out = np.zeros((P, 3 * P), np.float32)
        for g in range(3):
            out[:, g * P:(g + 1) * P] = bd(wg[g * D:(g + 1) * D])
        return out

    B, F = 'b', 'f'
    lin1 = np.zeros((P, P), np.float32)
    lin1[0:F_IN, 0:D] = kw["lin1_w"].T
    lin1[D:D + F_IN, D:2 * D] = kw["lin1_w"].T
    w["lin1_bd"] = (lin1, B)
    w["lin1_b"] = (col(kw["lin1_b"]), F)
    w["gate_w1a_bd"] = (bd(kw["gate_lin1_w"][:, :D]), B)
    w["attlRep"] = (rep1(kw["gate_att_l"]), B)
    w["gateattrRep2"] = (rep2(kw["gate_att_r"]), B)
    w["gate_w2_bd"] = (bd(kw["gate_lin2_w"]), B)
    w["gate_bias"] = (col(kw["gate_bias"]), F)
    w["gru0_wih"] = (gru_bd(kw["gru0_wih"]), B)
    w["gru0_whh"] = (gru_bd(kw["gru0_whh"]), B)
    w["gru0_bih"] = (gb(kw["gru0_bih"]), F)
    w["gru0_bhh"] = (gb(kw["gru0_bhh"]), F)
    w["gru0_bsum"] = (gb(kw["gru0_bih"] + kw["gru0_bhh"]), F)
    for l in range(4):
        pre = f"at{l}_"
        w[pre + "wT"] = (bd(kw["atom_lin_w"][l]), B)
        w[pre + "srcRep2"] = (rep2(kw["atom_att_src"][l]), B)
        w[pre + "dstRep2"] = (rep2(kw["atom_att_dst"][l]), B)
        w[pre + "bias"] = (col(kw["atom_bias"][l]), F)
        w[pre + "gru_wih"] = (gru_bd(kw["atom_gru_wih"][l]), B)
        w[pre + "gru_whh"] = (gru_bd(kw["atom_gru_whh"][l]), B)
        w[pre + "gru_bih"] = (gb(kw["atom_gru_bih"][l]), F)
        w[pre + "gru_bhh"] = (gb(kw["atom_gru_bhh"][l]), F)
        w[pre + "gru_bsum"] = (gb(kw["atom_gru_bih"][l] + kw["atom_gru_bhh"][l]), F)
    w["mol_bd"] = (bd(kw["mol_lin_w"]), B)
    w["mol_wT32"] = (kw["mol_lin_w"].T.copy(), F)
    w["molsrcRep2"] = (rep2(kw["mol_att_src"]), B)
    w["moldstCol"] = (kw["mol_att_dst"][:, None].copy(), F)
    w["mol_biasRep"] = (rep1(kw["mol_bias"]), F)
    w["mol_gru_wih"] = (kw["mol_gru_wih"].T.copy(), F)
    w["mol_gru_whh"] = (kw["mol_gru_whh"].T.copy(), F)
    w["mol_gru_bih"] = (gb(kw["mol_gru_bih"])[:D], F)
    w["mol_gru_bhh"] = (gb(kw["mol_gru_bhh"])[:D], F)
    w["mol_gru_bsum"] = (gb(kw["mol_gru_bih"] + kw["mol_gru_bhh"])[:D], F)
    w["lin2_wT"] = (kw["lin2_w"].T.copy(), F)
    w["lin2_b"] = (kw["lin2_b"][:, None].copy(), F)
    out = {}
    for k, (v, tag) in w.items():
        v = np.ascontiguousarray(v, np.float32)
        out[k] = v.astype(F16) if tag == B else v
    return out


def _build(n_pad, t_tiles, meta_in, TOTC, wmeta):
    meta, sbp = meta_in
    import concourse.bacc as bacc
    import concourse.mybir as mybir
    import concourse.tile as tile
    from concourse.masks import make_identity

    dt = mybir.dt
    AF = mybir.ActivationFunctionType
    OP = mybir.AluOpType
    AX = mybir.AxisListType
    BF = dt.float16
    F32 = dt.float32

    HC = n_pad // 2
    HT = t_tiles // 2
    NCH = (HC + 511) // 512
    MXC = max(sum(cl[1] for cl in calls) for calls in meta)
    NW = 4
    sbr = [b * 2 * P for b in sbp]                 # per-core slice row bounds
    srows = [sbr[i + 1] - sbr[i] for i in range(NW)]
    blk0 = [0]
    for i in range(NW):
        blk0.append(blk0[-1] + srows[i] * NC)

    nc = bacc.Bacc("TRN2", target_bir_lowering=False, debug=False, num_devices=NC,
                   num_swdge_queues=4)

    xfm_h = nc.dram_tensor("xfm", [P, HC], BF, kind="ExternalInput")
    idx_h = nc.dram_tensor("idx16", [P, TOTC * 8], dt.int16, kind="ExternalInput")
    s256_h = nc.dram_tensor("s256", [P, TOTC * 2 * P], BF, kind="ExternalInput")
    be_h = nc.dram_tensor("b_e", [P, TOTC * D], BF, kind="ExternalInput")
    grel_h = nc.dram_tensor("grel", [P, t_tiles], BF, kind="ExternalInput")
    iota_h = nc.dram_tensor("iota256", [P, 2 * P], BF, kind="ExternalInput")
    cst_h = {k: nc.dram_tensor("w_" + k, list(s_), BF if isbf else F32,
                               kind="ExternalInput")
             for k, (s_, isbf) in wmeta.items()}
    y_out = nc.dram_tensor("y", [1, G_LOC], F32, kind="ExternalOutput")

    with tile.TileContext(nc) as tc:
      with (
        tc.tile_pool(name="cst", bufs=1) as cst,
        tc.tile_pool(name="st", bufs=1) as st,
        tc.tile_pool(name="ep", bufs=2) as ep,
        tc.tile_pool(name="sp", bufs=2) as sp,
        tc.tile_pool(name="ps", bufs=2, space="PSUM") as ps,
        tc.tile_pool(name="ps_seg", bufs=2, space="PSUM") as ps_seg,
        tc.tile_pool(name="ps_big", bufs=2, space="PSUM") as ps_big,
        tc.tile_pool(name="dram", bufs=1, space="DRAM") as dp,
      ):
        def load(name):
            h = cst_h[name]
            t = cst.tile(list(h.shape), h.dtype, name="c_" + name)
            nc.sync.dma_start(out=t[:], in_=h[:])
            return t
        Wt = {k: load(k) for k in cst_h}
        idx_sb = cst.tile([P, TOTC * 8], dt.int16, name="idx_sb")
        nc.sync.dma_start(out=idx_sb[:], in_=idx_h[:])
        grel_sb = cst.tile([P, t_tiles], BF, name="grel_sb")
        nc.sync.dma_start(out=grel_sb[:], in_=grel_h[:])
        grel32 = cst.tile([P, t_tiles], F32, name="grel32")
        nc.vector.tensor_copy(out=grel32[:], in_=grel_sb[:])
        iota_sb = cst.tile([P, 2 * P], BF, name="iota_sb")
        nc.sync.dma_start(out=iota_sb[:], in_=iota_h[:])
        identb = cst.tile([P, P], BF, name="identb")
        make_identity(nc, identb[:])
        ident = cst.tile([P, P], F32, name="ident")
        make_identity(nc, ident[:])
        ones1 = cst.tile([1, P], BF, name="ones1")
        nc.vector.memset(ones1[:], 1.0)
        onesf = cst.tile([P, P], BF, name="onesf")
        nc.vector.memset(onesf[:], 1.0)

        XC = st.tile([P, HC], F32, name="XC")
        XCb = st.tile([P, HC], BF, name="XCb")
        HXb = st.tile([P, HC], BF, name="HXb")
        ad_nm = st.tile([P, t_tiles], F32, name="ad_nm")
        adTs = st.tile([P, P], BF, name="adTs")
        row_all = st.tile([P, HT * 2 * WG], BF, name="row_all")
        tbl_locs = [dp.tile([n_pad, WG], BF, name=f"tbl_loc{i}") for i in range(5)]
        tbl_alls = [[dp.tile([NC * srows[w], WG], BF, addr_space="Shared",
                             name=f"tbl_all{i}_{w}") for w in range(NW)]
                    for i in range(5)]

        def mm_node(dst, wkey, srcb, act=AF.Copy, bias=None, alpha=0.0):
            for ci in range(NCH):
                c0, c1 = ci * 512, min((ci + 1) * 512, HC)
                pt = ps_big.tile([P, 512], F32, name="mmp", tag="mmp")
                nc.tensor.matmul(pt[:, 0:c1 - c0], Wt[wkey][:], srcb[:, c0:c1],
                                 start=True, stop=True)
                b = Wt[bias][:, 0:1] if bias else 0.0
                nc.scalar.activation(dst[:, c0:c1], pt[:, 0:c1 - c0],
                                     act, bias=b, alpha=alpha)

        def elu_inplace(t_fm, bias):
            for ci in range(NCH):
                c0, c1 = ci * 512, min((ci + 1) * 512, HC)
                w_ = c1 - c0
                v = t_fm[:, c0:c1]
                tin = sp.tile([P, 512], F32, name="eluin", tag="eluin", bufs=1)
                nc.vector.tensor_scalar(out=tin[:, 0:w_], in0=v,
                                        scalar1=Wt[bias][:, 0:1],
                                        scalar2=None, op0=OP.add)
                r = sp.tile([P, 512], F32, name="elur", tag="elur", bufs=1)
                nc.scalar.activation(r[:, 0:w_], tin[:, 0:w_], AF.Relu)
                nc.vector.tensor_sub(tin[:, 0:w_], tin[:, 0:w_], r[:, 0:w_])
                nc.scalar.activation(tin[:, 0:w_], tin[:, 0:w_], AF.Exp)
                nc.vector.scalar_tensor_tensor(
                    out=v, in0=tin[:, 0:w_], scalar=-1.0,
                    in1=r[:, 0:w_], op0=OP.add, op1=OP.add)

        def gru_relu(pre):
            wih = Wt[pre + "wih"]
            whh = Wt[pre + "whh"]
            bs = Wt[pre + "bsum"]
            for ci in range(NCH):
                c0, c1 = ci * 512, min((ci + 1) * 512, HC)
                w_ = c1 - c0
                def gate2(g):
                    pt = ps_big.tile([P, 512], F32, name="grup", tag="mmp")
                    nc.tensor.matmul(pt[:, 0:w_], wih[:, g * P:(g + 1) * P],
                                     HXb[:, c0:c1], start=True, stop=False)
                    nc.tensor.matmul(pt[:, 0:w_], whh[:, g * P:(g + 1) * P],
                                     XCb[:, c0:c1], start=False, stop=True)
                    return pt
                pr = gate2(0)
                r = sp.tile([P, 512], F32, name="grur", tag="grur")
                nc.scalar.activation(r[:, 0:w_], pr[:, 0:w_], AF.Sigmoid,
                                     bias=bs[:, 0:1])
                pz = gate2(1)
                z = sp.tile([P, 512], F32, name="gruz", tag="gruz", bufs=1)
                nc.scalar.activation(z[:, 0:w_], pz[:, 0:w_], AF.Sigmoid,
                                     bias=bs[:, 1:2])
                pin = ps_big.tile([P, 512], F32, name="grupi", tag="mmp")
                nc.tensor.matmul(pin[:, 0:w_], wih[:, 2 * P:3 * P],
                                 HXb[:, c0:c1], start=True, stop=True)
                phn = ps_big.tile([P, 512], F32, name="gruph", tag="mmp")
                nc.tensor.matmul(phn[:, 0:w_], whh[:, 2 * P:3 * P],
                                 XCb[:, c0:c1], start=True, stop=True)
                hn = sp.tile([P, 512], F32, name="gruhn", tag="gruhn", bufs=1)
                nc.vector.tensor_scalar(out=hn[:, 0:w_], in0=phn[:, 0:w_],
                                        scalar1=Wt[pre + "bhh"][:, 2:3],
                                        scalar2=None, op0=OP.add)
                nc.vector.tensor_mul(hn[:, 0:w_], hn[:, 0:w_], r[:, 0:w_])
                nc.vector.tensor_tensor(out=hn[:, 0:w_], in0=hn[:, 0:w_],
                                        in1=pin[:, 0:w_], op=OP.add)
                n_t = sp.tile([P, 512], F32, name="grun", tag="grun", bufs=1)
                nc.scalar.activation(n_t[:, 0:w_], hn[:, 0:w_], AF.Tanh,
                                     bias=Wt[pre + "bih"][:, 2:3])
                d_t = sp.tile([P, 512], F32, name="grud", tag="grud", bufs=1)
                nc.vector.tensor_sub(d_t[:, 0:w_], XC[:, c0:c1], n_t[:, 0:w_])
                nc.vector.tensor_mul(d_t[:, 0:w_], d_t[:, 0:w_], z[:, 0:w_])
                nc.vector.tensor_tensor(out=d_t[:, 0:w_], in0=d_t[:, 0:w_],
                                        in1=n_t[:, 0:w_], op=OP.add)
                nc.scalar.activation(XC[:, c0:c1], d_t[:, 0:w_], AF.Relu)

        def build_table(srcb, srcRep2, dstRep2, ad_from_xc, li):
            for tp in range(HT):
                cc = tp * P
                pT = ps.tile([P, P], BF, name="tabT", tag="miscP")
                nc.tensor.transpose(out=pT[:], in_=srcb[:, cc:cc + P],
                                    identity=identb[:])
                row3 = row_all[:, tp * 2 * WG:(tp + 1) * 2 * WG].rearrange(
                    "p (h w) -> p h w", w=WG)
                nc.vector.tensor_copy(
                    out=row3[:, :, 0:D],
                    in_=pT[:].rearrange("q (h f) -> q h f", f=D))
                if srcRep2 is None:
                    nc.vector.memset(row3[:, :, D:W], 0.0)
                else:
                    tmp = sp.tile([P, P], F32, name="tabm", tag="tabm", bufs=1)
                    nc.vector.tensor_tensor(out=tmp[:], in0=pT[:],
                                            in1=Wt[srcRep2][:], op=OP.mult)
                    asr = sp.tile([P, 2], F32, name="asr", tag="asr")
                    nc.vector.tensor_reduce(asr[:].unsqueeze(2),
                                            tmp[:].rearrange("q (h f) -> q h f", f=D),
                                            axis=AX.X, op=OP.add)
                    nc.vector.tensor_copy(out=row3[:, :, D:W],
                                          in_=asr[:].unsqueeze(2))
                if ad_from_xc:
                    pTx = ps.tile([P, P], BF, name="tabTx", tag="miscP")
                    nc.tensor.transpose(out=pTx[:], in_=XCb[:, cc:cc + P],
                                        identity=identb[:])
                    dsrc = pTx
                else:
                    dsrc = pT
                tmp2 = sp.tile([P, P], F32, name="tabm2", tag="tabm2", bufs=1)
                nc.vector.tensor_tensor(out=tmp2[:], in0=dsrc[:],
                                        in1=Wt[dstRep2][:], op=OP.mult)
                nc.vector.tensor_reduce(ad_nm[:, 2 * tp:2 * tp + 2].unsqueeze(2),
                                        tmp2[:].rearrange("q (h f) -> q h f", f=D),
                                        axis=AX.X, op=OP.add)
            for w_ in range(NW):
                nc.sync.dma_start(
                    out=tbl_locs[li][sbr[w_]:sbr[w_ + 1], :].rearrange(
                        "(tp h p) w -> p tp h w", h=2, p=P),
                    in_=row_all[:, sbp[w_] * 2 * WG:sbp[w_ + 1] * 2 * WG])
            # alpha_dst transposed: adTs[j, q] = ad of node (tile j, row q)
            pAd = ps.tile([P, P], F32, name="adT", tag="miscP")
            nc.tensor.transpose(out=pAd[0:t_tiles, :], in_=ad_nm[:],
                                identity=ident[:])
            nc.vector.tensor_copy(out=adTs[0:t_tiles, :], in_=pAd[0:t_tiles, :])
            for w_ in range(NW):
                nc.gpsimd.collective_compute(
                    "AllGather", mybir.AluOpType.bypass,
                    replica_groups=[list(range(NC))],
                    ins=[tbl_locs[li][sbr[w_]:sbr[w_ + 1], :].opt()],
                    outs=[tbl_alls[li][w_].opt()])

        def edge_phase(is_gate, li):
            """Message round; writes agg (normalized, fp16) into HXb (fm)."""
            col0 = 0
            qrr = 0
            for tp in range(HT):
                calls = meta[tp]
                TC = sum(cl[1] for cl in calls)
                ar2 = sp.tile([1, 2 * P], BF, name="ar2", tag="ar2")
                nc.sync.dma_start(out=ar2[0:1, :], in_=adTs[2 * tp:2 * tp + 2, :])
                prp = ps.tile([P, 2 * P], F32, name="repP", tag="rowP")
                nc.tensor.matmul(prp[:], ones1[:], ar2[0:1, :],
                                 start=True, stop=True)
                rep_pair = sp.tile([P, 2 * P], BF, name="rep_pair", tag="rep_pair")
                nc.vector.tensor_copy(out=rep_pair[:], in_=prp[:])
                gt = ep.tile([P, MXC * WG], BF, name="gt", tag="gt", bufs=3)
                cc = 0
                for w_, ncols, acols, bcol0, amax, bmax in calls:
                    nc.gpsimd.dma_gather(
                        out_ap=gt[:, cc * WG:(cc + ncols) * WG].rearrange(
                            "p (c w) -> p c w", w=WG),
                        in_ap=tbl_alls[li][w_][:],
                        idxs_ap=idx_sb[:, (col0 + cc) * 8:(col0 + cc + ncols) * 8],
                        num_idxs=ncols * P, num_idxs_reg=ncols * P,
                        elem_size=WG, queue_num=qrr % 4)
                    qrr += 1
                    cc += ncols
                gt3 = gt[:].rearrange("p (c w) -> p c w", w=WG)
                s256 = ep.tile([P, MXC * 2 * P], BF, name="s256", tag="s256")
                nc.sync.dma_start(out=s256[:, 0:TC * 2 * P],
                                  in_=s256_h[:, col0 * 2 * P:(col0 + TC) * 2 * P])
                sat = ep.tile([P, MXC * 2 * P], BF, name="sat", tag="sat", bufs=1)
                nc.vector.tensor_tensor(
                    out=sat[:, :TC * 2 * P].rearrange("p (c q) -> p c q", q=2 * P),
                    in0=s256[:, :TC * 2 * P].rearrange("p (c q) -> p c q", q=2 * P),
                    in1=rep_pair[:].unsqueeze(1).to_broadcast([P, TC, 2 * P]),
                    op=OP.mult)
                s3 = sat[:].rearrange("p (c q) -> p c q", q=2 * P)
                wdt = P
                while wdt >= 1:
                    nc.vector.tensor_tensor(
                        out=s3[:, 0:TC, 0:wdt], in0=s3[:, 0:TC, 0:wdt],
                        in1=s3[:, 0:TC, wdt:2 * wdt], op=OP.add)
                    wdt //= 2
                aslot = s3[:, 0:TC, 0:1].squeeze(2)
                q = sp.tile([P, MXC], F32, name="q", tag="q")
                if is_gate:
                    be_sb = ep.tile([P, MXC * D], BF, name="be_sb", tag="be_sb", bufs=1)
                    nc.sync.dma_start(out=be_sb[:, 0:TC * D],
                                      in_=be_h[:, col0 * D:(col0 + TC) * D])
                    m_all = ep.tile([P, MXC * D], BF, name="m_all", tag="m_all")
                    m3 = m_all[:].rearrange("p (c w) -> p c w", w=D)
                    nc.vector.tensor_tensor(
                        out=m3[:, 0:TC, :], in0=gt3[:, 0:TC, 0:D],
                        in1=be_sb[:, 0:TC * D].rearrange("p (c w) -> p c w", w=D),
                        op=OP.add)
                    nc.scalar.activation(m_all[:, :TC * D], m_all[:, :TC * D],
                                         AF.Prelu, alpha=NEG)
                    lt = ep.tile([P, MXC * D], BF, name="lt", tag="lt", bufs=1)
                    nc.vector.tensor_tensor(
                        out=lt[:, :TC * D], in0=m3[:, 0:TC, :],
                        in1=Wt["attlRep"][:].unsqueeze(1).to_broadcast([P, TC, D]),
                        op=OP.mult)
                    nc.vector.tensor_reduce(q[:, 0:TC].unsqueeze(2),
                                            lt[:, :TC * D].rearrange(
                                                "p (c w) -> p c w", w=D),
                                            axis=AX.X, op=OP.add)
                    nc.vector.tensor_tensor(out=q[:, 0:TC], in0=q[:, 0:TC],
                                            in1=aslot, op=OP.add)
                    msg3 = m3
                else:
                    nc.vector.tensor_tensor(out=q[:, 0:TC],
                                            in0=gt3[:, 0:TC, D:D + 1].squeeze(2),
                                            in1=aslot, op=OP.add)
                    msg3 = gt3
                e_t = sp.tile([P, MXC], F32, name="e_t", tag="e_t")
                nc.scalar.activation(e_t[:, 0:TC], q[:, 0:TC], AF.Prelu, alpha=NEG)
                nc.scalar.activation(e_t[:, 0:TC], e_t[:, 0:TC], AF.Exp)
                rhs = ep.tile([P, MXC * W], BF, name="rhs", tag="rhs")
                r3 = rhs[:].rearrange("p (c w) -> p c w", w=W)
                nc.vector.tensor_tensor(
                    out=r3[:, 0:TC, 0:D], in0=msg3[:, 0:TC, 0:D],
                    in1=e_t[:, 0:TC].unsqueeze(2).to_broadcast([P, TC, D]),
                    op=OP.mult)
                nc.vector.tensor_copy(out=r3[:, 0:TC, D:W],
                                      in_=e_t[:, 0:TC].unsqueeze(2))
                amms = []
                bmms = []
                cc = 0
                for w_, ncols, acols, bcol0, amax, bmax in calls:
                    if amax:
                        amms += [cc + c for c in range(acols)]
                    if bmax:
                        bmms += [cc + c for c in range(bcol0, ncols)]
                    cc += ncols
                aggb = sp.tile([P, P], BF, name="aggb", tag="aggb")
                for h, mms, qofs in ((0, amms, 0), (1, bmms, P)):
                    pseg = ps_seg.tile([P, W], F32, name="pseg", tag="pseg")
                    for i, c in enumerate(mms):
                        nc.tensor.matmul(
                            pseg[:],
                            s256[:, c * 2 * P + qofs:c * 2 * P + qofs + P],
                            rhs[:, c * W:(c + 1) * W],
                            start=(i == 0), stop=(i == len(mms) - 1))
                    sn = sp.tile([P, 1], F32, name="sn", tag="sn")
                    nc.vector.tensor_single_scalar(out=sn[:], in_=pseg[:, D:W],
                                                   scalar=1e-16, op=OP.add)
                    rcp = sp.tile([P, 1], F32, name="rcp", tag="rcp")
                    nc.vector.reciprocal(rcp[:], sn[:])
                    nc.vector.tensor_tensor(out=aggb[:, h * D:(h + 1) * D],
                                            in0=pseg[:, 0:D],
                                            in1=rcp[:].to_broadcast([P, D]),
                                            op=OP.mult)
                pT2 = ps.tile([P, P], F32, name="aggT", tag="miscP")
                nc.tensor.matmul(pT2[:], aggb[:], identb[:], start=True, stop=True)
                nc.vector.tensor_copy(out=HXb[:, tp * P:(tp + 1) * P], in_=pT2[:])
                col0 += TC

        # ================= forward =================
        for ci in range(NCH):
            c0, c1 = ci * 512, min((ci + 1) * 512, HC)
            xin = sp.tile([P, 512], BF, name="xin", tag="xin", bufs=1)
            nc.sync.dma_start(out=xin[:, :c1 - c0], in_=xfm_h[:, c0:c1])
            pt = ps_big.tile([P, 512], F32, name="mmp0", tag="mmp")
            nc.tensor.matmul(pt[:, 0:c1 - c0], Wt["lin1_bd"][:],
                             xin[:, 0:c1 - c0], start=True, stop=True)
            nc.scalar.activation(XC[:, c0:c1], pt[:, 0:c1 - c0],
                                 AF.Prelu, bias=Wt["lin1_b"][:, 0:1], alpha=NEG)
        nc.vector.tensor_copy(out=XCb[:], in_=XC[:])
        # --- GATEConv ---
        mm_node(HXb, "gate_w1a_bd", XCb)
        build_table(HXb, None, "gateattrRep2", True, 0)
        edge_phase(True, 0)
        mm_node(HXb, "gate_w2_bd", HXb)
        elu_inplace(HXb, "gate_bias")
        gru_relu("gru0_")
        nc.vector.tensor_copy(out=XCb[:], in_=XC[:])
        # --- atom layers ---
        for l in range(4):
            pre = f"at{l}_"
            mm_node(HXb, pre + "wT", XCb)
            build_table(HXb, pre + "srcRep2", pre + "dstRep2", False, 1 + l)
            edge_phase(False, 1 + l)
            elu_inplace(HXb, pre + "bias")
            gru_relu(pre + "gru_")
            nc.vector.tensor_copy(out=XCb[:], in_=XC[:])

        # ================= readout =================
        mm_node(HXb, "mol_bd", XCb)          # xs into HXb
        asrc_nm = st.tile([P, t_tiles], F32, name="asrc_nm")
        for tp in range(HT):
            pT = ps.tile([P, P], BF, name="xsT", tag="miscP")
            nc.tensor.transpose(out=pT[:], in_=HXb[:, tp * P:(tp + 1) * P],
                                identity=identb[:])
            nc.vector.tensor_copy(out=row_all[:, tp * 2 * D:(tp + 1) * 2 * D],
                                  in_=pT[:])
            tmp = sp.tile([P, P], F32, name="xsm", tag="tabm", bufs=1)
            nc.vector.tensor_tensor(out=tmp[:], in0=pT[:],
                                    in1=Wt["molsrcRep2"][:], op=OP.mult)
            nc.vector.tensor_reduce(asrc_nm[:, 2 * tp:2 * tp + 2].unsqueeze(2),
                                    tmp[:].rearrange("q (h f) -> q h f", f=D),
                                    axis=AX.X, op=OP.add)
        for tp in range(HT):
            pT = ps.tile([P, P], BF, name="xcT", tag="miscP")
            nc.tensor.transpose(out=pT[:], in_=XCb[:, tp * P:(tp + 1) * P],
                                identity=identb[:])
            nc.vector.tensor_copy(out=XCb[:, tp * P:(tp + 1) * P], in_=pT[:])
        xc_nm = XCb
        xs_nm = row_all
        OUTT = st.tile([D, 2 * P], F32, name="OUTT")
        for k in range(2):
            pg = ps_seg.tile([P, D], F32, name="pg", tag="pseg")
            for tp in range(HT):
                sgp = sp.tile([P, 2 * 2 * P], BF, name="sgp", tag="sgp")
                for h in range(2):
                    nc.vector.tensor_scalar(
                        out=sgp[:, h * 2 * P:(h + 1) * 2 * P], in0=iota_sb[:],
                        scalar1=grel32[:, 2 * tp + h:2 * tp + h + 1],
                        scalar2=None, op0=OP.is_equal)
                for h in range(2):
                    j = 2 * tp + h
                    nc.tensor.matmul(
                        pg[:],
                        sgp[:, h * 2 * P + k * P:h * 2 * P + (k + 1) * P],
                        xc_nm[:, j * D:(j + 1) * D],
                        start=(j == 0), stop=(j == t_tiles - 1))
            og = sp.tile([P, D], F32, name="og", tag="og")
            nc.scalar.activation(og[:], pg[:], AF.Relu)
            pTo = ps.tile([D, P], F32, name="ogT", tag="miscP")
            nc.tensor.transpose(out=pTo[:], in_=og[:], identity=ident[:])
            nc.vector.tensor_copy(out=OUTT[:, k * P:(k + 1) * P], in_=pTo[:])
        HG = st.tile([D, 2 * P], F32, name="HG")
        for ts in range(3):
            pxd = ps_big.tile([D, 512], F32, name="xdp", tag="mmp")
            nc.tensor.matmul(pxd[:, 0:2 * P], Wt["mol_wT32"][:], OUTT[:],
                             start=True, stop=True)
            xds = sp.tile([D, 2 * P], F32, name="xds", tag="xds", bufs=1)
            nc.vector.tensor_copy(out=xds[:], in_=pxd[:, 0:2 * P])
            pag = ps.tile([1, 2 * P], F32, name="agp", tag="rowP")
            nc.tensor.matmul(pag[:], Wt["moldstCol"][:], xds[:],
                             start=True, stop=True)
            agr = sp.tile([1, 2 * P], BF, name="agr", tag="agr")
            nc.vector.tensor_copy(out=agr[:], in_=pag[:])
            prg = ps.tile([P, 2 * P], F32, name="repG", tag="rowP")
            nc.tensor.matmul(prg[:], ones1[:], agr[:], start=True, stop=True)
            rep_agr = sp.tile([P, 2 * P], BF, name="rep_agr", tag="rep_agr", bufs=1)
            nc.vector.tensor_copy(out=rep_agr[:], in_=prg[:])
            ag2 = sp.tile([P, 2], F32, name="ag2", tag="ag2")
            psg = [ps_seg.tile([P, W], F32, name=f"psg{k}", tag="pseg")
                   for k in range(2)]
            for tp in range(HT):
                sgp = sp.tile([P, 2 * 2 * P], BF, name="sgp2", tag="sgp")
                for h in range(2):
                    nc.vector.tensor_scalar(
                        out=sgp[:, h * 2 * P:(h + 1) * 2 * P], in0=iota_sb[:],
                        scalar1=grel32[:, 2 * tp + h:2 * tp + h + 1],
                        scalar2=None, op0=OP.is_equal)
                scr2 = sp.tile([P, 2 * 2 * P], BF, name="scr2", tag="scr")
                g3 = scr2[:].rearrange("p (h q) -> p h q", q=2 * P)
                nc.vector.tensor_tensor(
                    out=g3[:],
                    in0=sgp[:].rearrange("p (h q) -> p h q", q=2 * P),
                    in1=rep_agr[:].unsqueeze(1).to_broadcast([P, 2, 2 * P]),
                    op=OP.mult)
                wdt = P
                while wdt >= 1:
                    nc.vector.tensor_tensor(
                        out=g3[:, :, 0:wdt], in0=g3[:, :, 0:wdt],
                        in1=g3[:, :, wdt:2 * wdt], op=OP.add)
                    wdt //= 2
                nc.vector.tensor_copy(out=ag2[:], in_=g3[:, :, 0:1].squeeze(2))
                q2 = sp.tile([P, 2], F32, name="q2", tag="q2")
                nc.vector.tensor_tensor(out=q2[:], in0=asrc_nm[:, 2 * tp:2 * tp + 2],
                                        in1=ag2[:], op=OP.add)
                nc.scalar.activation(q2[:], q2[:], AF.Prelu, alpha=NEG)
                nc.scalar.activation(q2[:], q2[:], AF.Exp)
                rh = ep.tile([P, 2 * W], BF, name="rh", tag="rh", bufs=2)
                rh3 = rh[:].rearrange("p (h w) -> p h w", w=W)
                nc.vector.tensor_tensor(
                    out=rh3[:, :, 0:D],
                    in0=xs_nm[:, tp * 2 * D:(tp + 1) * 2 * D].rearrange(
                        "p (h f) -> p h f", f=D),
                    in1=q2[:].unsqueeze(2).to_broadcast([P, 2, D]),
                    op=OP.mult)
                nc.vector.tensor_copy(out=rh3[:, :, D:W], in_=q2[:].unsqueeze(2))
                for h in range(2):
                    j = 2 * tp + h
                    for k in range(2):
                        nc.tensor.matmul(
                            psg[k][:],
                            sgp[:, h * 2 * P + k * P:h * 2 * P + (k + 1) * P],
                            rh[:, h * W:(h + 1) * W],
                            start=(j == 0), stop=(j == t_tiles - 1))
            for k in range(2):
                sn = sp.tile([P, 1], F32, name="sng", tag="sn")
                nc.vector.tensor_single_scalar(out=sn[:], in_=psg[k][:, D:W],
                                               scalar=1e-16, op=OP.add)
                rcp = sp.tile([P, 1], F32, name="rcpg", tag="rcp")
                nc.vector.reciprocal(rcp[:], sn[:])
                aggg = sp.tile([P, D], F32, name="aggg", tag="aggg")
                nc.vector.tensor_tensor(out=aggg[:], in0=psg[k][:, 0:D],
                                        in1=rcp[:].to_broadcast([P, D]), op=OP.mult)
                nc.vector.tensor_tensor(out=aggg[:], in0=aggg[:],
                                        in1=Wt["mol_biasRep"][:], op=OP.add)
                r = sp.tile([P, D], F32, name="rg", tag="rg")
                nc.scalar.activation(r[:], aggg[:], AF.Relu)
                xm = sp.tile([P, D], F32, name="xmg", tag="xmg")
                nc.vector.tensor_sub(xm[:], aggg[:], r[:])
                nc.scalar.activation(xm[:], xm[:], AF.Exp)
                nc.vector.scalar_tensor_tensor(out=aggg[:], in0=xm[:], scalar=-1.0,
                                               in1=r[:], op0=OP.add, op1=OP.add)
                pTh = ps.tile([D, P], F32, name="hgT", tag="miscP")
                nc.tensor.transpose(out=pTh[:], in_=aggg[:], identity=ident[:])
                nc.vector.tensor_copy(out=HG[:, k * P:(k + 1) * P], in_=pTh[:])
            wih = Wt["mol_gru_wih"]
            whh = Wt["mol_gru_whh"]
            bs = Wt["mol_gru_bsum"]
            def gate2g(g):
                pt = ps_big.tile([D, 512], F32, name="ggp", tag="mmp")
                nc.tensor.matmul(pt[:, 0:2 * P], wih[:, g * D:(g + 1) * D], HG[:],
                                 start=True, stop=False)
                nc.tensor.matmul(pt[:, 0:2 * P], whh[:, g * D:(g + 1) * D], OUTT[:],
                                 start=False, stop=True)
                return pt
            prg2 = gate2g(0)
            rg2 = sp.tile([D, 2 * P], F32, name="ggr", tag="ggr", bufs=1)
            nc.scalar.activation(rg2[:], prg2[:, 0:2 * P], AF.Sigmoid, bias=bs[:, 0:1])
            pzg = gate2g(1)
            zg = sp.tile([D, 2 * P], F32, name="ggz", tag="ggz", bufs=1)
            nc.scalar.activation(zg[:], pzg[:, 0:2 * P], AF.Sigmoid, bias=bs[:, 1:2])
            pig = ps_big.tile([D, 512], F32, name="ggpi", tag="mmp")
            nc.tensor.matmul(pig[:, 0:2 * P], wih[:, 2 * D:3 * D], HG[:],
                             start=True, stop=True)
            phg = ps_big.tile([D, 512], F32, name="ggph", tag="mmp")
            nc.tensor.matmul(phg[:, 0:2 * P], whh[:, 2 * D:3 * D], OUTT[:],
                             start=True, stop=True)
            hng = sp.tile([D, 2 * P], F32, name="gghn", tag="gghn", bufs=1)
            nc.vector.tensor_scalar(out=hng[:], in0=phg[:, 0:2 * P],
                                    scalar1=Wt["mol_gru_bhh"][:, 2:3],
                                    scalar2=None, op0=OP.add)
            nc.vector.tensor_mul(hng[:], hng[:], rg2[:])
            nc.vector.tensor_tensor(out=hng[:], in0=hng[:], in1=pig[:, 0:2 * P],
                                    op=OP.add)
            ng = sp.tile([D, 2 * P], F32, name="ggn", tag="ggn", bufs=1)
            nc.scalar.activation(ng[:], hng[:], AF.Tanh,
                                 bias=Wt["mol_gru_bih"][:, 2:3])
            dg = sp.tile([D, 2 * P], F32, name="ggd", tag="ggd", bufs=1)
            nc.vector.tensor_sub(dg[:], OUTT[:], ng[:])
            nc.vector.tensor_mul(dg[:], dg[:], zg[:])
            nc.vector.tensor_tensor(out=dg[:], in0=dg[:], in1=ng[:], op=OP.add)
            nc.scalar.activation(OUTT[:], dg[:], AF.Relu)
        py = ps.tile([1, 2 * P], F32, name="py", tag="rowP")
        nc.tensor.matmul(py[:], Wt["lin2_wT"][:], OUTT[:], start=True, stop=True)
        ysb = sp.tile([1, 2 * P], F32, name="ysb", tag="ysb")
        nc.vector.tensor_scalar(out=ysb[:], in0=py[:], scalar1=Wt["lin2_b"][0:1, 0:1],
                                scalar2=None, op0=OP.add)
        nc.sync.dma_start(out=y_out[:], in_=ysb[0:1, 0:G_LOC])
    nc.compile()
    return nc


_CACHE = {}


def kernel(**inputs):
    from concourse.bass_utils import run_bass_kernel_spmd
    x = np.asarray(inputs["x"], np.float32)
    ei = np.asarray(inputs["edge_index"])
    ea = np.asarray(inputs["edge_attr"], np.float32)
    bt = np.asarray(inputs["batch"])
    per, n_pad, t_tiles, meta, TOTC = _prep(x, ei, ea, bt)
    kwf = {k: np.asarray(v, np.float32) for k, v in inputs.items()
           if k not in ("x", "edge_index", "edge_attr", "batch")}
    weights = _mk_weights(kwf)
    key = (n_pad, TOTC, tuple(tuple(tuple(cl) for cl in calls) for calls in meta[0]),
           tuple(meta[1]))
    if key not in _CACHE:
        _CACHE[key] = _build(n_pad, t_tiles, meta, TOTC,
                             {k: (v.shape, v.dtype == F16)
                              for k, v in weights.items()})
    nc = _CACHE[key]
    iota = np.tile(np.arange(2 * P).astype(np.float32)[None, :], (P, 1)).astype(F16)
    w1b = kwf["gate_lin1_w"][:, D:]
    in_maps = []
    for c in range(NC):
        b_e = (per[c]["attr_s"] @ w1b.T).astype(F16)
        b_e = np.ascontiguousarray(
            b_e.reshape(TOTC, P, D).transpose(1, 0, 2).reshape(P, TOTC * D))
        m = dict(xfm=per[c]["xfm"], idx16=per[c]["idx16"], s256=per[c]["s256"],
                 b_e=b_e, grel=per[c]["grel"], iota256=iota)
        for k, v in weights.items():
            m["w_" + k] = v
        in_maps.append(m)
    res = run_bass_kernel_spmd(nc, in_maps, core_ids=list(range(NC)))
    return np.concatenate([res.results[c]["y"][0] for c in range(NC)]).astype(np.float32)


# revision 23
# speedup vs baseline: 1.9136x; 1.0232x over previous
"""AttentiveFP forward on 8 Trainium2 NeuronCores (Bass/Tile).

Sharding: 2048 graphs (nodes contiguous, batch sorted) split into 8 blocks of
256 graphs; each core owns the edges whose dst node falls in its block. Per
round each core computes its nodes' features, all-gathers a compact per-node
table [xt | alpha_src] (fp16, 65 wide), expands it locally to 256B-aligned
rows, then fetches per-edge src rows with nc.gpsimd.dma_gather (the token
gather ucode: thousands of int16 indices per call, round-robined over 4 SWDGE
queues). Indices are int16, so slots are grouped per (node-tile-pair,
32768-row source window); within a call, tile-a slots carry rel in [0,128)
and tile-b slots rel in [128,256), so one 256-wide is_equal one-hot serves
both tiles' PSUM segment-matmuls and the alpha_dst select (one-hot x
replicated alpha row, reduced on DVE). Per-edge alpha_dst needs no gather
(dst is always local). Node phases run feature-major, half-packed, with
block-diagonal [128,128] fp16 weights; GRU hidden state stays fp32. The gate
round's edge-attr term (W1b @ e_attr) is precomputed on the host per slot.
Readout uses a 256-wide graph one-hot per tile-pair plus a replicated
per-graph alpha row (no gathers).

Softmax max-subtraction is skipped (logits O(1), shift-invariant).
"""
import sys
sys.path.insert(0, '/opt/trn_rl_repo')
sys.path.insert(0, '/root/.axon_site')

import numpy as np

F16 = np.float16
NC = 8
D = 64
G_TOT = 2048
G_LOC = G_TOT // NC
F_IN = 25
E_DIM = 4
NEG = 0.01
P = 128
W = 65            # compact table row: [xt(64) | alpha_src]
WG = 128          # gathered row width (256B-aligned)
WIN = 32768       # int16 index window (rows)


def _prep(x, edge_index, edge_attr, batch):
    src = edge_index[0].astype(np.int64)
    dst = edge_index[1].astype(np.int64)
    batch = batch.astype(np.int64)

    gstart = np.searchsorted(batch, np.arange(0, G_TOT + 1, G_LOC))
    n0 = gstart[:-1]
    nloc = np.diff(gstart)
    n_pad = int(np.ceil((nloc.max() + 1) / 256) * 256)
    t_tiles = n_pad // P
    HC = n_pad // 2
    HT = t_tiles // 2
    NW = 4
    bp = HT // NW
    pr_cnt = [bp + (1 if i < HT % NW else 0) for i in range(NW)]
    sbp = np.cumsum([0] + pr_cnt)                 # pair boundaries per slice
    sbr = sbp * 2 * P                             # row boundaries per slice
    srows = np.diff(sbr)

    def pi_row(n):
        h = n // HC
        r = n % HC
        return (2 * (r // P) + h) * P + (r % P)

    src_dev = np.searchsorted(gstart[1:], src, side='right')
    dst_dev = np.searchsorted(gstart[1:], dst, side='right')
    pr_all = pi_row(src - n0[src_dev])
    w_all = np.searchsorted(sbr[1:], pr_all, side='right')
    blk0 = np.concatenate([[0], np.cumsum(srows * NC)])
    gidx_all = (blk0[w_all] + src_dev * srows[w_all]
                + (pr_all - sbr[w_all]))

    # ---- pass 1: bucket edges per core into (pair, window, half) ----
    buckets = [[[[None, None] for _ in range(NW)] for _ in range(HT)]
               for _ in range(NC)]
    for c in range(NC):
        sel = np.where(dst_dev == c)[0]
        dl = dst[sel] - n0[c]
        j_dst = 2 * ((dl % HC) // P) + dl // HC
        p_dst = dl % P
        gi = gidx_all[sel]
        w_of = np.searchsorted(blk0[1:], gi, side='right')
        for tp in range(HT):
            for h in range(2):
                m = j_dst == 2 * tp + h
                gi_m, p_m, w_m, sel_m = gi[m], p_dst[m], w_of[m], sel[m]
                for w in range(NW):
                    mm = w_m == w
                    buckets[c][tp][w][h] = (gi_m[mm] - blk0[w], p_m[mm],
                                            sel_m[mm])
    # ---- pass 2: SPMD-uniform call metadata (max counts over cores) ----
    meta = []       # per pair: [w, ncols, acols, bcol0, amax, bmax]
    for tp in range(HT):
        calls = []
        for w in range(NW):
            amax = max(len(buckets[c][tp][w][0][0]) for c in range(NC))
            bmax = max(len(buckets[c][tp][w][1][0]) for c in range(NC))
            if amax + bmax == 0:
                continue
            ncols = (amax + bmax + P - 1) // P
            calls.append([w, ncols, (amax + P - 1) // P, amax // P, amax, bmax])
        if not any(cl[4] for cl in calls):
            calls.insert(0, [0, 1, 1, 0, P, 0])
        if not any(cl[5] for cl in calls):
            calls.append([0, 1, 0, 0, 0, P])
        meta.append(calls)
    TOTC = sum(cl[1] for calls in meta for cl in calls)

    per = []
    for c in range(NC):
        idx16 = np.zeros((16, TOTC * 8), np.int16)
        rel = np.full((P, TOTC), 300.0, np.float32)
        attr_s = np.zeros((TOTC * P, E_DIM), np.float32)
        col0 = 0
        for tp in range(HT):
            for w_, ncols, acols, bcol0, amax, bmax in meta[tp]:
                flat_idx = np.zeros(ncols * P, np.int16)
                flat_rel = np.full(ncols * P, 300.0, np.float32)
                flat_attr = np.zeros((ncols * P, E_DIM), np.float32)
                pos = 0
                for h, hmax in ((0, amax), (1, bmax)):
                    gi_l, p_l, sel_l = buckets[c][tp][w_][h]
                    k = len(gi_l)
                    flat_idx[pos:pos + k] = gi_l.astype(np.int16)
                    flat_rel[pos:pos + k] = p_l + h * P
                    flat_attr[pos:pos + k] = edge_attr[sel_l]
                    pos += hmax
                idx16[:, col0 * 8:(col0 + ncols) * 8] = \
                    flat_idx.reshape(ncols * 8, 16).T
                rel[:, col0:col0 + ncols] = flat_rel.reshape(ncols, P).T
                attr_s[col0 * P:(col0 + ncols) * P] = flat_attr
                col0 += ncols
        s256 = (rel.astype(np.int32)[:, :, None] ==
                np.arange(2 * P, dtype=np.int32)[None, None, :]).astype(F16)
        per.append(dict(idx16=np.tile(idx16, (8, 1)),
                        s256=np.ascontiguousarray(s256.reshape(P, TOTC * 2 * P)),
                        attr_s=attr_s))
        nl = int(nloc[c])
        gl = batch[n0[c]:n0[c] + nl] - G_LOC * c
        grel = np.full((P, t_tiles), 300.0, np.float32)
        n_ids = np.arange(n_pad)
        h_a = n_ids // HC
        j_a = 2 * ((n_ids % HC) // P) + h_a
        p_a = n_ids % P
        valid = n_ids < nl
        grel[p_a[valid], j_a[valid]] = gl[n_ids[valid]]
        per[c]['grel'] = np.ascontiguousarray(grel.astype(F16))
        xp = np.zeros((n_pad, F_IN), np.float32)
        xp[:nl] = x[n0[c]:n0[c] + nl]
        xfm = np.zeros((P, HC), np.float32)
        xfm[:F_IN] = xp[:HC].T
        xfm[D:D + F_IN] = xp[HC:].T
        per[c]['xfm'] = xfm.astype(F16)
    return per, n_pad, t_tiles, (meta, [int(v) for v in sbp]), TOTC


def _mk_weights(kw):
    w = {}
    def bd(a):
        t = a.T
        z = np.zeros((P, P), np.float32)
        z[0:D, 0:D] = t
        z[D:2 * D, D:2 * D] = t
        return z
    def col(a):
        return np.concatenate([a, a])[:, None]
    def rep2(a):
        return np.tile(np.concatenate([a, a])[None, :], (P, 1))
    def rep1(a):
        return np.tile(a[None, :], (P, 1))
    def gb(a):
        t = a.reshape(3, D).T
        return np.concatenate([t, t], 0)
    def gru_bd(wg):
        out = np.zeros((P, 3 * P), np.float32)
        for g in range(3):
            out[:, g * P:(g + 1) * P] = bd(wg[g * D:(g + 1) * D])
        return out

    B, F = 'b', 'f'
    lin1 = np.zeros((P, P), np.float32)
    lin1[0:F_IN, 0:D] = kw["lin1_w"].T
    lin1[D:D + F_IN, D:2 * D] = kw["lin1_w"].T
    w["lin1_bd"] = (lin1, B)
    w["lin1_b"] = (col(kw["lin1_b"]), F)
    w["gate_w1a_bd"] = (bd(kw["gate_lin1_w"][:, :D]), B)
    w["attlRep"] = (rep1(kw["gate_att_l"]), B)
    w["gateattrRep2"] = (rep2(kw["gate_att_r"]), B)
    w["gate_w2_bd"] = (bd(kw["gate_lin2_w"]), B)
    w["gate_bias"] = (col(kw["gate_bias"]), F)
    w["gru0_wih"] = (gru_bd(kw["gru0_wih"]), B)
    w["gru0_whh"] = (gru_bd(kw["gru0_whh"]), B)
    w["gru0_bih"] = (gb(kw["gru0_bih"]), F)
    w["gru0_bhh"] = (gb(kw["gru0_bhh"]), F)
    w["gru0_bsum"] = (gb(kw["gru0_bih"] + kw["gru0_bhh"]), F)
    for l in range(4):
        pre = f"at{l}_"
        w[pre + "wT"] = (bd(kw["atom_lin_w"][l]), B)
        w[pre + "srcRep2"] = (rep2(kw["atom_att_src"][l]), B)
        w[pre + "dstRep2"] = (rep2(kw["atom_att_dst"][l]), B)
        w[pre + "bias"] = (col(kw["atom_bias"][l]), F)
        w[pre + "gru_wih"] = (gru_bd(kw["atom_gru_wih"][l]), B)
        w[pre + "gru_whh"] = (gru_bd(kw["atom_gru_whh"][l]), B)
        w[pre + "gru_bih"] = (gb(kw["atom_gru_bih"][l]), F)
        w[pre + "gru_bhh"] = (gb(kw["atom_gru_bhh"][l]), F)
        w[pre + "gru_bsum"] = (gb(kw["atom_gru_bih"][l] + kw["atom_gru_bhh"][l]), F)
    w["mol_bd"] = (bd(kw["mol_lin_w"]), B)
    w["mol_wT32"] = (kw["mol_lin_w"].T.copy(), F)
    w["molsrcRep2"] = (rep2(kw["mol_att_src"]), B)
    w["moldstCol"] = (kw["mol_att_dst"][:, None].copy(), F)
    w["mol_biasRep"] = (rep1(kw["mol_bias"]), F)
    w["mol_gru_wih"] = (kw["mol_gru_wih"].T.copy(), F)
    w["mol_gru_whh"] = (kw["mol_gru_whh"].T.copy(), F)
    w["mol_gru_bih"] = (gb(kw["mol_gru_bih"])[:D], F)
    w["mol_gru_bhh"] = (gb(kw["mol_gru_bhh"])[:D], F)
    w["mol_gru_bsum"] = (gb(kw["mol_gru_bih"] + kw["mol_gru_bhh"])[:D], F)
    w["lin2_wT"] = (kw["lin2_w"].T.copy(), F)
    w["lin2_b"] = (kw["lin2_b"][:, None].copy(), F)
    out = {}
    for k, (v, tag) in w.items():
        v = np.ascontiguousarray(v, np.float32)
        out[k] = v.astype(F16) if tag == B else v
    return out


def _build(n_pad, t_tiles, meta_in, TOTC, wmeta):
    meta, sbp = meta_in
    import concourse.bacc as bacc
    import concourse.mybir as mybir
    import concourse.tile as tile
    from concourse.masks import make_identity

    dt = mybir.dt
    AF = mybir.ActivationFunctionType
    OP = mybir.AluOpType
    AX = mybir.AxisListType
    BF = dt.float16
    F32 = dt.float32

    HC = n_pad // 2
    HT = t_tiles // 2
    NCH = (HC + 511) // 512
    MXC = max(sum(cl[1] for cl in calls) for calls in meta)
    NW = 4
    sbr = [b * 2 * P for b in sbp]                 # per-core slice row bounds
    srows = [sbr[i + 1] - sbr[i] for i in range(NW)]
    blk0 = [0]
    for i in range(NW):
        blk0.append(blk0[-1] + srows[i] * NC)

    nc = bacc.Bacc("TRN2", target_bir_lowering=False, debug=False, num_devices=NC,
                   num_swdge_queues=4)

    xfm_h = nc.dram_tensor("xfm", [P, HC], BF, kind="ExternalInput")
    idx_h = nc.dram_tensor("idx16", [P, TOTC * 8], dt.int16, kind="ExternalInput")
    s256_h = nc.dram_tensor("s256", [P, TOTC * 2 * P], BF, kind="ExternalInput")
    be_h = nc.dram_tensor("b_e", [P, TOTC * D], BF, kind="ExternalInput")
    grel_h = nc.dram_tensor("grel", [P, t_tiles], BF, kind="ExternalInput")
    iota_h = nc.dram_tensor("iota256", [P, 2 * P], BF, kind="ExternalInput")
    cst_h = {k: nc.dram_tensor("w_" + k, list(s_), BF if isbf else F32,
                               kind="ExternalInput")
             for k, (s_, isbf) in wmeta.items()}
    y_out = nc.dram_tensor("y", [1, G_LOC], F32, kind="ExternalOutput")

    with tile.TileContext(nc) as tc:
      with (
        tc.tile_pool(name="cst", bufs=1) as cst,
        tc.tile_pool(name="st", bufs=1) as st,
        tc.tile_pool(name="ep", bufs=2) as ep,
        tc.tile_pool(name="sp", bufs=2) as sp,
        tc.tile_pool(name="ps", bufs=2, space="PSUM") as ps,
        tc.tile_pool(name="ps_seg", bufs=2, space="PSUM") as ps_seg,
        tc.tile_pool(name="ps_big", bufs=2, space="PSUM") as ps_big,
        tc.tile_pool(name="dram", bufs=1, space="DRAM") as dp,
      ):
        def load(name):
            h = cst_h[name]
            t = cst.tile(list(h.shape), h.dtype, name="c_" + name)
            nc.sync.dma_start(out=t[:], in_=h[:])
            return t
        Wt = {k: load(k) for k in cst_h}
        idx_sb = cst.tile([P, TOTC * 8], dt.int16, name="idx_sb")
        nc.sync.dma_start(out=idx_sb[:], in_=idx_h[:])
        grel_sb = cst.tile([P, t_tiles], BF, name="grel_sb")
        nc.sync.dma_start(out=grel_sb[:], in_=grel_h[:])
        grel32 = cst.tile([P, t_tiles], F32, name="grel32")
        nc.vector.tensor_copy(out=grel32[:], in_=grel_sb[:])
        iota_sb = cst.tile([P, 2 * P], BF, name="iota_sb")
        nc.sync.dma_start(out=iota_sb[:], in_=iota_h[:])
        identb = cst.tile([P, P], BF, name="identb")
        make_identity(nc, identb[:])
        ident = cst.tile([P, P], F32, name="ident")
        make_identity(nc, ident[:])
        ones1 = cst.tile([1, P], BF, name="ones1")
        nc.vector.memset(ones1[:], 1.0)
        onesf = cst.tile([P, P], BF, name="onesf")
        nc.vector.memset(onesf[:], 1.0)

        XC = st.tile([P, HC], F32, name="XC")
        XCb = st.tile([P, HC], BF, name="XCb")
        HXb = st.tile([P, HC], BF, name="HXb")
        ad_nm = st.tile([P, t_tiles], F32, name="ad_nm")
        adTs = st.tile([P, P], BF, name="adTs")
        row_all = st.tile([P, HT * 2 * WG], BF, name="row_all")
        tbl_locs = [dp.tile([n_pad, WG], BF, name=f"tbl_loc{i}") for i in range(5)]
        tbl_alls = [[dp.tile([NC * srows[w], WG], BF, addr_space="Shared",
                             name=f"tbl_all{i}_{w}") for w in range(NW)]
                    for i in range(5)]

        def mm_node(dst, wkey, srcb, act=AF.Copy, bias=None, alpha=0.0):
            for ci in range(NCH):
                c0, c1 = ci * 512, min((ci + 1) * 512, HC)
                pt = ps_big.tile([P, 512], F32, name="mmp", tag="mmp")
                nc.tensor.matmul(pt[:, 0:c1 - c0], Wt[wkey][:], srcb[:, c0:c1],
                                 start=True, stop=True)
                b = Wt[bias][:, 0:1] if bias else 0.0
                nc.scalar.activation(dst[:, c0:c1], pt[:, 0:c1 - c0],
                                     act, bias=b, alpha=alpha)

        def elu_inplace(t_fm, bias):
            for ci in range(NCH):
                c0, c1 = ci * 512, min((ci + 1) * 512, HC)
                w_ = c1 - c0
                v = t_fm[:, c0:c1]
                tin = sp.tile([P, 512], F32, name="eluin", tag="eluin", bufs=1)
                nc.scalar.activation(tin[:, 0:w_], v, AF.Prelu,
                                     bias=Wt[bias][:, 0:1], alpha=1.0)
                r = sp.tile([P, 512], F32, name="elur", tag="elur", bufs=1)
                nc.scalar.activation(r[:, 0:w_], tin[:, 0:w_], AF.Relu)
                nc.vector.tensor_sub(tin[:, 0:w_], tin[:, 0:w_], r[:, 0:w_])
                nc.scalar.activation(tin[:, 0:w_], tin[:, 0:w_], AF.Exp)
                nc.vector.scalar_tensor_tensor(
                    out=v, in0=tin[:, 0:w_], scalar=-1.0,
                    in1=r[:, 0:w_], op0=OP.add, op1=OP.add)

        def gru_relu(pre):
            wih = Wt[pre + "wih"]
            whh = Wt[pre + "whh"]
            bs = Wt[pre + "bsum"]
            for ci in range(NCH):
                c0, c1 = ci * 512, min((ci + 1) * 512, HC)
                w_ = c1 - c0
                def gate2(g):
                    pt = ps_big.tile([P, 512], F32, name="grup", tag="mmp")
                    nc.tensor.matmul(pt[:, 0:w_], wih[:, g * P:(g + 1) * P],
                                     HXb[:, c0:c1], start=True, stop=False)
                    nc.tensor.matmul(pt[:, 0:w_], whh[:, g * P:(g + 1) * P],
                                     XCb[:, c0:c1], start=False, stop=True)
                    return pt
                pr = gate2(0)
                r = sp.tile([P, 512], F32, name="grur", tag="grur", bufs=1)
                nc.scalar.activation(r[:, 0:w_], pr[:, 0:w_], AF.Sigmoid,
                                     bias=bs[:, 0:1])
                pz = gate2(1)
                z = sp.tile([P, 512], F32, name="gruz", tag="gruz", bufs=1)
                nc.scalar.activation(z[:, 0:w_], pz[:, 0:w_], AF.Sigmoid,
                                     bias=bs[:, 1:2])
                pin = ps_big.tile([P, 512], F32, name="grupi", tag="mmp")
                nc.tensor.matmul(pin[:, 0:w_], wih[:, 2 * P:3 * P],
                                 HXb[:, c0:c1], start=True, stop=True)
                phn = ps_big.tile([P, 512], F32, name="gruph", tag="mmp")
                nc.tensor.matmul(phn[:, 0:w_], whh[:, 2 * P:3 * P],
                                 XCb[:, c0:c1], start=True, stop=True)
                hn = sp.tile([P, 512], F32, name="gruhn", tag="gruhn", bufs=1)
                nc.scalar.activation(hn[:, 0:w_], phn[:, 0:w_], AF.Prelu,
                                     bias=Wt[pre + "bhh"][:, 2:3], alpha=1.0)
                nc.vector.tensor_mul(hn[:, 0:w_], hn[:, 0:w_], r[:, 0:w_])
                nc.vector.tensor_tensor(out=hn[:, 0:w_], in0=hn[:, 0:w_],
                                        in1=pin[:, 0:w_], op=OP.add)
                n_t = sp.tile([P, 512], F32, name="grun", tag="grun", bufs=1)
                nc.scalar.activation(n_t[:, 0:w_], hn[:, 0:w_], AF.Tanh,
                                     bias=Wt[pre + "bih"][:, 2:3])
                d_t = sp.tile([P, 512], F32, name="grud", tag="grud", bufs=1)
                nc.vector.tensor_sub(d_t[:, 0:w_], XC[:, c0:c1], n_t[:, 0:w_])
                nc.vector.tensor_mul(d_t[:, 0:w_], d_t[:, 0:w_], z[:, 0:w_])
                nc.vector.tensor_tensor(out=d_t[:, 0:w_], in0=d_t[:, 0:w_],
                                        in1=n_t[:, 0:w_], op=OP.add)
                nc.scalar.activation(XC[:, c0:c1], d_t[:, 0:w_], AF.Relu)

        def build_table(srcb, srcRep2, dstRep2, ad_from_xc, li):
            for tp in range(HT):
                cc = tp * P
                pT = ps.tile([P, P], BF, name="tabT", tag="miscP")
                nc.tensor.transpose(out=pT[:], in_=srcb[:, cc:cc + P],
                                    identity=identb[:])
                row3 = row_all[:, tp * 2 * WG:(tp + 1) * 2 * WG].rearrange(
                    "p (h w) -> p h w", w=WG)
                nc.vector.tensor_copy(
                    out=row3[:, :, 0:D],
                    in_=pT[:].rearrange("q (h f) -> q h f", f=D))
                if srcRep2 is None:
                    nc.vector.memset(row3[:, :, D:W], 0.0)
                else:
                    tmp = sp.tile([P, P], F32, name="tabm", tag="tabm", bufs=1)
                    nc.vector.tensor_tensor(out=tmp[:], in0=pT[:],
                                            in1=Wt[srcRep2][:], op=OP.mult)
                    asr = sp.tile([P, 2], F32, name="asr", tag="asr")
                    nc.vector.tensor_reduce(asr[:].unsqueeze(2),
                                            tmp[:].rearrange("q (h f) -> q h f", f=D),
                                            axis=AX.X, op=OP.add)
                    nc.vector.tensor_copy(out=row3[:, :, D:W],
                                          in_=asr[:].unsqueeze(2))
                if ad_from_xc:
                    pTx = ps.tile([P, P], BF, name="tabTx", tag="miscP")
                    nc.tensor.transpose(out=pTx[:], in_=XCb[:, cc:cc + P],
                                        identity=identb[:])
                    dsrc = pTx
                else:
                    dsrc = pT
                tmp2 = sp.tile([P, P], F32, name="tabm2", tag="tabm2", bufs=1)
                nc.vector.tensor_tensor(out=tmp2[:], in0=dsrc[:],
                                        in1=Wt[dstRep2][:], op=OP.mult)
                nc.vector.tensor_reduce(ad_nm[:, 2 * tp:2 * tp + 2].unsqueeze(2),
                                        tmp2[:].rearrange("q (h f) -> q h f", f=D),
                                        axis=AX.X, op=OP.add)
            for w_ in range(NW):
                nc.sync.dma_start(
                    out=tbl_locs[li][sbr[w_]:sbr[w_ + 1], :].rearrange(
                        "(tp h p) w -> p tp h w", h=2, p=P),
                    in_=row_all[:, sbp[w_] * 2 * WG:sbp[w_ + 1] * 2 * WG])
            # alpha_dst transposed: adTs[j, q] = ad of node (tile j, row q)
            pAd = ps.tile([P, P], F32, name="adT", tag="miscP")
            nc.tensor.transpose(out=pAd[0:t_tiles, :], in_=ad_nm[:],
                                identity=ident[:])
            nc.vector.tensor_copy(out=adTs[0:t_tiles, :], in_=pAd[0:t_tiles, :])
            for w_ in range(NW):
                nc.gpsimd.collective_compute(
                    "AllGather", mybir.AluOpType.bypass,
                    replica_groups=[list(range(NC))],
                    ins=[tbl_locs[li][sbr[w_]:sbr[w_ + 1], :].opt()],
                    outs=[tbl_alls[li][w_].opt()])

        def edge_phase(is_gate, li):
            """Message round; writes agg (normalized, fp16) into HXb (fm)."""
            col0 = 0
            qrr = 0
            for tp in range(HT):
                calls = meta[tp]
                TC = sum(cl[1] for cl in calls)
                ar2 = sp.tile([1, 2 * P], BF, name="ar2", tag="ar2")
                nc.sync.dma_start(out=ar2[0:1, :], in_=adTs[2 * tp:2 * tp + 2, :])
                prp = ps.tile([P, 2 * P], F32, name="repP", tag="rowP")
                nc.tensor.matmul(prp[:], ones1[:], ar2[0:1, :],
                                 start=True, stop=True)
                rep_pair = sp.tile([P, 2 * P], BF, name="rep_pair", tag="rep_pair")
                nc.vector.tensor_copy(out=rep_pair[:], in_=prp[:])
                gt = ep.tile([P, MXC * WG], BF, name="gt", tag="gt", bufs=3)
                cc = 0
                for w_, ncols, acols, bcol0, amax, bmax in calls:
                    nc.gpsimd.dma_gather(
                        out_ap=gt[:, cc * WG:(cc + ncols) * WG].rearrange(
                            "p (c w) -> p c w", w=WG),
                        in_ap=tbl_alls[li][w_][:],
                        idxs_ap=idx_sb[:, (col0 + cc) * 8:(col0 + cc + ncols) * 8],
                        num_idxs=ncols * P, num_idxs_reg=ncols * P,
                        elem_size=WG, queue_num=qrr % 4)
                    qrr += 1
                    cc += ncols
                gt3 = gt[:].rearrange("p (c w) -> p c w", w=WG)
                s256 = ep.tile([P, MXC * 2 * P], BF, name="s256", tag="s256")
                nc.sync.dma_start(out=s256[:, 0:TC * 2 * P],
                                  in_=s256_h[:, col0 * 2 * P:(col0 + TC) * 2 * P])
                sat = ep.tile([P, MXC * 2 * P], BF, name="sat", tag="sat", bufs=1)
                nc.vector.tensor_tensor(
                    out=sat[:, :TC * 2 * P].rearrange("p (c q) -> p c q", q=2 * P),
                    in0=s256[:, :TC * 2 * P].rearrange("p (c q) -> p c q", q=2 * P),
                    in1=rep_pair[:].unsqueeze(1).to_broadcast([P, TC, 2 * P]),
                    op=OP.mult)
                s3 = sat[:].rearrange("p (c q) -> p c q", q=2 * P)
                wdt = P
                while wdt >= 16:
                    nc.vector.tensor_tensor(
                        out=s3[:, 0:TC, 0:wdt], in0=s3[:, 0:TC, 0:wdt],
                        in1=s3[:, 0:TC, wdt:2 * wdt], op=OP.add)
                    wdt //= 2
                asl = sp.tile([P, MXC], F32, name="asl", tag="asl")
                nc.vector.tensor_reduce(asl[:, 0:TC].unsqueeze(2),
                                        s3[:, 0:TC, 0:16], axis=AX.X, op=OP.add)
                aslot = asl[:, 0:TC]
                q = sp.tile([P, MXC], F32, name="q", tag="q")
                if is_gate:
                    be_sb = ep.tile([P, MXC * D], BF, name="be_sb", tag="be_sb", bufs=1)
                    nc.sync.dma_start(out=be_sb[:, 0:TC * D],
                                      in_=be_h[:, col0 * D:(col0 + TC) * D])
                    m_all = ep.tile([P, MXC * D], BF, name="m_all", tag="m_all")
                    m3 = m_all[:].rearrange("p (c w) -> p c w", w=D)
                    nc.vector.tensor_tensor(
                        out=m3[:, 0:TC, :], in0=gt3[:, 0:TC, 0:D],
                        in1=be_sb[:, 0:TC * D].rearrange("p (c w) -> p c w", w=D),
                        op=OP.add)
                    nc.scalar.activation(m_all[:, :TC * D], m_all[:, :TC * D],
                                         AF.Prelu, alpha=NEG)
                    lt = ep.tile([P, MXC * D], BF, name="lt", tag="lt", bufs=1)
                    nc.vector.tensor_tensor(
                        out=lt[:, :TC * D], in0=m3[:, 0:TC, :],
                        in1=Wt["attlRep"][:].unsqueeze(1).to_broadcast([P, TC, D]),
                        op=OP.mult)
                    nc.vector.tensor_reduce(q[:, 0:TC].unsqueeze(2),
                                            lt[:, :TC * D].rearrange(
                                                "p (c w) -> p c w", w=D),
                                            axis=AX.X, op=OP.add)
                    nc.vector.tensor_tensor(out=q[:, 0:TC], in0=q[:, 0:TC],
                                            in1=aslot, op=OP.add)
                    msg3 = m3
                else:
                    nc.vector.tensor_tensor(out=q[:, 0:TC],
                                            in0=gt3[:, 0:TC, D:D + 1].squeeze(2),
                                            in1=aslot, op=OP.add)
                    msg3 = gt3
                e_t = sp.tile([P, MXC], F32, name="e_t", tag="e_t")
                nc.scalar.activation(e_t[:, 0:TC], q[:, 0:TC], AF.Prelu, alpha=NEG)
                nc.scalar.activation(e_t[:, 0:TC], e_t[:, 0:TC], AF.Exp)
                rhs = ep.tile([P, MXC * W], BF, name="rhs", tag="rhs")
                r3 = rhs[:].rearrange("p (c w) -> p c w", w=W)
                nc.vector.tensor_tensor(
                    out=r3[:, 0:TC, 0:D], in0=msg3[:, 0:TC, 0:D],
                    in1=e_t[:, 0:TC].unsqueeze(2).to_broadcast([P, TC, D]),
                    op=OP.mult)
                nc.vector.tensor_copy(out=r3[:, 0:TC, D:W],
                                      in_=e_t[:, 0:TC].unsqueeze(2))
                amms = []
                bmms = []
                cc = 0
                for w_, ncols, acols, bcol0, amax, bmax in calls:
                    if amax:
                        amms += [cc + c for c in range(acols)]
                    if bmax:
                        bmms += [cc + c for c in range(bcol0, ncols)]
                    cc += ncols
                aggb = sp.tile([P, P], BF, name="aggb", tag="aggb")
                for h, mms, qofs in ((0, amms, 0), (1, bmms, P)):
                    pseg = ps_seg.tile([P, W], F32, name="pseg", tag="pseg")
                    for i, c in enumerate(mms):
                        nc.tensor.matmul(
                            pseg[:],
                            s256[:, c * 2 * P + qofs:c * 2 * P + qofs + P],
                            rhs[:, c * W:(c + 1) * W],
                            start=(i == 0), stop=(i == len(mms) - 1))
                    sn = sp.tile([P, 1], F32, name="sn", tag="sn")
                    nc.vector.tensor_single_scalar(out=sn[:], in_=pseg[:, D:W],
                                                   scalar=1e-16, op=OP.add)
                    rcp = sp.tile([P, 1], F32, name="rcp", tag="rcp")
                    nc.vector.reciprocal(rcp[:], sn[:])
                    nc.vector.tensor_tensor(out=aggb[:, h * D:(h + 1) * D],
                                            in0=pseg[:, 0:D],
                                            in1=rcp[:].to_broadcast([P, D]),
                                            op=OP.mult)
                pT2 = ps.tile([P, P], F32, name="aggT", tag="miscP")
                nc.tensor.matmul(pT2[:], aggb[:], identb[:], start=True, stop=True)
                nc.vector.tensor_copy(out=HXb[:, tp * P:(tp + 1) * P], in_=pT2[:])
                col0 += TC

        # ================= forward =================
        for ci in range(NCH):
            c0, c1 = ci * 512, min((ci + 1) * 512, HC)
            xin = sp.tile([P, 512], BF, name="xin", tag="xin", bufs=1)
            nc.sync.dma_start(out=xin[:, :c1 - c0], in_=xfm_h[:, c0:c1])
            pt = ps_big.tile([P, 512], F32, name="mmp0", tag="mmp")
            nc.tensor.matmul(pt[:, 0:c1 - c0], Wt["lin1_bd"][:],
                             xin[:, 0:c1 - c0], start=True, stop=True)
            nc.scalar.activation(XC[:, c0:c1], pt[:, 0:c1 - c0],
                                 AF.Prelu, bias=Wt["lin1_b"][:, 0:1], alpha=NEG)
        nc.vector.tensor_copy(out=XCb[:], in_=XC[:])
        # --- GATEConv ---
        mm_node(HXb, "gate_w1a_bd", XCb)
        build_table(HXb, None, "gateattrRep2", True, 0)
        edge_phase(True, 0)
        mm_node(HXb, "gate_w2_bd", HXb)
        elu_inplace(HXb, "gate_bias")
        gru_relu("gru0_")
        nc.vector.tensor_copy(out=XCb[:], in_=XC[:])
        # --- atom layers ---
        for l in range(4):
            pre = f"at{l}_"
            mm_node(HXb, pre + "wT", XCb)
            build_table(HXb, pre + "srcRep2", pre + "dstRep2", False, 1 + l)
            edge_phase(False, 1 + l)
            elu_inplace(HXb, pre + "bias")
            gru_relu(pre + "gru_")
            nc.vector.tensor_copy(out=XCb[:], in_=XC[:])

        # ================= readout =================
        mm_node(HXb, "mol_bd", XCb)          # xs into HXb
        asrc_nm = st.tile([P, t_tiles], F32, name="asrc_nm")
        for tp in range(HT):
            pT = ps.tile([P, P], BF, name="xsT", tag="miscP")
            nc.tensor.transpose(out=pT[:], in_=HXb[:, tp * P:(tp + 1) * P],
                                identity=identb[:])
            nc.vector.tensor_copy(out=row_all[:, tp * 2 * D:(tp + 1) * 2 * D],
                                  in_=pT[:])
            tmp = sp.tile([P, P], F32, name="xsm", tag="tabm", bufs=1)
            nc.vector.tensor_tensor(out=tmp[:], in0=pT[:],
                                    in1=Wt["molsrcRep2"][:], op=OP.mult)
            nc.vector.tensor_reduce(asrc_nm[:, 2 * tp:2 * tp + 2].unsqueeze(2),
                                    tmp[:].rearrange("q (h f) -> q h f", f=D),
                                    axis=AX.X, op=OP.add)
        for tp in range(HT):
            pT = ps.tile([P, P], BF, name="xcT", tag="miscP")
            nc.tensor.transpose(out=pT[:], in_=XCb[:, tp * P:(tp + 1) * P],
                                identity=identb[:])
            nc.vector.tensor_copy(out=XCb[:, tp * P:(tp + 1) * P], in_=pT[:])
        xc_nm = XCb
        xs_nm = row_all
        OUTT = st.tile([D, 2 * P], F32, name="OUTT")
        for k in range(2):
            pg = ps_seg.tile([P, D], F32, name="pg", tag="pseg")
            for tp in range(HT):
                sgp = sp.tile([P, 2 * 2 * P], BF, name="sgp", tag="sgp")
                for h in range(2):
                    nc.vector.tensor_scalar(
                        out=sgp[:, h * 2 * P:(h + 1) * 2 * P], in0=iota_sb[:],
                        scalar1=grel32[:, 2 * tp + h:2 * tp + h + 1],
                        scalar2=None, op0=OP.is_equal)
                for h in range(2):
                    j = 2 * tp + h
                    nc.tensor.matmul(
                        pg[:],
                        sgp[:, h * 2 * P + k * P:h * 2 * P + (k + 1) * P],
                        xc_nm[:, j * D:(j + 1) * D],
                        start=(j == 0), stop=(j == t_tiles - 1))
            og = sp.tile([P, D], F32, name="og", tag="og")
            nc.scalar.activation(og[:], pg[:], AF.Relu)
            pTo = ps.tile([D, P], F32, name="ogT", tag="miscP")
            nc.tensor.transpose(out=pTo[:], in_=og[:], identity=ident[:])
            nc.vector.tensor_copy(out=OUTT[:, k * P:(k + 1) * P], in_=pTo[:])
        HG = st.tile([D, 2 * P], F32, name="HG")
        for ts in range(3):
            pxd = ps_big.tile([D, 512], F32, name="xdp", tag="mmp")
            nc.tensor.matmul(pxd[:, 0:2 * P], Wt["mol_wT32"][:], OUTT[:],
                             start=True, stop=True)
            xds = sp.tile([D, 2 * P], F32, name="xds", tag="xds", bufs=1)
            nc.vector.tensor_copy(out=xds[:], in_=pxd[:, 0:2 * P])
            pag = ps.tile([1, 2 * P], F32, name="agp", tag="rowP")
            nc.tensor.matmul(pag[:], Wt["moldstCol"][:], xds[:],
                             start=True, stop=True)
            agr = sp.tile([1, 2 * P], BF, name="agr", tag="agr")
            nc.vector.tensor_copy(out=agr[:], in_=pag[:])
            prg = ps.tile([P, 2 * P], F32, name="repG", tag="rowP")
            nc.tensor.matmul(prg[:], ones1[:], agr[:], start=True, stop=True)
            rep_agr = sp.tile([P, 2 * P], BF, name="rep_agr", tag="rep_agr", bufs=1)
            nc.vector.tensor_copy(out=rep_agr[:], in_=prg[:])
            ag2 = sp.tile([P, 2], F32, name="ag2", tag="ag2")
            psg = [ps_seg.tile([P, W], F32, name=f"psg{k}", tag="pseg")
                   for k in range(2)]
            for tp in range(HT):
                sgp = sp.tile([P, 2 * 2 * P], BF, name="sgp2", tag="sgp")
                for h in range(2):
                    nc.vector.tensor_scalar(
                        out=sgp[:, h * 2 * P:(h + 1) * 2 * P], in0=iota_sb[:],
                        scalar1=grel32[:, 2 * tp + h:2 * tp + h + 1],
                        scalar2=None, op0=OP.is_equal)
                scr2 = sp.tile([P, 2 * 2 * P], BF, name="scr2", tag="scr")
                g3 = scr2[:].rearrange("p (h q) -> p h q", q=2 * P)
                nc.vector.tensor_tensor(
                    out=g3[:],
                    in0=sgp[:].rearrange("p (h q) -> p h q", q=2 * P),
                    in1=rep_agr[:].unsqueeze(1).to_broadcast([P, 2, 2 * P]),
                    op=OP.mult)
                wdt = P
                while wdt >= 16:
                    nc.vector.tensor_tensor(
                        out=g3[:, :, 0:wdt], in0=g3[:, :, 0:wdt],
                        in1=g3[:, :, wdt:2 * wdt], op=OP.add)
                    wdt //= 2
                nc.vector.tensor_reduce(ag2[:].unsqueeze(2), g3[:, :, 0:16],
                                        axis=AX.X, op=OP.add)
                q2 = sp.tile([P, 2], F32, name="q2", tag="q2")
                nc.vector.tensor_tensor(out=q2[:], in0=asrc_nm[:, 2 * tp:2 * tp + 2],
                                        in1=ag2[:], op=OP.add)
                nc.scalar.activation(q2[:], q2[:], AF.Prelu, alpha=NEG)
                nc.scalar.activation(q2[:], q2[:], AF.Exp)
                rh = ep.tile([P, 2 * W], BF, name="rh", tag="rh", bufs=2)
                rh3 = rh[:].rearrange("p (h w) -> p h w", w=W)
                nc.vector.tensor_tensor(
                    out=rh3[:, :, 0:D],
                    in0=xs_nm[:, tp * 2 * D:(tp + 1) * 2 * D].rearrange(
                        "p (h f) -> p h f", f=D),
                    in1=q2[:].unsqueeze(2).to_broadcast([P, 2, D]),
                    op=OP.mult)
                nc.vector.tensor_copy(out=rh3[:, :, D:W], in_=q2[:].unsqueeze(2))
                for h in range(2):
                    j = 2 * tp + h
                    for k in range(2):
                        nc.tensor.matmul(
                            psg[k][:],
                            sgp[:, h * 2 * P + k * P:h * 2 * P + (k + 1) * P],
                            rh[:, h * W:(h + 1) * W],
                            start=(j == 0), stop=(j == t_tiles - 1))
            for k in range(2):
                sn = sp.tile([P, 1], F32, name="sng", tag="sn")
                nc.vector.tensor_single_scalar(out=sn[:], in_=psg[k][:, D:W],
                                               scalar=1e-16, op=OP.add)
                rcp = sp.tile([P, 1], F32, name="rcpg", tag="rcp")
                nc.vector.reciprocal(rcp[:], sn[:])
                aggg = sp.tile([P, D], F32, name="aggg", tag="aggg")
                nc.vector.tensor_tensor(out=aggg[:], in0=psg[k][:, 0:D],
                                        in1=rcp[:].to_broadcast([P, D]), op=OP.mult)
                nc.vector.tensor_tensor(out=aggg[:], in0=aggg[:],
                                        in1=Wt["mol_biasRep"][:], op=OP.add)
                r = sp.tile([P, D], F32, name="rg", tag="rg")
                nc.scalar.activation(r[:], aggg[:], AF.Relu)
                xm = sp.tile([P, D], F32, name="xmg", tag="xmg")
                nc.vector.tensor_sub(xm[:], aggg[:], r[:])
                nc.scalar.activation(xm[:], xm[:], AF.Exp)
                nc.vector.scalar_tensor_tensor(out=aggg[:], in0=xm[:], scalar=-1.0,
                                               in1=r[:], op0=OP.add, op1=OP.add)
                pTh = ps.tile([D, P], F32, name="hgT", tag="miscP")
                nc.tensor.transpose(out=pTh[:], in_=aggg[:], identity=ident[:])
                nc.vector.tensor_copy(out=HG[:, k * P:(k + 1) * P], in_=pTh[:])
            wih = Wt["mol_gru_wih"]
            whh = Wt["mol_gru_whh"]
            bs = Wt["mol_gru_bsum"]
            def gate2g(g):
                pt = ps_big.tile([D, 512], F32, name="ggp", tag="mmp")
                nc.tensor.matmul(pt[:, 0:2 * P], wih[:, g * D:(g + 1) * D], HG[:],
                                 start=True, stop=False)
                nc.tensor.matmul(pt[:, 0:2 * P], whh[:, g * D:(g + 1) * D], OUTT[:],
                                 start=False, stop=True)
                return pt
            prg2 = gate2g(0)
            rg2 = sp.tile([D, 2 * P], F32, name="ggr", tag="ggr", bufs=1)
            nc.scalar.activation(rg2[:], prg2[:, 0:2 * P], AF.Sigmoid, bias=bs[:, 0:1])
            pzg = gate2g(1)
            zg = sp.tile([D, 2 * P], F32, name="ggz", tag="ggz", bufs=1)
            nc.scalar.activation(zg[:], pzg[:, 0:2 * P], AF.Sigmoid, bias=bs[:, 1:2])
            pig = ps_big.tile([D, 512], F32, name="ggpi", tag="mmp")
            nc.tensor.matmul(pig[:, 0:2 * P], wih[:, 2 * D:3 * D], HG[:],
                             start=True, stop=True)
            phg = ps_big.tile([D, 512], F32, name="ggph", tag="mmp")
            nc.tensor.matmul(phg[:, 0:2 * P], whh[:, 2 * D:3 * D], OUTT[:],
                             start=True, stop=True)
            hng = sp.tile([D, 2 * P], F32, name="gghn", tag="gghn", bufs=1)
            nc.vector.tensor_scalar(out=hng[:], in0=phg[:, 0:2 * P],
                                    scalar1=Wt["mol_gru_bhh"][:, 2:3],
                                    scalar2=None, op0=OP.add)
            nc.vector.tensor_mul(hng[:], hng[:], rg2[:])
            nc.vector.tensor_tensor(out=hng[:], in0=hng[:], in1=pig[:, 0:2 * P],
                                    op=OP.add)
            ng = sp.tile([D, 2 * P], F32, name="ggn", tag="ggn", bufs=1)
            nc.scalar.activation(ng[:], hng[:], AF.Tanh,
                                 bias=Wt["mol_gru_bih"][:, 2:3])
            dg = sp.tile([D, 2 * P], F32, name="ggd", tag="ggd", bufs=1)
            nc.vector.tensor_sub(dg[:], OUTT[:], ng[:])
            nc.vector.tensor_mul(dg[:], dg[:], zg[:])
            nc.vector.tensor_tensor(out=dg[:], in0=dg[:], in1=ng[:], op=OP.add)
            nc.scalar.activation(OUTT[:], dg[:], AF.Relu)
        py = ps.tile([1, 2 * P], F32, name="py", tag="rowP")
        nc.tensor.matmul(py[:], Wt["lin2_wT"][:], OUTT[:], start=True, stop=True)
        ysb = sp.tile([1, 2 * P], F32, name="ysb", tag="ysb")
        nc.vector.tensor_scalar(out=ysb[:], in0=py[:], scalar1=Wt["lin2_b"][0:1, 0:1],
                                scalar2=None, op0=OP.add)
        nc.sync.dma_start(out=y_out[:], in_=ysb[0:1, 0:G_LOC])
    nc.compile()
    return nc


_CACHE = {}


def kernel(**inputs):
    from concourse.bass_utils import run_bass_kernel_spmd
    x = np.asarray(inputs["x"], np.float32)
    ei = np.asarray(inputs["edge_index"])
    ea = np.asarray(inputs["edge_attr"], np.float32)
    bt = np.asarray(inputs["batch"])
    per, n_pad, t_tiles, meta, TOTC = _prep(x, ei, ea, bt)
    kwf = {k: np.asarray(v, np.float32) for k, v in inputs.items()
           if k not in ("x", "edge_index", "edge_attr", "batch")}
    weights = _mk_weights(kwf)
    key = (n_pad, TOTC, tuple(tuple(tuple(cl) for cl in calls) for calls in meta[0]),
           tuple(meta[1]))
    if key not in _CACHE:
        _CACHE[key] = _build(n_pad, t_tiles, meta, TOTC,
                             {k: (v.shape, v.dtype == F16)
                              for k, v in weights.items()})
    nc = _CACHE[key]
    iota = np.tile(np.arange(2 * P).astype(np.float32)[None, :], (P, 1)).astype(F16)
    w1b = kwf["gate_lin1_w"][:, D:]
    in_maps = []
    for c in range(NC):
        b_e = (per[c]["attr_s"] @ w1b.T).astype(F16)
        b_e = np.ascontiguousarray(
            b_e.reshape(TOTC, P, D).transpose(1, 0, 2).reshape(P, TOTC * D))
        m = dict(xfm=per[c]["xfm"], idx16=per[c]["idx16"], s256=per[c]["s256"],
                 b_e=b_e, grel=per[c]["grel"], iota256=iota)
        for k, v in weights.items():
            m["w_" + k] = v
        in_maps.append(m)
    res = run_bass_kernel_spmd(nc, in_maps, core_ids=list(range(NC)))
    return np.concatenate([res.results[c]["y"][0] for c in range(NC)]).astype(np.float32)


# revision 24
# speedup vs baseline: 1.9510x; 1.0196x over previous
"""AttentiveFP forward on 8 Trainium2 NeuronCores (Bass/Tile).

Sharding: 2048 graphs (nodes contiguous, batch sorted) split into 8 blocks of
256 graphs; each core owns the edges whose dst node falls in its block. Per
round each core computes its nodes' features, all-gathers a compact per-node
table [xt | alpha_src] (fp16, 65 wide), expands it locally to 256B-aligned
rows, then fetches per-edge src rows with nc.gpsimd.dma_gather (the token
gather ucode: thousands of int16 indices per call, round-robined over 4 SWDGE
queues). Indices are int16, so slots are grouped per (node-tile-pair,
32768-row source window); within a call, tile-a slots carry rel in [0,128)
and tile-b slots rel in [128,256), so one 256-wide is_equal one-hot serves
both tiles' PSUM segment-matmuls and the alpha_dst select (one-hot x
replicated alpha row, reduced on DVE). Per-edge alpha_dst needs no gather
(dst is always local). Node phases run feature-major, half-packed, with
block-diagonal [128,128] fp16 weights; GRU hidden state stays fp32. The gate
round's edge-attr term (W1b @ e_attr) is precomputed on the host per slot.
Readout uses a 256-wide graph one-hot per tile-pair plus a replicated
per-graph alpha row (no gathers).

Softmax max-subtraction is skipped (logits O(1), shift-invariant).
"""
import sys
sys.path.insert(0, '/opt/trn_rl_repo')
sys.path.insert(0, '/root/.axon_site')

import numpy as np

F16 = np.float16
NC = 8
D = 64
G_TOT = 2048
G_LOC = G_TOT // NC
F_IN = 25
E_DIM = 4
NEG = 0.01
P = 128
W = 65            # compact table row: [xt(64) | alpha_src]
WG = 128          # gathered row width (256B-aligned)
WIN = 32768       # int16 index window (rows)


def _prep(x, edge_index, edge_attr, batch):
    src = edge_index[0].astype(np.int64)
    dst = edge_index[1].astype(np.int64)
    batch = batch.astype(np.int64)

    gstart = np.searchsorted(batch, np.arange(0, G_TOT + 1, G_LOC))
    n0 = gstart[:-1]
    nloc = np.diff(gstart)
    n_pad = int(np.ceil((nloc.max() + 1) / 256) * 256)
    t_tiles = n_pad // P
    HC = n_pad // 2
    HT = t_tiles // 2
    NW = 4
    bp = HT // NW
    pr_cnt = [bp + (1 if i < HT % NW else 0) for i in range(NW)]
    sbp = np.cumsum([0] + pr_cnt)                 # pair boundaries per slice
    sbr = sbp * 2 * P                             # row boundaries per slice
    srows = np.diff(sbr)

    def pi_row(n):
        h = n // HC
        r = n % HC
        return (2 * (r // P) + h) * P + (r % P)

    src_dev = np.searchsorted(gstart[1:], src, side='right')
    dst_dev = np.searchsorted(gstart[1:], dst, side='right')
    pr_all = pi_row(src - n0[src_dev])
    w_all = np.searchsorted(sbr[1:], pr_all, side='right')
    blk0 = np.concatenate([[0], np.cumsum(srows * NC)])
    gidx_all = (blk0[w_all] + src_dev * srows[w_all]
                + (pr_all - sbr[w_all]))

    # ---- pass 1: bucket edges per core into (pair, window, half) ----
    buckets = [[[[None, None] for _ in range(NW)] for _ in range(HT)]
               for _ in range(NC)]
    for c in range(NC):
        sel = np.where(dst_dev == c)[0]
        dl = dst[sel] - n0[c]
        j_dst = 2 * ((dl % HC) // P) + dl // HC
        p_dst = dl % P
        gi = gidx_all[sel]
        w_of = np.searchsorted(blk0[1:], gi, side='right')
        for tp in range(HT):
            for h in range(2):
                m = j_dst == 2 * tp + h
                gi_m, p_m, w_m, sel_m = gi[m], p_dst[m], w_of[m], sel[m]
                for w in range(NW):
                    mm = w_m == w
                    buckets[c][tp][w][h] = (gi_m[mm] - blk0[w], p_m[mm],
                                            sel_m[mm])
    # ---- pass 2: SPMD-uniform call metadata (max counts over cores) ----
    meta = []       # per pair: [w, ncols, acols, bcol0, amax, bmax]
    for tp in range(HT):
        calls = []
        for w in range(NW):
            amax = max(len(buckets[c][tp][w][0][0]) for c in range(NC))
            bmax = max(len(buckets[c][tp][w][1][0]) for c in range(NC))
            if amax + bmax == 0:
                continue
            ncols = (amax + bmax + P - 1) // P
            calls.append([w, ncols, (amax + P - 1) // P, amax // P, amax, bmax])
        if not any(cl[4] for cl in calls):
            calls.insert(0, [0, 1, 1, 0, P, 0])
        if not any(cl[5] for cl in calls):
            calls.append([0, 1, 0, 0, 0, P])
        meta.append(calls)
    TOTC = sum(cl[1] for calls in meta for cl in calls)

    per = []
    for c in range(NC):
        idx16 = np.zeros((16, TOTC * 8), np.int16)
        rel = np.full((P, TOTC), 300.0, np.float32)
        attr_s = np.zeros((TOTC * P, E_DIM), np.float32)
        col0 = 0
        for tp in range(HT):
            for w_, ncols, acols, bcol0, amax, bmax in meta[tp]:
                flat_idx = np.zeros(ncols * P, np.int16)
                flat_rel = np.full(ncols * P, 300.0, np.float32)
                flat_attr = np.zeros((ncols * P, E_DIM), np.float32)
                pos = 0
                for h, hmax in ((0, amax), (1, bmax)):
                    gi_l, p_l, sel_l = buckets[c][tp][w_][h]
                    k = len(gi_l)
                    flat_idx[pos:pos + k] = gi_l.astype(np.int16)
                    flat_rel[pos:pos + k] = p_l + h * P
                    flat_attr[pos:pos + k] = edge_attr[sel_l]
                    pos += hmax
                idx16[:, col0 * 8:(col0 + ncols) * 8] = \
                    flat_idx.reshape(ncols * 8, 16).T
                rel[:, col0:col0 + ncols] = flat_rel.reshape(ncols, P).T
                attr_s[col0 * P:(col0 + ncols) * P] = flat_attr
                col0 += ncols
        s256 = (rel.astype(np.int32)[:, :, None] ==
                np.arange(2 * P, dtype=np.int32)[None, None, :]).astype(F16)
        per.append(dict(idx16=np.tile(idx16, (8, 1)),
                        s256=np.ascontiguousarray(s256.reshape(P, TOTC * 2 * P)),
                        attr_s=attr_s))
        nl = int(nloc[c])
        gl = batch[n0[c]:n0[c] + nl] - G_LOC * c
        grel = np.full((P, t_tiles), 300.0, np.float32)
        n_ids = np.arange(n_pad)
        h_a = n_ids // HC
        j_a = 2 * ((n_ids % HC) // P) + h_a
        p_a = n_ids % P
        valid = n_ids < nl
        grel[p_a[valid], j_a[valid]] = gl[n_ids[valid]]
        per[c]['grel'] = np.ascontiguousarray(grel.astype(F16))
        xp = np.zeros((n_pad, F_IN), np.float32)
        xp[:nl] = x[n0[c]:n0[c] + nl]
        xfm = np.zeros((P, HC), np.float32)
        xfm[:F_IN] = xp[:HC].T
        xfm[D:D + F_IN] = xp[HC:].T
        per[c]['xfm'] = xfm.astype(F16)
    return per, n_pad, t_tiles, (meta, [int(v) for v in sbp]), TOTC


def _mk_weights(kw):
    w = {}
    def bd(a):
        t = a.T
        z = np.zeros((P, P), np.float32)
        z[0:D, 0:D] = t
        z[D:2 * D, D:2 * D] = t
        return z
    def col(a):
        return np.concatenate([a, a])[:, None]
    def rep2(a):
        return np.tile(np.concatenate([a, a])[None, :], (P, 1))
    def rep1(a):
        return np.tile(a[None, :], (P, 1))
    def gb(a):
        t = a.reshape(3, D).T
        return np.concatenate([t, t], 0)
    def gru_bd(wg):
        out = np.zeros((P, 3 * P), np.float32)
        for g in range(3):
            out[:, g * P:(g + 1) * P] = bd(wg[g * D:(g + 1) * D])
        return out

    B, F = 'b', 'f'
    lin1 = np.zeros((P, P), np.float32)
    lin1[0:F_IN, 0:D] = kw["lin1_w"].T
    lin1[D:D + F_IN, D:2 * D] = kw["lin1_w"].T
    w["lin1_bd"] = (lin1, B)
    w["lin1_b"] = (col(kw["lin1_b"]), F)
    w["gate_w1a_bd"] = (bd(kw["gate_lin1_w"][:, :D]), B)
    w["attlRep"] = (rep1(kw["gate_att_l"]), B)
    w["gateattrRep2"] = (rep2(kw["gate_att_r"]), B)
    w["gate_w2_bd"] = (bd(kw["gate_lin2_w"]), B)
    w["gate_bias"] = (col(kw["gate_bias"]), F)
    w["gru0_wih"] = (gru_bd(kw["gru0_wih"]), B)
    w["gru0_whh"] = (gru_bd(kw["gru0_whh"]), B)
    w["gru0_bih"] = (gb(kw["gru0_bih"]), F)
    w["gru0_bhh"] = (gb(kw["gru0_bhh"]), F)
    w["gru0_bsum"] = (gb(kw["gru0_bih"] + kw["gru0_bhh"]), F)
    for l in range(4):
        pre = f"at{l}_"
        w[pre + "wT"] = (bd(kw["atom_lin_w"][l]), B)
        w[pre + "srcRep2"] = (rep2(kw["atom_att_src"][l]), B)
        w[pre + "dstRep2"] = (rep2(kw["atom_att_dst"][l]), B)
        w[pre + "bias"] = (col(kw["atom_bias"][l]), F)
        w[pre + "gru_wih"] = (gru_bd(kw["atom_gru_wih"][l]), B)
        w[pre + "gru_whh"] = (gru_bd(kw["atom_gru_whh"][l]), B)
        w[pre + "gru_bih"] = (gb(kw["atom_gru_bih"][l]), F)
        w[pre + "gru_bhh"] = (gb(kw["atom_gru_bhh"][l]), F)
        w[pre + "gru_bsum"] = (gb(kw["atom_gru_bih"][l] + kw["atom_gru_bhh"][l]), F)
    w["mol_bd"] = (bd(kw["mol_lin_w"]), B)
    w["mol_wT32"] = (kw["mol_lin_w"].T.copy(), F)
    w["molsrcRep2"] = (rep2(kw["mol_att_src"]), B)
    w["moldstCol"] = (kw["mol_att_dst"][:, None].copy(), F)
    w["mol_biasRep"] = (rep1(kw["mol_bias"]), F)
    w["mol_gru_wih"] = (kw["mol_gru_wih"].T.copy(), F)
    w["mol_gru_whh"] = (kw["mol_gru_whh"].T.copy(), F)
    w["mol_gru_bih"] = (gb(kw["mol_gru_bih"])[:D], F)
    w["mol_gru_bhh"] = (gb(kw["mol_gru_bhh"])[:D], F)
    w["mol_gru_bsum"] = (gb(kw["mol_gru_bih"] + kw["mol_gru_bhh"])[:D], F)
    w["lin2_wT"] = (kw["lin2_w"].T.copy(), F)
    w["lin2_b"] = (kw["lin2_b"][:, None].copy(), F)
    out = {}
    for k, (v, tag) in w.items():
        v = np.ascontiguousarray(v, np.float32)
        out[k] = v.astype(F16) if tag == B else v
    return out


def _build(n_pad, t_tiles, meta_in, TOTC, wmeta):
    meta, sbp = meta_in
    import concourse.bacc as bacc
    import concourse.mybir as mybir
    import concourse.tile as tile
    from concourse.masks import make_identity

    dt = mybir.dt
    AF = mybir.ActivationFunctionType
    OP = mybir.AluOpType
    AX = mybir.AxisListType
    BF = dt.float16
    F32 = dt.float32

    HC = n_pad // 2
    HT = t_tiles // 2
    NCH = (HC + 511) // 512
    MXC = max(sum(cl[1] for cl in calls) for calls in meta)
    NW = 4
    sbr = [b * 2 * P for b in sbp]                 # per-core slice row bounds
    srows = [sbr[i + 1] - sbr[i] for i in range(NW)]
    blk0 = [0]
    for i in range(NW):
        blk0.append(blk0[-1] + srows[i] * NC)

    nc = bacc.Bacc("TRN2", target_bir_lowering=False, debug=False, num_devices=NC,
                   num_swdge_queues=4)

    xfm_h = nc.dram_tensor("xfm", [P, HC], BF, kind="ExternalInput")
    idx_h = nc.dram_tensor("idx16", [P, TOTC * 8], dt.int16, kind="ExternalInput")
    s256_h = nc.dram_tensor("s256", [P, TOTC * 2 * P], BF, kind="ExternalInput")
    be_h = nc.dram_tensor("b_e", [P, TOTC * D], BF, kind="ExternalInput")
    grel_h = nc.dram_tensor("grel", [P, t_tiles], BF, kind="ExternalInput")
    iota_h = nc.dram_tensor("iota256", [P, 2 * P], BF, kind="ExternalInput")
    cst_h = {k: nc.dram_tensor("w_" + k, list(s_), BF if isbf else F32,
                               kind="ExternalInput")
             for k, (s_, isbf) in wmeta.items()}
    y_out = nc.dram_tensor("y", [1, G_LOC], F32, kind="ExternalOutput")

    with tile.TileContext(nc) as tc:
      with (
        tc.tile_pool(name="cst", bufs=1) as cst,
        tc.tile_pool(name="st", bufs=1) as st,
        tc.tile_pool(name="ep", bufs=2) as ep,
        tc.tile_pool(name="sp", bufs=2) as sp,
        tc.tile_pool(name="ps", bufs=2, space="PSUM") as ps,
        tc.tile_pool(name="ps_seg", bufs=2, space="PSUM") as ps_seg,
        tc.tile_pool(name="ps_big", bufs=2, space="PSUM") as ps_big,
        tc.tile_pool(name="dram", bufs=1, space="DRAM") as dp,
      ):
        def load(name):
            h = cst_h[name]
            t = cst.tile(list(h.shape), h.dtype, name="c_" + name)
            nc.sync.dma_start(out=t[:], in_=h[:])
            return t
        Wt = {k: load(k) for k in cst_h}
        idx_sb = cst.tile([P, TOTC * 8], dt.int16, name="idx_sb")
        nc.sync.dma_start(out=idx_sb[:], in_=idx_h[:])
        grel_sb = cst.tile([P, t_tiles], BF, name="grel_sb")
        nc.sync.dma_start(out=grel_sb[:], in_=grel_h[:])
        grel32 = cst.tile([P, t_tiles], F32, name="grel32")
        nc.vector.tensor_copy(out=grel32[:], in_=grel_sb[:])
        iota_sb = cst.tile([P, 2 * P], BF, name="iota_sb")
        nc.sync.dma_start(out=iota_sb[:], in_=iota_h[:])
        identb = cst.tile([P, P], BF, name="identb")
        make_identity(nc, identb[:])
        ident = cst.tile([P, P], F32, name="ident")
        make_identity(nc, ident[:])
        ones1 = cst.tile([1, P], BF, name="ones1")
        nc.vector.memset(ones1[:], 1.0)
        onesf = cst.tile([P, P], BF, name="onesf")
        nc.vector.memset(onesf[:], 1.0)

        XC = st.tile([P, HC], F32, name="XC")
        XCb = st.tile([P, HC], BF, name="XCb")
        HXb = st.tile([P, HC], BF, name="HXb")
        ad_nm = st.tile([P, t_tiles], F32, name="ad_nm")
        adTs = st.tile([P, P], BF, name="adTs")
        row_all = st.tile([P, HT * 2 * WG], BF, name="row_all")
        tbl_locs = [dp.tile([n_pad, WG], BF, name=f"tbl_loc{i}") for i in range(5)]
        tbl_alls = [[dp.tile([NC * srows[w], WG], BF, addr_space="Shared",
                             name=f"tbl_all{i}_{w}") for w in range(NW)]
                    for i in range(5)]

        def mm_node(dst, wkey, srcb, act=AF.Copy, bias=None, alpha=0.0):
            for ci in range(NCH):
                c0, c1 = ci * 512, min((ci + 1) * 512, HC)
                pt = ps_big.tile([P, 512], F32, name="mmp", tag="mmp")
                nc.tensor.matmul(pt[:, 0:c1 - c0], Wt[wkey][:], srcb[:, c0:c1],
                                 start=True, stop=True)
                b = Wt[bias][:, 0:1] if bias else 0.0
                nc.scalar.activation(dst[:, c0:c1], pt[:, 0:c1 - c0],
                                     act, bias=b, alpha=alpha)

        def elu_inplace(t_fm, bias):
            for ci in range(NCH):
                c0, c1 = ci * 512, min((ci + 1) * 512, HC)
                w_ = c1 - c0
                v = t_fm[:, c0:c1]
                tin = sp.tile([P, 512], F32, name="eluin", tag="eluin", bufs=1)
                nc.scalar.activation(tin[:, 0:w_], v, AF.Prelu,
                                     bias=Wt[bias][:, 0:1], alpha=1.0)
                r = sp.tile([P, 512], F32, name="elur", tag="elur", bufs=1)
                nc.scalar.activation(r[:, 0:w_], tin[:, 0:w_], AF.Relu)
                nc.vector.tensor_sub(tin[:, 0:w_], tin[:, 0:w_], r[:, 0:w_])
                nc.scalar.activation(tin[:, 0:w_], tin[:, 0:w_], AF.Exp)
                nc.vector.scalar_tensor_tensor(
                    out=v, in0=tin[:, 0:w_], scalar=-1.0,
                    in1=r[:, 0:w_], op0=OP.add, op1=OP.add)

        def gru_relu(pre, nxt=None):
            wih = Wt[pre + "wih"]
            whh = Wt[pre + "whh"]
            bs = Wt[pre + "bsum"]
            for ci in range(NCH):
                c0, c1 = ci * 512, min((ci + 1) * 512, HC)
                w_ = c1 - c0
                def gate2(g):
                    pt = ps_big.tile([P, 512], F32, name="grup", tag="mmp")
                    nc.tensor.matmul(pt[:, 0:w_], wih[:, g * P:(g + 1) * P],
                                     HXb[:, c0:c1], start=True, stop=False)
                    nc.tensor.matmul(pt[:, 0:w_], whh[:, g * P:(g + 1) * P],
                                     XCb[:, c0:c1], start=False, stop=True)
                    return pt
                pr = gate2(0)
                r = sp.tile([P, 512], F32, name="grur", tag="grur", bufs=1)
                nc.scalar.activation(r[:, 0:w_], pr[:, 0:w_], AF.Sigmoid,
                                     bias=bs[:, 0:1])
                pz = gate2(1)
                z = sp.tile([P, 512], F32, name="gruz", tag="gruz", bufs=1)
                nc.scalar.activation(z[:, 0:w_], pz[:, 0:w_], AF.Sigmoid,
                                     bias=bs[:, 1:2])
                pin = ps_big.tile([P, 512], F32, name="grupi", tag="mmp")
                nc.tensor.matmul(pin[:, 0:w_], wih[:, 2 * P:3 * P],
                                 HXb[:, c0:c1], start=True, stop=True)
                phn = ps_big.tile([P, 512], F32, name="gruph", tag="mmp")
                nc.tensor.matmul(phn[:, 0:w_], whh[:, 2 * P:3 * P],
                                 XCb[:, c0:c1], start=True, stop=True)
                hn = sp.tile([P, 512], F32, name="gruhn", tag="gruhn", bufs=1)
                nc.scalar.activation(hn[:, 0:w_], phn[:, 0:w_], AF.Prelu,
                                     bias=Wt[pre + "bhh"][:, 2:3], alpha=1.0)
                nc.vector.tensor_mul(hn[:, 0:w_], hn[:, 0:w_], r[:, 0:w_])
                nc.vector.tensor_tensor(out=hn[:, 0:w_], in0=hn[:, 0:w_],
                                        in1=pin[:, 0:w_], op=OP.add)
                n_t = sp.tile([P, 512], F32, name="grun", tag="grun", bufs=1)
                nc.scalar.activation(n_t[:, 0:w_], hn[:, 0:w_], AF.Tanh,
                                     bias=Wt[pre + "bih"][:, 2:3])
                d_t = sp.tile([P, 512], F32, name="grud", tag="grud", bufs=1)
                nc.vector.tensor_sub(d_t[:, 0:w_], XC[:, c0:c1], n_t[:, 0:w_])
                nc.vector.tensor_mul(d_t[:, 0:w_], d_t[:, 0:w_], z[:, 0:w_])
                nc.vector.tensor_tensor(out=d_t[:, 0:w_], in0=d_t[:, 0:w_],
                                        in1=n_t[:, 0:w_], op=OP.add)
                nc.scalar.activation(XC[:, c0:c1], d_t[:, 0:w_], AF.Relu)
                nc.vector.tensor_copy(out=XCb[:, c0:c1], in_=XC[:, c0:c1])
                if nxt is not None:
                    ptn = ps_big.tile([P, 512], F32, name="mmpn", tag="mmp")
                    nc.tensor.matmul(ptn[:, 0:w_], Wt[nxt][:], XCb[:, c0:c1],
                                     start=True, stop=True)
                    nc.scalar.activation(HXb[:, c0:c1], ptn[:, 0:w_], AF.Copy)

        def build_table(srcb, srcRep2, dstRep2, ad_from_xc, li):
            for tp in range(HT):
                cc = tp * P
                pT = ps.tile([P, P], BF, name="tabT", tag="miscP")
                nc.tensor.transpose(out=pT[:], in_=srcb[:, cc:cc + P],
                                    identity=identb[:])
                row3 = row_all[:, tp * 2 * WG:(tp + 1) * 2 * WG].rearrange(
                    "p (h w) -> p h w", w=WG)
                nc.vector.tensor_copy(
                    out=row3[:, :, 0:D],
                    in_=pT[:].rearrange("q (h f) -> q h f", f=D))
                if srcRep2 is None:
                    nc.vector.memset(row3[:, :, D:W], 0.0)
                else:
                    tmp = sp.tile([P, P], F32, name="tabm", tag="tabm", bufs=1)
                    nc.vector.tensor_tensor(out=tmp[:], in0=pT[:],
                                            in1=Wt[srcRep2][:], op=OP.mult)
                    asr = sp.tile([P, 2], F32, name="asr", tag="asr")
                    nc.vector.tensor_reduce(asr[:].unsqueeze(2),
                                            tmp[:].rearrange("q (h f) -> q h f", f=D),
                                            axis=AX.X, op=OP.add)
                    nc.vector.tensor_copy(out=row3[:, :, D:W],
                                          in_=asr[:].unsqueeze(2))
                if ad_from_xc:
                    pTx = ps.tile([P, P], BF, name="tabTx", tag="miscP")
                    nc.tensor.transpose(out=pTx[:], in_=XCb[:, cc:cc + P],
                                        identity=identb[:])
                    dsrc = pTx
                else:
                    dsrc = pT
                tmp2 = sp.tile([P, P], F32, name="tabm2", tag="tabm2", bufs=1)
                nc.vector.tensor_tensor(out=tmp2[:], in0=dsrc[:],
                                        in1=Wt[dstRep2][:], op=OP.mult)
                nc.vector.tensor_reduce(ad_nm[:, 2 * tp:2 * tp + 2].unsqueeze(2),
                                        tmp2[:].rearrange("q (h f) -> q h f", f=D),
                                        axis=AX.X, op=OP.add)
            for w_ in range(NW):
                nc.sync.dma_start(
                    out=tbl_locs[li][sbr[w_]:sbr[w_ + 1], :].rearrange(
                        "(tp h p) w -> p tp h w", h=2, p=P),
                    in_=row_all[:, sbp[w_] * 2 * WG:sbp[w_ + 1] * 2 * WG])
            # alpha_dst transposed: adTs[j, q] = ad of node (tile j, row q)
            pAd = ps.tile([P, P], F32, name="adT", tag="miscP")
            nc.tensor.transpose(out=pAd[0:t_tiles, :], in_=ad_nm[:],
                                identity=ident[:])
            nc.vector.tensor_copy(out=adTs[0:t_tiles, :], in_=pAd[0:t_tiles, :])
            for w_ in range(NW):
                nc.gpsimd.collective_compute(
                    "AllGather", mybir.AluOpType.bypass,
                    replica_groups=[list(range(NC))],
                    ins=[tbl_locs[li][sbr[w_]:sbr[w_ + 1], :].opt()],
                    outs=[tbl_alls[li][w_].opt()])

        def edge_phase(is_gate, li):
            """Message round; writes agg (normalized, fp16) into HXb (fm)."""
            col0 = 0
            qrr = 0
            for tp in range(HT):
                calls = meta[tp]
                TC = sum(cl[1] for cl in calls)
                ar2 = sp.tile([1, 2 * P], BF, name="ar2", tag="ar2")
                nc.sync.dma_start(out=ar2[0:1, :], in_=adTs[2 * tp:2 * tp + 2, :])
                prp = ps.tile([P, 2 * P], F32, name="repP", tag="rowP")
                nc.tensor.matmul(prp[:], ones1[:], ar2[0:1, :],
                                 start=True, stop=True)
                rep_pair = sp.tile([P, 2 * P], BF, name="rep_pair", tag="rep_pair")
                nc.vector.tensor_copy(out=rep_pair[:], in_=prp[:])
                gt = ep.tile([P, MXC * WG], BF, name="gt", tag="gt", bufs=3)
                cc = 0
                for w_, ncols, acols, bcol0, amax, bmax in calls:
                    nc.gpsimd.dma_gather(
                        out_ap=gt[:, cc * WG:(cc + ncols) * WG].rearrange(
                            "p (c w) -> p c w", w=WG),
                        in_ap=tbl_alls[li][w_][:],
                        idxs_ap=idx_sb[:, (col0 + cc) * 8:(col0 + cc + ncols) * 8],
                        num_idxs=ncols * P, num_idxs_reg=ncols * P,
                        elem_size=WG, queue_num=qrr % 4)
                    qrr += 1
                    cc += ncols
                gt3 = gt[:].rearrange("p (c w) -> p c w", w=WG)
                s256 = ep.tile([P, MXC * 2 * P], BF, name="s256", tag="s256")
                nc.sync.dma_start(out=s256[:, 0:TC * 2 * P],
                                  in_=s256_h[:, col0 * 2 * P:(col0 + TC) * 2 * P])
                sat = ep.tile([P, MXC * 2 * P], BF, name="sat", tag="sat", bufs=1)
                nc.vector.tensor_tensor(
                    out=sat[:, :TC * 2 * P].rearrange("p (c q) -> p c q", q=2 * P),
                    in0=s256[:, :TC * 2 * P].rearrange("p (c q) -> p c q", q=2 * P),
                    in1=rep_pair[:].unsqueeze(1).to_broadcast([P, TC, 2 * P]),
                    op=OP.mult)
                s3 = sat[:].rearrange("p (c q) -> p c q", q=2 * P)
                wdt = P
                while wdt >= 16:
                    nc.vector.tensor_tensor(
                        out=s3[:, 0:TC, 0:wdt], in0=s3[:, 0:TC, 0:wdt],
                        in1=s3[:, 0:TC, wdt:2 * wdt], op=OP.add)
                    wdt //= 2
                asl = sp.tile([P, MXC], F32, name="asl", tag="asl")
                nc.vector.tensor_reduce(asl[:, 0:TC].unsqueeze(2),
                                        s3[:, 0:TC, 0:16], axis=AX.X, op=OP.add)
                aslot = asl[:, 0:TC]
                q = sp.tile([P, MXC], F32, name="q", tag="q")
                if is_gate:
                    be_sb = ep.tile([P, MXC * D], BF, name="be_sb", tag="be_sb", bufs=1)
                    nc.sync.dma_start(out=be_sb[:, 0:TC * D],
                                      in_=be_h[:, col0 * D:(col0 + TC) * D])
                    m_all = ep.tile([P, MXC * D], BF, name="m_all", tag="m_all")
                    m3 = m_all[:].rearrange("p (c w) -> p c w", w=D)
                    nc.vector.tensor_tensor(
                        out=m3[:, 0:TC, :], in0=gt3[:, 0:TC, 0:D],
                        in1=be_sb[:, 0:TC * D].rearrange("p (c w) -> p c w", w=D),
                        op=OP.add)
                    nc.scalar.activation(m_all[:, :TC * D], m_all[:, :TC * D],
                                         AF.Prelu, alpha=NEG)
                    lt = ep.tile([P, MXC * D], BF, name="lt", tag="lt", bufs=1)
                    nc.vector.tensor_tensor(
                        out=lt[:, :TC * D], in0=m3[:, 0:TC, :],
                        in1=Wt["attlRep"][:].unsqueeze(1).to_broadcast([P, TC, D]),
                        op=OP.mult)
                    nc.vector.tensor_reduce(q[:, 0:TC].unsqueeze(2),
                                            lt[:, :TC * D].rearrange(
                                                "p (c w) -> p c w", w=D),
                                            axis=AX.X, op=OP.add)
                    nc.vector.tensor_tensor(out=q[:, 0:TC], in0=q[:, 0:TC],
                                            in1=aslot, op=OP.add)
                    msg3 = m3
                else:
                    nc.vector.tensor_tensor(out=q[:, 0:TC],
                                            in0=gt3[:, 0:TC, D:D + 1].squeeze(2),
                                            in1=aslot, op=OP.add)
                    msg3 = gt3
                e_t = sp.tile([P, MXC], F32, name="e_t", tag="e_t")
                nc.scalar.activation(e_t[:, 0:TC], q[:, 0:TC], AF.Prelu, alpha=NEG)
                nc.scalar.activation(e_t[:, 0:TC], e_t[:, 0:TC], AF.Exp)
                rhs = ep.tile([P, MXC * W], BF, name="rhs", tag="rhs")
                r3 = rhs[:].rearrange("p (c w) -> p c w", w=W)
                nc.vector.tensor_tensor(
                    out=r3[:, 0:TC, 0:D], in0=msg3[:, 0:TC, 0:D],
                    in1=e_t[:, 0:TC].unsqueeze(2).to_broadcast([P, TC, D]),
                    op=OP.mult)
                nc.vector.tensor_copy(out=r3[:, 0:TC, D:W],
                                      in_=e_t[:, 0:TC].unsqueeze(2))
                amms = []
                bmms = []
                cc = 0
                for w_, ncols, acols, bcol0, amax, bmax in calls:
                    if amax:
                        amms += [cc + c for c in range(acols)]
                    if bmax:
                        bmms += [cc + c for c in range(bcol0, ncols)]
                    cc += ncols
                aggb = sp.tile([P, P], BF, name="aggb", tag="aggb")
                for h, mms, qofs in ((0, amms, 0), (1, bmms, P)):
                    pseg = ps_seg.tile([P, W], F32, name="pseg", tag="pseg")
                    for i, c in enumerate(mms):
                        nc.tensor.matmul(
                            pseg[:],
                            s256[:, c * 2 * P + qofs:c * 2 * P + qofs + P],
                            rhs[:, c * W:(c + 1) * W],
                            start=(i == 0), stop=(i == len(mms) - 1))
                    sn = sp.tile([P, 1], F32, name="sn", tag="sn")
                    nc.vector.tensor_single_scalar(out=sn[:], in_=pseg[:, D:W],
                                                   scalar=1e-16, op=OP.add)
                    rcp = sp.tile([P, 1], F32, name="rcp", tag="rcp")
                    nc.vector.reciprocal(rcp[:], sn[:])
                    nc.vector.tensor_tensor(out=aggb[:, h * D:(h + 1) * D],
                                            in0=pseg[:, 0:D],
                                            in1=rcp[:].to_broadcast([P, D]),
                                            op=OP.mult)
                pT2 = ps.tile([P, P], F32, name="aggT", tag="miscP")
                nc.tensor.matmul(pT2[:], aggb[:], identb[:], start=True, stop=True)
                nc.vector.tensor_copy(out=HXb[:, tp * P:(tp + 1) * P], in_=pT2[:])
                col0 += TC

        # ================= forward =================
        for ci in range(NCH):
            c0, c1 = ci * 512, min((ci + 1) * 512, HC)
            xin = sp.tile([P, 512], BF, name="xin", tag="xin", bufs=1)
            nc.sync.dma_start(out=xin[:, :c1 - c0], in_=xfm_h[:, c0:c1])
            pt = ps_big.tile([P, 512], F32, name="mmp0", tag="mmp")
            nc.tensor.matmul(pt[:, 0:c1 - c0], Wt["lin1_bd"][:],
                             xin[:, 0:c1 - c0], start=True, stop=True)
            nc.scalar.activation(XC[:, c0:c1], pt[:, 0:c1 - c0],
                                 AF.Prelu, bias=Wt["lin1_b"][:, 0:1], alpha=NEG)
        nc.vector.tensor_copy(out=XCb[:], in_=XC[:])
        # --- GATEConv ---
        mm_node(HXb, "gate_w1a_bd", XCb)
        build_table(HXb, None, "gateattrRep2", True, 0)
        edge_phase(True, 0)
        mm_node(HXb, "gate_w2_bd", HXb)
        elu_inplace(HXb, "gate_bias")
        gru_relu("gru0_", nxt="at0_wT")
        # --- atom layers ---
        for l in range(4):
            pre = f"at{l}_"
            build_table(HXb, pre + "srcRep2", pre + "dstRep2", False, 1 + l)
            edge_phase(False, 1 + l)
            elu_inplace(HXb, pre + "bias")
            gru_relu(pre + "gru_",
                     nxt=(f"at{l + 1}_wT" if l < 3 else "mol_bd"))

        # ================= readout =================
        asrc_nm = st.tile([P, t_tiles], F32, name="asrc_nm")
        for tp in range(HT):
            pT = ps.tile([P, P], BF, name="xsT", tag="miscP")
            nc.tensor.transpose(out=pT[:], in_=HXb[:, tp * P:(tp + 1) * P],
                                identity=identb[:])
            nc.vector.tensor_copy(out=row_all[:, tp * 2 * D:(tp + 1) * 2 * D],
                                  in_=pT[:])
            tmp = sp.tile([P, P], F32, name="xsm", tag="tabm", bufs=1)
            nc.vector.tensor_tensor(out=tmp[:], in0=pT[:],
                                    in1=Wt["molsrcRep2"][:], op=OP.mult)
            nc.vector.tensor_reduce(asrc_nm[:, 2 * tp:2 * tp + 2].unsqueeze(2),
                                    tmp[:].rearrange("q (h f) -> q h f", f=D),
                                    axis=AX.X, op=OP.add)
        for tp in range(HT):
            pT = ps.tile([P, P], BF, name="xcT", tag="miscP")
            nc.tensor.transpose(out=pT[:], in_=XCb[:, tp * P:(tp + 1) * P],
                                identity=identb[:])
            nc.vector.tensor_copy(out=XCb[:, tp * P:(tp + 1) * P], in_=pT[:])
        xc_nm = XCb
        xs_nm = row_all
        OUTT = st.tile([D, 2 * P], F32, name="OUTT")
        for k in range(2):
            pg = ps_seg.tile([P, D], F32, name="pg", tag="pseg")
            for tp in range(HT):
                sgp = sp.tile([P, 2 * 2 * P], BF, name="sgp", tag="sgp")
                for h in range(2):
                    nc.vector.tensor_scalar(
                        out=sgp[:, h * 2 * P:(h + 1) * 2 * P], in0=iota_sb[:],
                        scalar1=grel32[:, 2 * tp + h:2 * tp + h + 1],
                        scalar2=None, op0=OP.is_equal)
                for h in range(2):
                    j = 2 * tp + h
                    nc.tensor.matmul(
                        pg[:],
                        sgp[:, h * 2 * P + k * P:h * 2 * P + (k + 1) * P],
                        xc_nm[:, j * D:(j + 1) * D],
                        start=(j == 0), stop=(j == t_tiles - 1))
            og = sp.tile([P, D], F32, name="og", tag="og")
            nc.scalar.activation(og[:], pg[:], AF.Relu)
            pTo = ps.tile([D, P], F32, name="ogT", tag="miscP")
            nc.tensor.transpose(out=pTo[:], in_=og[:], identity=ident[:])
            nc.vector.tensor_copy(out=OUTT[:, k * P:(k + 1) * P], in_=pTo[:])
        HG = st.tile([D, 2 * P], F32, name="HG")
        for ts in range(3):
            pxd = ps_big.tile([D, 512], F32, name="xdp", tag="mmp")
            nc.tensor.matmul(pxd[:, 0:2 * P], Wt["mol_wT32"][:], OUTT[:],
                             start=True, stop=True)
            xds = sp.tile([D, 2 * P], F32, name="xds", tag="xds", bufs=1)
            nc.vector.tensor_copy(out=xds[:], in_=pxd[:, 0:2 * P])
            pag = ps.tile([1, 2 * P], F32, name="agp", tag="rowP")
            nc.tensor.matmul(pag[:], Wt["moldstCol"][:], xds[:],
                             start=True, stop=True)
            agr = sp.tile([1, 2 * P], BF, name="agr", tag="agr")
            nc.vector.tensor_copy(out=agr[:], in_=pag[:])
            prg = ps.tile([P, 2 * P], F32, name="repG", tag="rowP")
            nc.tensor.matmul(prg[:], ones1[:], agr[:], start=True, stop=True)
            rep_agr = sp.tile([P, 2 * P], BF, name="rep_agr", tag="rep_agr", bufs=1)
            nc.vector.tensor_copy(out=rep_agr[:], in_=prg[:])
            ag2 = sp.tile([P, 2], F32, name="ag2", tag="ag2")
            psg = [ps_seg.tile([P, W], F32, name=f"psg{k}", tag="pseg")
                   for k in range(2)]
            for tp in range(HT):
                sgp = sp.tile([P, 2 * 2 * P], BF, name="sgp2", tag="sgp")
                for h in range(2):
                    nc.vector.tensor_scalar(
                        out=sgp[:, h * 2 * P:(h + 1) * 2 * P], in0=iota_sb[:],
                        scalar1=grel32[:, 2 * tp + h:2 * tp + h + 1],
                        scalar2=None, op0=OP.is_equal)
                scr2 = sp.tile([P, 2 * 2 * P], BF, name="scr2", tag="scr")
                g3 = scr2[:].rearrange("p (h q) -> p h q", q=2 * P)
                nc.vector.tensor_tensor(
                    out=g3[:],
                    in0=sgp[:].rearrange("p (h q) -> p h q", q=2 * P),
                    in1=rep_agr[:].unsqueeze(1).to_broadcast([P, 2, 2 * P]),
                    op=OP.mult)
                wdt = P
                while wdt >= 16:
                    nc.vector.tensor_tensor(
                        out=g3[:, :, 0:wdt], in0=g3[:, :, 0:wdt],
                        in1=g3[:, :, wdt:2 * wdt], op=OP.add)
                    wdt //= 2
                nc.vector.tensor_reduce(ag2[:].unsqueeze(2), g3[:, :, 0:16],
                                        axis=AX.X, op=OP.add)
                q2 = sp.tile([P, 2], F32, name="q2", tag="q2")
                nc.vector.tensor_tensor(out=q2[:], in0=asrc_nm[:, 2 * tp:2 * tp + 2],
                                        in1=ag2[:], op=OP.add)
                nc.scalar.activation(q2[:], q2[:], AF.Prelu, alpha=NEG)
                nc.scalar.activation(q2[:], q2[:], AF.Exp)
                rh = ep.tile([P, 2 * W], BF, name="rh", tag="rh", bufs=2)
                rh3 = rh[:].rearrange("p (h w) -> p h w", w=W)
                nc.vector.tensor_tensor(
                    out=rh3[:, :, 0:D],
                    in0=xs_nm[:, tp * 2 * D:(tp + 1) * 2 * D].rearrange(
                        "p (h f) -> p h f", f=D),
                    in1=q2[:].unsqueeze(2).to_broadcast([P, 2, D]),
                    op=OP.mult)
                nc.vector.tensor_copy(out=rh3[:, :, D:W], in_=q2[:].unsqueeze(2))
                for h in range(2):
                    j = 2 * tp + h
                    for k in range(2):
                        nc.tensor.matmul(
                            psg[k][:],
                            sgp[:, h * 2 * P + k * P:h * 2 * P + (k + 1) * P],
                            rh[:, h * W:(h + 1) * W],
                            start=(j == 0), stop=(j == t_tiles - 1))
            for k in range(2):
                sn = sp.tile([P, 1], F32, name="sng", tag="sn")
                nc.vector.tensor_single_scalar(out=sn[:], in_=psg[k][:, D:W],
                                               scalar=1e-16, op=OP.add)
                rcp = sp.tile([P, 1], F32, name="rcpg", tag="rcp")
                nc.vector.reciprocal(rcp[:], sn[:])
                aggg = sp.tile([P, D], F32, name="aggg", tag="aggg")
                nc.vector.tensor_tensor(out=aggg[:], in0=psg[k][:, 0:D],
                                        in1=rcp[:].to_broadcast([P, D]), op=OP.mult)
                nc.vector.tensor_tensor(out=aggg[:], in0=aggg[:],
                                        in1=Wt["mol_biasRep"][:], op=OP.add)
                r = sp.tile([P, D], F32, name="rg", tag="rg")
                nc.scalar.activation(r[:], aggg[:], AF.Relu)
                xm = sp.tile([P, D], F32, name="xmg", tag="xmg")
                nc.vector.tensor_sub(xm[:], aggg[:], r[:])
                nc.scalar.activation(xm[:], xm[:], AF.Exp)
                nc.vector.scalar_tensor_tensor(out=aggg[:], in0=xm[:], scalar=-1.0,
                                               in1=r[:], op0=OP.add, op1=OP.add)
                pTh = ps.tile([D, P], F32, name="hgT", tag="miscP")
                nc.tensor.transpose(out=pTh[:], in_=aggg[:], identity=ident[:])
                nc.vector.tensor_copy(out=HG[:, k * P:(k + 1) * P], in_=pTh[:])
            wih = Wt["mol_gru_wih"]
            whh = Wt["mol_gru_whh"]
            bs = Wt["mol_gru_bsum"]
            def gate2g(g):
                pt = ps_big.tile([D, 512], F32, name="ggp", tag="mmp")
                nc.tensor.matmul(pt[:, 0:2 * P], wih[:, g * D:(g + 1) * D], HG[:],
                                 start=True, stop=False)
                nc.tensor.matmul(pt[:, 0:2 * P], whh[:, g * D:(g + 1) * D], OUTT[:],
                                 start=False, stop=True)
                return pt
            prg2 = gate2g(0)
            rg2 = sp.tile([D, 2 * P], F32, name="ggr", tag="ggr", bufs=1)
            nc.scalar.activation(rg2[:], prg2[:, 0:2 * P], AF.Sigmoid, bias=bs[:, 0:1])
            pzg = gate2g(1)
            zg = sp.tile([D, 2 * P], F32, name="ggz", tag="ggz", bufs=1)
            nc.scalar.activation(zg[:], pzg[:, 0:2 * P], AF.Sigmoid, bias=bs[:, 1:2])
            pig = ps_big.tile([D, 512], F32, name="ggpi", tag="mmp")
            nc.tensor.matmul(pig[:, 0:2 * P], wih[:, 2 * D:3 * D], HG[:],
                             start=True, stop=True)
            phg = ps_big.tile([D, 512], F32, name="ggph", tag="mmp")
            nc.tensor.matmul(phg[:, 0:2 * P], whh[:, 2 * D:3 * D], OUTT[:],
                             start=True, stop=True)
            hng = sp.tile([D, 2 * P], F32, name="gghn", tag="gghn", bufs=1)
            nc.vector.tensor_scalar(out=hng[:], in0=phg[:, 0:2 * P],
                                    scalar1=Wt["mol_gru_bhh"][:, 2:3],
                                    scalar2=None, op0=OP.add)
            nc.vector.tensor_mul(hng[:], hng[:], rg2[:])
            nc.vector.tensor_tensor(out=hng[:], in0=hng[:], in1=pig[:, 0:2 * P],
                                    op=OP.add)
            ng = sp.tile([D, 2 * P], F32, name="ggn", tag="ggn", bufs=1)
            nc.scalar.activation(ng[:], hng[:], AF.Tanh,
                                 bias=Wt["mol_gru_bih"][:, 2:3])
            dg = sp.tile([D, 2 * P], F32, name="ggd", tag="ggd", bufs=1)
            nc.vector.tensor_sub(dg[:], OUTT[:], ng[:])
            nc.vector.tensor_mul(dg[:], dg[:], zg[:])
            nc.vector.tensor_tensor(out=dg[:], in0=dg[:], in1=ng[:], op=OP.add)
            nc.scalar.activation(OUTT[:], dg[:], AF.Relu)
        py = ps.tile([1, 2 * P], F32, name="py", tag="rowP")
        nc.tensor.matmul(py[:], Wt["lin2_wT"][:], OUTT[:], start=True, stop=True)
        ysb = sp.tile([1, 2 * P], F32, name="ysb", tag="ysb")
        nc.vector.tensor_scalar(out=ysb[:], in0=py[:], scalar1=Wt["lin2_b"][0:1, 0:1],
                                scalar2=None, op0=OP.add)
        nc.sync.dma_start(out=y_out[:], in_=ysb[0:1, 0:G_LOC])
    nc.compile()
    return nc


_CACHE = {}


def kernel(**inputs):
    from concourse.bass_utils import run_bass_kernel_spmd
    x = np.asarray(inputs["x"], np.float32)
    ei = np.asarray(inputs["edge_index"])
    ea = np.asarray(inputs["edge_attr"], np.float32)
    bt = np.asarray(inputs["batch"])
    per, n_pad, t_tiles, meta, TOTC = _prep(x, ei, ea, bt)
    kwf = {k: np.asarray(v, np.float32) for k, v in inputs.items()
           if k not in ("x", "edge_index", "edge_attr", "batch")}
    weights = _mk_weights(kwf)
    key = (n_pad, TOTC, tuple(tuple(tuple(cl) for cl in calls) for calls in meta[0]),
           tuple(meta[1]))
    if key not in _CACHE:
        _CACHE[key] = _build(n_pad, t_tiles, meta, TOTC,
                             {k: (v.shape, v.dtype == F16)
                              for k, v in weights.items()})
    nc = _CACHE[key]
    iota = np.tile(np.arange(2 * P).astype(np.float32)[None, :], (P, 1)).astype(F16)
    w1b = kwf["gate_lin1_w"][:, D:]
    in_maps = []
    for c in range(NC):
        b_e = (per[c]["attr_s"] @ w1b.T).astype(F16)
        b_e = np.ascontiguousarray(
            b_e.reshape(TOTC, P, D).transpose(1, 0, 2).reshape(P, TOTC * D))
        m = dict(xfm=per[c]["xfm"], idx16=per[c]["idx16"], s256=per[c]["s256"],
                 b_e=b_e, grel=per[c]["grel"], iota256=iota)
        for k, v in weights.items():
            m["w_" + k] = v
        in_maps.append(m)
    res = run_bass_kernel_spmd(nc, in_maps, core_ids=list(range(NC)))
    return np.concatenate([res.results[c]["y"][0] for c in range(NC)]).astype(np.float32)
